# revision 6
# baseline (speedup 1.0000x reference)
"""Trainium2 Bass kernel for nn_DefectDetection (GAT + pooling + LSTM head).

Self-contained: accepts FULL inputs, shards across 8 NeuronCores internally.

Strategy:
  Dispatch A (8 cores, SPMD):
    - replicated small front-end (node-attention layer, gpool1, GAT projections)
    - node-row-sharded dense [N,N] attention maps (64 rows x 16 heads / core),
      with the sparse node2node e3 term built from a host-packed slot grid via
      one matmul + gpsimd ap_gather (no 64MiB dense read)
    - edge-sharded edge-attr score reduction (es)
    - per-core outputs: es slice, gpool2 partials (P,Z), Wh2 rows, hs0
  Host in between: pure data movement (concat / scatter by precomputed indices).
  Dispatch B (1 core): pooled-graph attention (256 nodes), edge pool 2, gpool3,
    2-layer bi-LSTM (T=1) with bf16 weights, fc + softmax -> [2].
"""
import numpy as np
from contextlib import ExitStack

import concourse.bass as bass
import concourse.bacc as bacc
import concourse.tile as tile
import concourse.mybir as mybir
from concourse.bass_utils import run_bass_kernel_spmd

F32 = mybir.dt.float32
BF16 = mybir.dt.bfloat16
I16 = mybir.dt.int16
AF = mybir.ActivationFunctionType
ALU = mybir.AluOpType
AX = mybir.AxisListType

N, E, HID, NH, OUT, NCLS, LH = 512, 8192, 64, 16, 128, 2, 128
NC = 8          # cores
NPC = N // NC   # 64 nodes per core
S = 64          # slot grid per node
EPC = E // NC   # 1024 edges per core (F stage)
D1 = NH * OUT   # 2048
N2 = N // 2     # 256
N3 = N // 4     # 128
JUMP = HID + D1 + OUT  # 2240

_cache = {}


def _ap(t, offset, dims):
    return bass.AP(tensor=t, offset=offset, ap=[list(d) for d in dims])


# ---------------------------------------------------------------- dispatch A
def build_A():
    nc = bacc.Bacc("TRN2", target_bir_lowering=False, debug=False, num_devices=NC)

    def inp(name, shape, dt=F32):
        return nc.dram_tensor(name, shape, dt, kind="ExternalInput").ap()

    def outp(name, shape, dt=F32):
        return nc.dram_tensor(name, shape, dt, kind="ExternalOutput").ap()

    featT = inp("featT", [HID, N])
    featTm = inp("featTm", [HID, NPC])
    W_sn = inp("W_sn", [HID, HID])
    a_sn = inp("a_sn", [HID, 1])
    Wg1 = inp("Wg1", [HID, 1])
    bg1 = inp("bg1", [1, 1])
    Wgat = inp("Wgat", [NH, HID, OUT])
    WgatT = inp("WgatT", [NH, OUT, HID])
    a12 = inp("a12", [NH, OUT, 2])
    a3t128 = inp("a3t128", [HID, 128])
    XP = inp("XP", [HID, NPC * S])
    gidx = inp("gidx", [128, 256], I16)
    adjmine = inp("adjmine", [NPC, N])
    selrep = inp("selrep", [NPC, NC * 128])
    selh2 = inp("selh2", [NH, 128])
    eaT = inp("eaT", [HID, EPC])
    Wegat = inp("Wegat", [NH, HID, OUT])
    a3oT = inp("a3oT", [OUT, NH])
    wp1ab = inp("wp1ab", [NH, OUT, 2])
    bp1 = inp("bp1", [1, 1])
    Wg2r = inp("Wg2r", [NH, OUT, 1])
    bg2 = inp("bg2", [1, 1])
    Wor = inp("Wor", [NH, OUT, OUT])
    ident = inp("ident", [128, 128])

    o_es = outp("o_es", [1, EPC])
    o_P = outp("o_P", [OUT, NH])
    o_Z = outp("o_Z", [1, 1])
    o_Wh2T = outp("o_Wh2T", [OUT, N2 // NC])
    o_hs0 = outp("o_hs0", [HID, 1])

    with tile.TileContext(nc) as tc, ExitStack() as ctx:
        sb = ctx.enter_context(tc.tile_pool(name="sb", bufs=1))
        sb2 = ctx.enter_context(tc.tile_pool(name="sb2", bufs=2))
        sb3 = ctx.enter_context(tc.tile_pool(name="sb3", bufs=3))
        psa = ctx.enter_context(tc.tile_pool(name="psa", bufs=1, space="PSUM"))
        psb = ctx.enter_context(tc.tile_pool(name="psb", bufs=2, space="PSUM"))
        dram = ctx.enter_context(tc.tile_pool(name="dram", bufs=1, space="DRAM"))

        def load(apx, shape, dt=F32, pool=sb, tag=None):
            t = pool.tile(shape, dt, tag=tag)
            nc.sync.dma_start(t[:], apx)
            return t

        featT_s = load(featT[:], [HID, N], tag="featT")
        featTm_s = load(featTm[:], [HID, NPC], tag="featTm")
        Wsn_s = load(W_sn[:], [HID, HID], tag="Wsn")
        asn_s = load(a_sn[:], [HID, 1], tag="asn")
        Wg1_s = load(Wg1[:], [HID, 1], tag="Wg1")
        bg1_s = load(bg1[:], [1, 1], tag="bg1")
        ident_s = load(ident[:], [128, 128], tag="ident")
        a3t_s = load(a3t128[:], [HID, 128], tag="a3t")
        XP_s = load(XP[:], [HID, NPC * S], tag="XP")
        gidx_s = load(gidx[:], [128, 256], I16, tag="gidx")
        adjm_s = load(adjmine[:], [NPC, N], tag="adjm")
        selh2_s = load(selh2[:], [NH, 128], tag="selh2")
        eaT_s = load(eaT[:], [HID, EPC], tag="eaT")
        selrep_s = load(selrep[:], [NPC, NC * 128], tag="selrep")
        a3oT_s = load(a3oT[:], [OUT, NH], tag="a3oT")
        bp1_s = load(bp1[:], [1, 1], tag="bp1")
        bg2_s = load(bg2[:], [1, 1], tag="bg2")

        ones1_128 = sb.tile([1, 128], F32, tag="ones1")
        nc.gpsimd.memset(ones1_128[:], 1.0)
        ones128 = sb.tile([128, 1], F32, tag="ones128")
        nc.gpsimd.memset(ones128[:], 1.0)

        def elu_inplace(src_ps, dst_sb, shape, pool=sb2, tagp="elu"):
            """dst = elu(src) where src is PSUM [p,f]; dst SBUF."""
            p, f = shape
            ex = pool.tile([p, f], F32, tag=tagp + "_ex")
            nc.scalar.activation(ex[:], src_ps, AF.Exp)
            rl = pool.tile([p, f], F32, tag=tagp + "_rl")
            nc.scalar.activation(rl[:], src_ps, AF.Relu)
            # dst = (min(ex,1) + rl) - 1
            nc.vector.scalar_tensor_tensor(dst_sb, ex[:], 1.0, rl[:],
                                           op0=ALU.min, op1=ALU.add)
            nc.vector.tensor_scalar(dst_sb, dst_sb, 1.0, None, op0=ALU.subtract)

        # ---------------- front: h = elu(sigmoid(lrelu(Wh0@a))*Wh0)
        def front(ft, width, tag):
            wh0_ps = psb.tile([HID, width], F32, tag="mm")
            nc.tensor.matmul(wh0_ps[:], Wsn_s[:], ft, start=True, stop=True)
            wh0 = sb.tile([HID, width], F32, tag="wh0_" + tag)
            nc.scalar.copy(wh0[:], wh0_ps[:])
            ga_ps = psb.tile([1, width], F32, tag="mm")
            nc.tensor.matmul(ga_ps[:], asn_s[:], wh0[:], start=True, stop=True)
            gl = sb.tile([1, width], F32, tag="gl_" + tag)
            nc.scalar.activation(gl[:], ga_ps[:], AF.Lrelu, alpha=0.2)
            gs = sb.tile([1, width], F32, tag="gs_" + tag)
            nc.scalar.activation(gs[:], gl[:], AF.Sigmoid)
            grep_ps = psb.tile([HID, width], F32, tag="mm")
            nc.tensor.matmul(grep_ps[:], ones1_128[:, :HID], gs[:], start=True, stop=True)
            hpre = sb.tile([HID, width], F32, tag="hpre_" + tag)
            nc.vector.tensor_tensor(hpre[:], wh0[:], grep_ps[:], ALU.mult)
            ht = sb.tile([HID, width], F32, tag="ht_" + tag)
            elu_inplace(hpre[:], ht[:], [HID, width], tagp="eluf_" + tag)
            return ht

        hT = front(featT_s[:], N, "full")          # [64, 512]
        hTm = front(featTm_s[:], NPC, "mine")      # [64, 64]

        # ---------------- gpool1 -> hs0
        g1_ps = psb.tile([1, N], F32, tag="mm")
        nc.tensor.matmul(g1_ps[:], Wg1_s[:], hT[:], start=True, stop=True)
        g1s = sb.tile([1, N], F32, tag="g1s")
        nc.scalar.activation(g1s[:], g1_ps[:], AF.Sigmoid, bias=bg1_s[:])
        nmax1 = sb.tile([1, 1], F32, tag="nmax1")
        nc.vector.tensor_reduce(nmax1[:], g1s[:], AX.X, ALU.max, negate=True)
        w1 = sb.tile([1, N], F32, tag="w1")
        z1 = sb.tile([1, 1], F32, tag="z1")
        nc.scalar.activation(w1[:], g1s[:], AF.Exp, bias=nmax1[:], accum_out=z1[:])
        iz1 = sb.tile([1, 1], F32, tag="iz1")
        nc.vector.reciprocal(iz1[:], z1[:])
        nc.vector.tensor_scalar(w1[:], w1[:], iz1[:], None, op0=ALU.mult)
        w1rep_ps = psb.tile([HID, N], F32, tag="mm")
        nc.tensor.matmul(w1rep_ps[:], ones1_128[:, :HID], w1[:], start=True, stop=True)
        hw = sb.tile([HID, N], F32, tag="hw")
        nc.vector.tensor_tensor(hw[:], hT[:], w1rep_ps[:], ALU.mult)
        hs0 = sb.tile([HID, 1], F32, tag="hs0")
        nc.vector.tensor_reduce(hs0[:], hw[:], AX.X, ALU.add)
        nc.sync.dma_start(o_hs0[:], hs0[:])

        # ---------------- v12 = WgatT[h] @ a12[h]  -> vall [64, 32]
        vall = sb.tile([HID, 2 * NH], F32, tag="vall")
        for h in range(NH):
            wgT_s = sb2.tile([OUT, HID], F32, tag="wgT")
            nc.sync.dma_start(wgT_s[:], WgatT[h])
            a12_s = sb2.tile([OUT, 2], F32, tag="a12s")
            nc.sync.dma_start(a12_s[:], a12[h])
            v_ps = psb.tile([HID, 2], F32, tag="mm")
            nc.tensor.matmul(v_ps[:], wgT_s[:], a12_s[:], start=True, stop=True)
            nc.vector.tensor_copy(vall[:, 2 * h:2 * h + 2], v_ps[:])

        # s1mine [16, 64] / s2all [16, 512]
        v1_ap = _ap(vall[:].tensor, 0, [[2 * NH, HID], [2, NH]])
        v2_ap = _ap(vall[:].tensor, 1, [[2 * NH, HID], [2, NH]])
        s1m_ps = psb.tile([NH, NPC], F32, tag="mm")
        nc.tensor.matmul(s1m_ps[:], v1_ap, hTm[:], start=True, stop=True)
        s1m = sb.tile([NH, NPC], F32, tag="s1m")
        nc.vector.tensor_copy(s1m[:], s1m_ps[:])
        s2a_ps = psb.tile([NH, N], F32, tag="mm")
        nc.tensor.matmul(s2a_ps[:], v2_ap, hT[:], start=True, stop=True)
        s2a = sb.tile([NH, N], F32, tag="s2a")
        nc.vector.tensor_copy(s2a[:], s2a_ps[:])
        # s2rep [128, 512]: row p -> s2a[p%16]
        s2rep_ps = psa.tile([128, N], F32, tag="s2rep")
        nc.tensor.matmul(s2rep_ps[:], selh2_s[:], s2a[:], start=True, stop=True)
        s2rep = sb.tile([128, N], F32, tag="s2repsb")
        nc.vector.tensor_copy(s2rep[:], s2rep_ps[:])

        # s1col [128, 8] via DRAM bounce: scratch [16, 64]
        scr = dram.tile([NH, NPC], F32, tag="scr")
        nc.sync.dma_start(scr[:], s1m[:])
        s1col = sb.tile([128, NC], F32, tag="s1col")
        with nc.allow_non_contiguous_dma(reason="s1col 4B gather"):
            for i in range(8):
                src_ap = _ap(scr[:].tensor, i, [[NPC, NH], [8, 8]])
                nc.sync.dma_start(s1col[16 * i:16 * (i + 1), :], src_ap)

        # ---------------- sc = a3-scores on slot grid, replicated rows
        sc_sb = sb.tile([128, NPC * S + 1], F32, tag="scsb")
        for q in range(8):
            scq_ps = psb.tile([128, 512], F32, tag="mm")
            nc.tensor.matmul(scq_ps[:], a3t_s[:], XP_s[:, 512 * q:512 * (q + 1)],
                             start=True, stop=True)
            nc.vector.tensor_copy(sc_sb[:, 512 * q:512 * (q + 1)], scq_ps[:])
        nc.gpsimd.memset(sc_sb[:, NPC * S:NPC * S + 1], 0.0)

        # ---------------- F stage: es over my 1024 edges
        esA_ps = psa.tile([1, 512], F32, tag="accA")
        esB_ps = psa.tile([1, 512], F32, tag="accB")
        sumo_ps = psa.tile([1, 1], F32, tag="accC")
        es_ps = [esA_ps, esB_ps]
        for h in range(NH):
            weg_s = sb2.tile([HID, OUT], F32, tag="weg")
            nc.sync.dma_start(weg_s[:], Wegat[h])
            st, sp = (h == 0), (h == NH - 1)
            for half in range(2):
                T_ps = psb.tile([128, 512], F32, tag="mm")
                nc.tensor.matmul(T_ps[:], weg_s[:], eaT_s[:, 512 * half:512 * (half + 1)],
                                 start=True, stop=True)
                ex = sb2.tile([128, 512], F32, tag="Fex")
                nc.scalar.activation(ex[:], T_ps[:], AF.Exp)
                rl = sb2.tile([128, 512], F32, tag="Frl")
                nc.scalar.activation(rl[:], T_ps[:], AF.Relu)
                eluP = sb2.tile([128, 512], F32, tag="eluP")
                nc.vector.scalar_tensor_tensor(eluP[:], ex[:], 1.0, rl[:],
                                               op0=ALU.min, op1=ALU.add)
                nc.tensor.matmul(es_ps[half][:], a3oT_s[:, h:h + 1], eluP[:],
                                 start=st, stop=sp)
            nc.tensor.matmul(sumo_ps[:], a3oT_s[:, h:h + 1], ones128[:], start=st, stop=sp)
        sumo = sb.tile([1, 1], F32, tag="sumosb")
        nc.vector.tensor_copy(sumo[:], sumo_ps[:])
        es_sb = sb.tile([1, EPC], F32, tag="essb")
        nc.vector.tensor_scalar(es_sb[:, :512], esA_ps[:], sumo[:], None, op0=ALU.subtract)
        nc.vector.tensor_scalar(es_sb[:, 512:], esB_ps[:], sumo[:], None, op0=ALU.subtract)
        nc.sync.dma_start(o_es[:], es_sb[:])

        # ---------------- e-stage: 8 tiles [128 (i*16+h), 512]
        att_tiles = []
        for t in range(8):
            e3g = sb2.tile([128, N], F32, tag="e3g")
            nc.gpsimd.ap_gather(e3g[:], sc_sb[:], gidx_s[:, 32 * t:32 * (t + 1)],
                                channels=128, num_elems=NPC * S + 1, d=1, num_idxs=N)
            e1 = sb2.tile([128, N], F32, tag="e1")
            nc.vector.tensor_tensor(e1[:], e3g[:], s2rep[:], ALU.add)
            lr = sb2.tile([128, N], F32, tag="lr")
            nc.scalar.activation(lr[:], e1[:], AF.Lrelu, bias=s1col[:, t:t + 1], alpha=0.2)
            adjrep_ps = psb.tile([128, N], F32, tag="mm")
            nc.tensor.matmul(adjrep_ps[:], selrep_s[:, 128 * t:128 * (t + 1)], adjm_s[:], start=True, stop=True)
            m1 = sb2.tile([128, N], F32, tag="m1")
            nc.vector.scalar_tensor_tensor(m1[:], lr[:], 1e9, adjrep_ps[:],
                                           op0=ALU.add, op1=ALU.mult)
            nmax = sb2.tile([128, 1], F32, tag="nmax")
            nc.vector.tensor_reduce(nmax[:], m1[:], AX.X, ALU.max, negate=True)
            pt = sb2.tile([128, N], F32, tag="pt")
            zt = sb2.tile([128, 1], F32, tag="zt")
            nc.scalar.activation(pt[:], m1[:], AF.Exp, bias=nmax[:], accum_out=zt[:])
            izt = sb2.tile([128, 1], F32, tag="izt")
            nc.vector.reciprocal(izt[:], zt[:])
            att = sb.tile([128, N], F32, tag=f"att{t}")
            nc.vector.tensor_scalar(att[:], pt[:], izt[:], None, op0=ALU.mult)
            att_tiles.append(att)

        # transposes -> attT[jc] [128, 1024] cols = t*128 + (i*16+h)
        attT = []
        for jc in range(4):
            bigt = sb.tile([128, 1024], F32, tag=f"attT{jc}")
            attT.append(bigt)
        for t in range(8):
            for jc in range(4):
                tp_ps = psb.tile([128, 128], F32, tag="mm")
                nc.tensor.transpose(tp_ps[:], att_tiles[t][:, 128 * jc:128 * (jc + 1)],
                                    ident_s[:])
                nc.vector.tensor_copy(attT[jc][:, 128 * t:128 * (t + 1)], tp_ps[:])

        # AV per head + elu
        hGelu = []
        for h in range(NH):
            wg_s = sb2.tile([HID, OUT], F32, tag="wgnat")
            nc.sync.dma_start(wg_s[:], Wgat[h])
            hg_ps = psa.tile([OUT, NPC], F32, tag="hg")
            for jc in range(4):
                wh_ps = psb.tile([128, OUT], F32, tag="mm")
                nc.tensor.matmul(wh_ps[:], hT[:, 128 * jc:128 * (jc + 1)], wg_s[:],
                                 start=True, stop=True)
                wh_sb = sb2.tile([128, OUT], F32, tag="whsb")
                nc.vector.tensor_copy(wh_sb[:], wh_ps[:])
                rhs = _ap(attT[jc][:].tensor, h, [[1024, 128], [128, 8], [16, 8]])
                nc.tensor.matmul(hg_ps[:], wh_sb[:], rhs, start=(jc == 0), stop=(jc == 3))
            hg = sb.tile([OUT, NPC], F32, tag=f"hgelu{h}")
            elu_inplace(hg_ps[:], hg[:], [OUT, NPC], tagp="elug")
            hGelu.append(hg)

        # pair gates
        dpa_ps = psa.tile([1, NPC], F32, tag="accA")
        dpb_ps = psa.tile([1, NPC], F32, tag="accB")
        for h in range(NH):
            wp_s = sb2.tile([OUT, 2], F32, tag="wps")
            nc.sync.dma_start(wp_s[:], wp1ab[h])
            st, sp = (h == 0), (h == NH - 1)
            nc.tensor.matmul(dpa_ps[:], wp_s[:, 0:1], hGelu[h][:], start=st, stop=sp)
            nc.tensor.matmul(dpb_ps[:], wp_s[:, 1:2], hGelu[h][:], start=st, stop=sp)
        dk = sb.tile([1, NPC // 2], F32, tag="dk")
        dasb = sb.tile([1, NPC], F32, tag="dasb")
        nc.vector.tensor_copy(dasb[:], dpa_ps[:])
        a_ap = _ap(dasb[:].tensor, 0, [[NPC, 1], [2, NPC // 2]])
        b_ap = _ap(dpb_ps[:].tensor, 1, [[NPC, 1], [2, NPC // 2]])
        nc.vector.tensor_tensor(dk[:], a_ap, b_ap, ALU.add)
        sgate = sb.tile([1, NPC // 2], F32, tag="sgate")
        nc.scalar.activation(sgate[:], dk[:], AF.Sigmoid, bias=bp1_s[:])
        srep_ps = psa.tile([128, NPC // 2], F32, tag="accC")
        nc.tensor.matmul(srep_ps[:], ones1_128[:], sgate[:], start=True, stop=True)

        h1T = []
        for h in range(NH):
            ev_ap = _ap(hGelu[h][:].tensor, 0, [[NPC, OUT], [2, NPC // 2]])
            od_ap = _ap(hGelu[h][:].tensor, 1, [[NPC, OUT], [2, NPC // 2]])
            t1 = sb2.tile([OUT, NPC // 2], F32, tag="pairsum")
            nc.vector.tensor_tensor(t1[:], ev_ap, od_ap, ALU.add)
            h1 = sb.tile([OUT, NPC // 2], F32, tag=f"h1T{h}")
            nc.vector.tensor_tensor(h1[:], t1[:], srep_ps[:], ALU.mult)
            h1T.append(h1)

        # g2 / u / Z / P
        g2_ps = psa.tile([1, NPC // 2], F32, tag="accA")
        for h in range(NH):
            wg2_s = sb2.tile([OUT, 1], F32, tag="wg2s")
            nc.sync.dma_start(wg2_s[:], Wg2r[h])
            nc.tensor.matmul(g2_ps[:], wg2_s[:], h1T[h][:],
                             start=(h == 0), stop=(h == NH - 1))
        sg2 = sb.tile([1, NPC // 2], F32, tag="sg2")
        nc.scalar.activation(sg2[:], g2_ps[:], AF.Sigmoid, bias=bg2_s[:])
        u = sb.tile([1, NPC // 2], F32, tag="u")
        nc.scalar.activation(u[:], sg2[:], AF.Exp)
        Zc = sb.tile([1, 1], F32, tag="Zc")
        nc.vector.tensor_reduce(Zc[:], u[:], AX.X, ALU.add)
        nc.sync.dma_start(o_Z[:], Zc[:])
        urep_ps = psa.tile([128, NPC // 2], F32, tag="accB")
        nc.tensor.matmul(urep_ps[:], ones1_128[:], u[:], start=True, stop=True)
        Pout = sb.tile([OUT, NH], F32, tag="Pout")
        for h in range(NH):
            pm = sb2.tile([OUT, NPC // 2], F32, tag="pm")
            nc.vector.tensor_tensor(pm[:], h1T[h][:], urep_ps[:OUT, :], ALU.mult)
            nc.vector.tensor_reduce(Pout[:, h:h + 1], pm[:], AX.X, ALU.add)
        nc.sync.dma_start(o_P[:], Pout[:])

        # Wh2T rows
        wh2_ps = psa.tile([OUT, NPC // 2], F32, tag="accC")
        for h in range(NH):
            wo_s = sb2.tile([OUT, OUT], F32, tag="wos")
            nc.sync.dma_start(wo_s[:], Wor[h])
            nc.tensor.matmul(wh2_ps[:], wo_s[:], h1T[h][:],
                             start=(h == 0), stop=(h == NH - 1))
        wh2 = sb.tile([OUT, NPC // 2], F32, tag="wh2sb")
        nc.vector.tensor_copy(wh2[:], wh2_ps[:])
        nc.sync.dma_start(o_Wh2T[:], wh2[:])

    nc.compile()
    return nc


# ---------------------------------------------------------------- dispatch B
def build_B():
    nc = bacc.Bacc("TRN2", target_bir_lowering=False, debug=False, num_devices=1)

    def inp(name, shape, dt=F32):
        return nc.dram_tensor(name, shape, dt, kind="ExternalInput").ap()

    adjm2 = inp("adjm2", [N2, N2])
    e3_2 = inp("e3_2", [N2, N2])
    Wh2T = inp("Wh2T", [OUT, N2])
    Wh2nat = inp("Wh2nat", [N2, OUT])
    a12o = inp("a12o", [OUT, 2])
    wp2ab = inp("wp2ab", [OUT, 2])
    bp2 = inp("bp2", [1, 1])
    Wg3 = inp("Wg3", [OUT, 1])
    bg3 = inp("bg3", [1, 1])
    fcWr = inp("fcWr", [2, LH, NCLS])
    fcb = inp("fcb", [1, NCLS])
    Pall = inp("Pall", [OUT, NC * NH])
    Zall = inp("Zall", [1, NC])
    hs0 = inp("hs0", [HID, 1])
    W0b = inp("W0b", [2, 18, 128, 4 * LH], BF16)   # row-chunked lhsT, bias row folded
    W1b = inp("W1b", [2, 3, 128, 4 * LH], BF16)
    ident = inp("ident", [128, 128])
    o_prob = nc.dram_tensor("o_prob", [1, NCLS], F32, kind="ExternalOutput").ap()

    with tile.TileContext(nc) as tc, ExitStack() as ctx:
        sb = ctx.enter_context(tc.tile_pool(name="sb", bufs=1))
        sb2 = ctx.enter_context(tc.tile_pool(name="sb2", bufs=2))
        psa = ctx.enter_context(tc.tile_pool(name="psa", bufs=1, space="PSUM"))
        psb = ctx.enter_context(tc.tile_pool(name="psb", bufs=2, space="PSUM"))

        def load(apx, shape, dt=F32, pool=sb, tag=None):
            t = pool.tile(shape, dt, tag=tag)
            nc.sync.dma_start(t[:], apx)
            return t

        ident_s = load(ident[:], [128, 128], tag="ident")
        ones1 = sb.tile([1, 128], F32, tag="ones1")
        nc.gpsimd.memset(ones1[:], 1.0)
        Pall_s = load(Pall[:], [OUT, NC * NH], tag="Pall")
        Zall_s = load(Zall[:], [1, NC], tag="Zall")
        hs0_s = load(hs0[:], [HID, 1], tag="hs0")
        Wh2T_s = load(Wh2T[:], [OUT, N2], tag="Wh2T")
        a12o_s = load(a12o[:], [OUT, 2], tag="a12o")
        wp2_s = load(wp2ab[:], [OUT, 2], tag="wp2")
        bp2_s = load(bp2[:], [1, 1], tag="bp2")
        Wg3_s = load(Wg3[:], [OUT, 1], tag="Wg3")
        bg3_s = load(bg3[:], [1, 1], tag="bg3")
        fcb_s = load(fcb[:], [1, NCLS], tag="fcb")

        # hs1 columns [128, 16] = sum_c Pall[:, c*16+h] / Z
        hs1c = sb.tile([OUT, NH], F32, tag="hs1c")
        src = _ap(Pall_s[:].tensor, 0, [[NC * NH, OUT], [1, NH], [NH, NC]])
        nc.vector.tensor_reduce(hs1c[:], src, AX.X, ALU.add)
        Zt = sb.tile([1, 1], F32, tag="Zt")
        nc.vector.tensor_reduce(Zt[:], Zall_s[:], AX.X, ALU.add)
        iZ = sb.tile([1, 1], F32, tag="iZ")
        nc.vector.reciprocal(iZ[:], Zt[:])
        izrep_ps = psa.tile([128, 1], F32, tag="r1")
        nc.tensor.matmul(izrep_ps[:], ones1[:], iZ[:], start=True, stop=True)
        izcol = sb.tile([128, 1], F32, tag="izcol")
        nc.vector.tensor_copy(izcol[:], izrep_ps[:])
        nc.vector.tensor_scalar(hs1c[:], hs1c[:], izcol[:OUT, :], None, op0=ALU.mult)

        # att2 scores
        s1o_ps = psa.tile([1, N2], F32, tag="r2")
        nc.tensor.matmul(s1o_ps[:], a12o_s[:, 0:1], Wh2T_s[:], start=True, stop=True)
        s2o_ps = psa.tile([1, N2], F32, tag="r3")
        nc.tensor.matmul(s2o_ps[:], a12o_s[:, 1:2], Wh2T_s[:], start=True, stop=True)
        s1o = sb.tile([1, N2], F32, tag="s1osb")
        nc.vector.tensor_copy(s1o[:], s1o_ps[:])
        s2o = sb.tile([1, N2], F32, tag="s2osb")
        nc.vector.tensor_copy(s2o[:], s2o_ps[:])
        s2orep_ps = psa.tile([128, N2], F32, tag="r4")
        nc.tensor.matmul(s2orep_ps[:], ones1[:], s2o[:], start=True, stop=True)

        att2 = []
        for t2 in range(2):
            s1c_ps = psb.tile([128, 1], F32, tag="mmB")
            nc.tensor.transpose(s1c_ps[:], s1o[:, 128 * t2:128 * (t2 + 1)], ident_s[0:1, 0:1])
            s1c = sb2.tile([128, 1], F32, tag="s1c")
            nc.vector.tensor_copy(s1c[:], s1c_ps[:])
            e3t = sb2.tile([128, N2], F32, tag="e3t")
            nc.sync.dma_start(e3t[:], e3_2[128 * t2:128 * (t2 + 1), :])
            adt = sb2.tile([128, N2], F32, tag="adt")
            nc.sync.dma_start(adt[:], adjm2[128 * t2:128 * (t2 + 1), :])
            e1 = sb2.tile([128, N2], F32, tag="e1b")
            nc.vector.tensor_tensor(e1[:], e3t[:], s2orep_ps[:], ALU.add)
            lr = sb2.tile([128, N2], F32, tag="lrb")
            nc.scalar.activation(lr[:], e1[:], AF.Lrelu, bias=s1c[:], alpha=0.2)
            m1 = sb2.tile([128, N2], F32, tag="m1b")
            nc.vector.scalar_tensor_tensor(m1[:], lr[:], 1e9, adt[:],
                                           op0=ALU.add, op1=ALU.mult)
            nmax = sb2.tile([128, 1], F32, tag="nmaxb")
            nc.vector.tensor_reduce(nmax[:], m1[:], AX.X, ALU.max, negate=True)
            pt = sb2.tile([128, N2], F32, tag="ptb")
            zt = sb2.tile([128, 1], F32, tag="ztb")
            nc.scalar.activation(pt[:], m1[:], AF.Exp, bias=nmax[:], accum_out=zt[:])
            izt = sb2.tile([128, 1], F32, tag="iztb")
            nc.vector.reciprocal(izt[:], zt[:])
            at = sb.tile([128, N2], F32, tag=f"att2_{t2}")
            nc.vector.tensor_scalar(at[:], pt[:], izt[:], None, op0=ALU.mult)
            att2.append(at)

        # att2T + h2T
        attT2 = []
        for lc in range(2):
            big = sb.tile([128, N2], F32, tag=f"attT2_{lc}")
            attT2.append(big)
        for t2 in range(2):
            for lc in range(2):
                tp_ps = psb.tile([128, 128], F32, tag="mmB")
                nc.tensor.transpose(tp_ps[:], att2[t2][:, 128 * lc:128 * (lc + 1)],
                                    ident_s[:])
                nc.vector.tensor_copy(attT2[lc][:, 128 * t2:128 * (t2 + 1)], tp_ps[:])
        h2_ps = psa.tile([OUT, N2], F32, tag="r5")
        for lc in range(2):
            w2n_s = sb2.tile([128, OUT], F32, tag="w2n")
            nc.sync.dma_start(w2n_s[:], Wh2nat[128 * lc:128 * (lc + 1), :])
            nc.tensor.matmul(h2_ps[:], w2n_s[:], attT2[lc][:],
                             start=(lc == 0), stop=(lc == 1))
        h2T = sb.tile([OUT, N2], F32, tag="h2T")
        nc.vector.tensor_copy(h2T[:], h2_ps[:])

        # edge pool 2
        dpa_ps = psa.tile([1, N2], F32, tag="r1")
        nc.tensor.matmul(dpa_ps[:], wp2_s[:, 0:1], h2T[:], start=True, stop=True)
        dpb_ps = psa.tile([1, N2], F32, tag="r2")
        nc.tensor.matmul(dpb_ps[:], wp2_s[:, 1:2], h2T[:], start=True, stop=True)
        dk2 = sb.tile([1, N3], F32, tag="dk2")
        dasb2 = sb.tile([1, N2], F32, tag="dasb2")
        nc.vector.tensor_copy(dasb2[:], dpa_ps[:])
        a_ap = _ap(dasb2[:].tensor, 0, [[N2, 1], [2, N3]])
        b_ap = _ap(dpb_ps[:].tensor, 1, [[N2, 1], [2, N3]])
        nc.vector.tensor_tensor(dk2[:], a_ap, b_ap, ALU.add)
        s2k = sb.tile([1, N3], F32, tag="s2k")
        nc.scalar.activation(s2k[:], dk2[:], AF.Sigmoid, bias=bp2_s[:])
        srep2_ps = psa.tile([128, N3], F32, tag="r3")
        nc.tensor.matmul(srep2_ps[:], ones1[:], s2k[:], start=True, stop=True)
        ev_ap = _ap(h2T[:].tensor, 0, [[N2, OUT], [2, N3]])
        od_ap = _ap(h2T[:].tensor, 1, [[N2, OUT], [2, N3]])
        t12 = sb.tile([OUT, N3], F32, tag="t12")
        nc.vector.tensor_tensor(t12[:], ev_ap, od_ap, ALU.add)
        h3T = sb.tile([OUT, N3], F32, tag="h3T")
        nc.vector.tensor_tensor(h3T[:], t12[:], srep2_ps[:OUT, :], ALU.mult)

        # gpool3 -> hs2 [128, 1]
        g3_ps = psa.tile([1, N3], F32, tag="r1")
        nc.tensor.matmul(g3_ps[:], Wg3_s[:], h3T[:], start=True, stop=True)
        g3s = sb.tile([1, N3], F32, tag="g3s")
        nc.scalar.activation(g3s[:], g3_ps[:], AF.Sigmoid, bias=bg3_s[:])
        nm3 = sb.tile([1, 1], F32, tag="nm3")
        nc.vector.tensor_reduce(nm3[:], g3s[:], AX.X, ALU.max, negate=True)
        w3 = sb.tile([1, N3], F32, tag="w3")
        z3 = sb.tile([1, 1], F32, tag="z3")
        nc.scalar.activation(w3[:], g3s[:], AF.Exp, bias=nm3[:], accum_out=z3[:])
        iz3 = sb.tile([1, 1], F32, tag="iz3")
        nc.vector.reciprocal(iz3[:], z3[:])
        nc.vector.tensor_scalar(w3[:], w3[:], iz3[:], None, op0=ALU.mult)
        w3rep_ps = psa.tile([128, N3], F32, tag="r2")
        nc.tensor.matmul(w3rep_ps[:], ones1[:], w3[:], start=True, stop=True)
        hw3 = sb.tile([OUT, N3], F32, tag="hw3")
        nc.vector.tensor_tensor(hw3[:], h3T[:], w3rep_ps[:OUT, :], ALU.mult)
        hs2 = sb.tile([OUT, 1], F32, tag="hs2")
        nc.vector.tensor_reduce(hs2[:], hw3[:], AX.X, ALU.add)

        # x chunks [128, 18] bf16: cols 0-15 hs1c, col16 [hs0; hs2[0:64]], col17 [hs2[64:]; 1]
        xc = sb.tile([128, 18], F32, tag="xc")
        nc.gpsimd.memset(xc[:], 0.0)
        nc.vector.tensor_copy(xc[:OUT, 0:NH], hs1c[:])
        nc.vector.tensor_copy(xc[:HID, 16:17], hs0_s[:])
        nc.sync.dma_start(xc[HID:128, 16:17], hs2[0:HID, :])
        nc.sync.dma_start(xc[0:HID, 17:18], hs2[HID:OUT, :])
        nc.gpsimd.memset(xc[HID:HID + 1, 17:18], 1.0)
        xcb = sb.tile([128, 18], BF16, tag="xcb")
        nc.vector.tensor_copy(xcb[:], xc[:])

        # LSTM layer 0 (M-orientation, skip f-gate m=1)
        h0 = []
        for d in range(2):
            g_ps = psa.tile([128, 4], F32, tag="gacc")
            for m in (0, 2, 3):
                for k in range(18):
                    rows = 65 if k == 17 else 128
                    w_s = sb2.tile([128, 128], BF16, tag="w0s")
                    nc.sync.dma_start(w_s[:rows, :], W0b[d, k, 0:rows, 128 * m:128 * (m + 1)])
                    nc.tensor.matmul(g_ps[:, m:m + 1], w_s[:rows, :], xcb[:rows, k:k + 1],
                                     start=(k == 0), stop=(k == 17))
            si = sb2.tile([128, 1], F32, tag="si")
            nc.scalar.activation(si[:], g_ps[:, 0:1], AF.Sigmoid)
            tg = sb2.tile([128, 1], F32, tag="tg")
            nc.scalar.activation(tg[:], g_ps[:, 2:3], AF.Tanh)
            so = sb2.tile([128, 1], F32, tag="so")
            nc.scalar.activation(so[:], g_ps[:, 3:4], AF.Sigmoid)
            c = sb2.tile([128, 1], F32, tag="c0")
            nc.vector.tensor_tensor(c[:], si[:], tg[:], ALU.mult)
            tc_ = sb2.tile([128, 1], F32, tag="tc0")
            nc.scalar.activation(tc_[:], c[:], AF.Tanh)
            hd = sb.tile([128, 1], F32, tag=f"h0_{d}")
            nc.vector.tensor_tensor(hd[:], so[:], tc_[:], ALU.mult)
            h0.append(hd)
        h0b_ = []
        for d in range(2):
            hb = sb.tile([128, 1], BF16, tag=f"h0b_{d}")
            nc.vector.tensor_copy(hb[:], h0[d][:])
            h0b_.append(hb)
        onesb = sb.tile([1, 1], BF16, tag="onesb")
        nc.gpsimd.memset(onesb[:], 1.0)

        # LSTM layer 1
        h1o = []
        for d in range(2):
            g_ps = psa.tile([128, 4], F32, tag="gacc")
            for m in (0, 2, 3):
                for k in range(3):
                    rows = 1 if k == 2 else 128
                    w_s = sb2.tile([128, 128], BF16, tag="w1s")
                    nc.sync.dma_start(w_s[:rows, :], W1b[d, k, 0:rows, 128 * m:128 * (m + 1)])
                    rhs = onesb[:] if k == 2 else h0b_[k][:]
                    nc.tensor.matmul(g_ps[:, m:m + 1], w_s[:rows, :], rhs,
                                     start=(k == 0), stop=(k == 2))
            si = sb2.tile([128, 1], F32, tag="si1")
            nc.scalar.activation(si[:], g_ps[:, 0:1], AF.Sigmoid)
            tg = sb2.tile([128, 1], F32, tag="tg1")
            nc.scalar.activation(tg[:], g_ps[:, 2:3], AF.Tanh)
            so = sb2.tile([128, 1], F32, tag="so1")
            nc.scalar.activation(so[:], g_ps[:, 3:4], AF.Sigmoid)
            c = sb2.tile([128, 1], F32, tag="c1")
            nc.vector.tensor_tensor(c[:], si[:], tg[:], ALU.mult)
            tc_ = sb2.tile([128, 1], F32, tag="tc1")
            nc.scalar.activation(tc_[:], c[:], AF.Tanh)
            hd = sb.tile([128, 1], F32, tag=f"h1_{d}")
            nc.vector.tensor_tensor(hd[:], so[:], tc_[:], ALU.mult)
            h1o.append(hd)

        # fc + softmax
        lg_ps = psa.tile([1, NCLS], F32, tag="r1")
        fcw0 = sb.tile([LH, NCLS], F32, tag="fcw0")
        nc.sync.dma_start(fcw0[:], fcWr[0])
        fcw1 = sb.tile([LH, NCLS], F32, tag="fcw1")
        nc.sync.dma_start(fcw1[:], fcWr[1])
        nc.tensor.matmul(lg_ps[:], h1o[0][:], fcw0[:], start=True, stop=False)
        nc.tensor.matmul(lg_ps[:], h1o[1][:], fcw1[:], start=False, stop=True)
        lg = sb.tile([1, NCLS], F32, tag="lg")
        nc.vector.tensor_tensor(lg[:], lg_ps[:], fcb_s[:], ALU.add)
        nmf = sb.tile([1, 1], F32, tag="nmf")
        nc.vector.tensor_reduce(nmf[:], lg[:], AX.X, ALU.max, negate=True)
        pf = sb.tile([1, NCLS], F32, tag="pf")
        zf = sb.tile([1, 1], F32, tag="zf")
        nc.scalar.activation(pf[:], lg[:], AF.Exp, bias=nmf[:], accum_out=zf[:])
        izf = sb.tile([1, 1], F32, tag="izf")
        nc.vector.reciprocal(izf[:], zf[:])
        prob = sb.tile([1, NCLS], F32, tag="prob")
        nc.vector.tensor_scalar(prob[:], pf[:], izf[:], None, op0=ALU.mult)
        nc.sync.dma_start(o_prob[:], prob[:])

    nc.compile()
    return nc


# ---------------------------------------------------------------- host prep
def _prep_A(inputs):
    """Build per-core input maps for dispatch A. Pure layout/indexing."""
    f32 = np.float32
    ei = np.asarray(inputs["edge_index"])
    feats = np.asarray(inputs["features"], f32)
    n2n = np.asarray(inputs["node2node_features"], f32)
    eattr = np.asarray(inputs["edgesAttr"], f32)
    adjacency = np.asarray(inputs["adjacency"], f32)

    src, dst = np.asarray(ei[0], np.int64), np.asarray(ei[1], np.int64)
    pairs = src * N + dst
    uniq = np.unique(pairs)
    us, ud = uniq // N, uniq % N
    # slot assignment per source node
    order = np.argsort(us, kind="stable")
    us, ud, uniq = us[order], ud[order], uniq[order]
    counts = np.bincount(us, minlength=N)
    assert counts.max() <= S, f"out-degree {counts.max()} > {S}"
    starts = np.zeros(N + 1, np.int64)
    np.cumsum(counts, out=starts[1:])
    slots = np.arange(len(us)) - starts[us]

    featT = np.ascontiguousarray(feats.T)
    eaT = np.ascontiguousarray(eattr.T)
    W_gat = np.asarray(inputs["W_gat"], f32)
    shared = {
        "featT": featT,
        "W_sn": np.asarray(inputs["W_sn"], f32),
        "a_sn": np.asarray(inputs["a_sn"], f32).reshape(HID, 1),
        "Wg1": np.asarray(inputs["Wg1"], f32).reshape(HID, 1),
        "bg1": np.asarray(inputs["bg1"], f32).reshape(1, 1),
        "Wgat": W_gat,
        "WgatT": np.ascontiguousarray(W_gat.transpose(0, 2, 1)),
        "a12": np.ascontiguousarray(np.stack(
            [np.asarray(inputs["a1_gat"], f32), np.asarray(inputs["a2_gat"], f32)], -1)),
        "a3t128": np.ascontiguousarray(np.tile(np.asarray(inputs["a3_gat"], f32).T, (1, 8))),
        "selh2": np.eye(NH, dtype=f32)[:, np.tile(np.arange(NH), 8)].reshape(NH, 128),
        "Wegat": np.asarray(inputs["We_gat"], f32),
        "a3oT": np.ascontiguousarray(np.asarray(inputs["a3_o"], f32).reshape(NH, OUT).T),
        "wp1ab": np.ascontiguousarray(np.stack([
            np.asarray(inputs["Wp1"], f32)[:D1, 0].reshape(NH, OUT),
            np.asarray(inputs["Wp1"], f32)[D1:, 0].reshape(NH, OUT)], -1)),
        "bp1": np.asarray(inputs["bp1"], f32).reshape(1, 1),
        "Wg2r": np.asarray(inputs["Wg2"], f32).reshape(NH, OUT, 1),
        "bg2": np.asarray(inputs["bg2"], f32).reshape(1, 1),
        "Wor": np.asarray(inputs["Wo"], f32).reshape(NH, OUT, OUT),
        "ident": np.eye(128, dtype=f32),
    }
    # selrep[t][r, p] = 1 iff r == 8t + p//16
    selrep = np.zeros((NPC, NC * 128), f32)
    for t in range(8):
        for p in range(128):
            selrep[8 * t + p // 16, 128 * t + p] = 1.0
    shared["selrep"] = selrep

    in_maps = []
    for c in range(NC):
        lo = c * NPC
        m = dict(shared)
        m["featTm"] = np.ascontiguousarray(featT[:, lo:lo + NPC])
        # slot grid XP [64, NPC*S] and gather idx
        mask = (us >= lo) & (us < lo + NPC)
        cs, cd, csl = us[mask] - lo, ud[mask], slots[mask]
        XP = np.zeros((NPC * S, HID), f32)
        XP[cs * S + csl] = n2n[uniq[mask]]
        m["XP"] = np.ascontiguousarray(XP.T)
        ptr = np.full((NPC, N), NPC * S, np.int64)
        ptr[cs, cd] = cs * S + csl
        g = np.zeros((128, 256), np.int16)
        for t in range(8):
            for gg in range(8):
                node = 8 * t + gg
                row = ptr[node]                      # [512]
                g[16 * gg:16 * gg + 16, 32 * t:32 * t + 32] = \
                    row.reshape(32, 16).T.astype(np.int16)
        m["gidx"] = g
        m["adjmine"] = np.ascontiguousarray(adjacency[lo:lo + NPC])
        m["eaT"] = np.ascontiguousarray(eaT[:, c * EPC:(c + 1) * EPC])
        in_maps.append(m)
    return in_maps, (src, dst)


def _prep_B(inputs, resA, ei_sd):
    f32 = np.float32
    src, dst = ei_sd
    es = np.concatenate([resA[c]["o_es"].reshape(-1) for c in range(NC)])
    s2, d2 = src // 2, dst // 2
    adj2 = np.zeros((N2, N2), f32)
    adj2[s2, d2] = 1.0
    e3_2 = np.zeros((N2, N2), f32)
    e3_2[s2, d2] = es  # numpy fancy assignment: last occurrence wins
    Wh2T = np.concatenate([resA[c]["o_Wh2T"] for c in range(NC)], axis=1)
    Pall = np.concatenate([resA[c]["o_P"] for c in range(NC)], axis=1)
    Zall = np.concatenate([resA[c]["o_Z"].reshape(1, 1) for c in range(NC)], axis=1)

    # LSTM weights: my-x order = [hs1(2048), hs0(64), hs2(128), bias(1)]
    perm = np.concatenate([np.arange(64, 2112), np.arange(0, 64), np.arange(2112, 2240)])
    W0 = np.zeros((2, 18, 128, 4 * LH), f32)
    for d in range(2):
        wt = np.asarray(inputs["Wih0"], f32)[d].T[perm]         # [2240, 512]
        wb = np.concatenate([wt, np.asarray(inputs["b0"], f32)[d][None, :]], 0)  # [2241,512]
        for k in range(18):
            rows = wb[128 * k:128 * (k + 1)]
            W0[d, k, :rows.shape[0], :] = rows
    W1 = np.zeros((2, 3, 128, 4 * LH), f32)
    for d in range(2):
        wt = np.asarray(inputs["Wih1"], f32)[d].T               # [256, 512]
        wb = np.concatenate([wt, np.asarray(inputs["b1"], f32)[d][None, :]], 0)
        for k in range(3):
            rows = wb[128 * k:128 * (k + 1)]
            W1[d, k, :rows.shape[0], :] = rows
    import ml_dtypes
    bf = ml_dtypes.bfloat16

    return {
        "adjm2": adj2,
        "e3_2": e3_2,
        "Wh2T": np.ascontiguousarray(Wh2T),
        "Wh2nat": np.ascontiguousarray(Wh2T.T),
        "a12o": np.ascontiguousarray(np.stack(
            [np.asarray(inputs["a1_o"], f32), np.asarray(inputs["a2_o"], f32)], -1)),
        "wp2ab": np.ascontiguousarray(np.stack(
            [np.asarray(inputs["Wp2"], f32)[:OUT, 0], np.asarray(inputs["Wp2"], f32)[OUT:, 0]], -1)),
        "bp2": np.asarray(inputs["bp2"], f32).reshape(1, 1),
        "Wg3": np.asarray(inputs["Wg3"], f32).reshape(OUT, 1),
        "bg3": np.asarray(inputs["bg3"], f32).reshape(1, 1),
        "fcWr": np.asarray(inputs["fc_W"], f32).reshape(2, LH, NCLS, order="C")
                  if False else np.stack([np.asarray(inputs["fc_W"], f32)[:LH],
                                          np.asarray(inputs["fc_W"], f32)[LH:]]),
        "fcb": np.asarray(inputs["fc_b"], f32).reshape(1, NCLS),
        "Pall": np.ascontiguousarray(Pall),
        "Zall": np.ascontiguousarray(Zall),
        "hs0": resA[0]["o_hs0"].reshape(HID, 1),
        "W0b": W0.astype(bf),
        "W1b": W1.astype(bf),
        "ident": np.eye(128, dtype=f32),
    }


# ---------------------------------------------------------------- entrypoint
def kernel(**inputs):
    if "A" not in _cache:
        _cache["A"] = build_A()
    if "B" not in _cache:
        _cache["B"] = build_B()
    in_maps, ei_sd = _prep_A(inputs)
    resA = run_bass_kernel_spmd(_cache["A"], in_maps, core_ids=list(range(NC))).results
    inB = _prep_B(inputs, resA, ei_sd)
    resB = run_bass_kernel_spmd(_cache["B"], [inB], core_ids=[0]).results
    return resB[0]["o_prob"].reshape(NCLS).astype(np.float32)


# revision 8
# speedup vs baseline: 1.4188x; 1.4188x over previous
"""Trainium2 Bass kernel for nn_DefectDetection (GAT + pooling + LSTM head).

Self-contained: accepts FULL inputs, shards across 8 NeuronCores internally.

Strategy:
  Dispatch A (8 cores, SPMD):
    - replicated small front-end (node-attention layer, gpool1, GAT projections)
    - node-row-sharded dense [N,N] attention maps (64 rows x 16 heads / core),
      with the sparse node2node e3 term built from a host-packed slot grid via
      one matmul + gpsimd ap_gather (no 64MiB dense read)
    - edge-sharded edge-attr score reduction (es)
    - per-core outputs: es slice, gpool2 partials (P,Z), Wh2 rows, hs0
  Host in between: pure data movement (concat / scatter by precomputed indices).
  Dispatch B (1 core): pooled-graph attention (256 nodes), edge pool 2, gpool3,
    2-layer bi-LSTM (T=1) with bf16 weights, fc + softmax -> [2].
"""
import numpy as np
from contextlib import ExitStack

import concourse.bass as bass
import concourse.bacc as bacc
import concourse.tile as tile
import concourse.mybir as mybir
from concourse.bass_utils import run_bass_kernel_spmd

F32 = mybir.dt.float32
BF16 = mybir.dt.bfloat16
I16 = mybir.dt.int16
AF = mybir.ActivationFunctionType
ALU = mybir.AluOpType
AX = mybir.AxisListType

N, E, HID, NH, OUT, NCLS, LH = 512, 8192, 64, 16, 128, 2, 128
NC = 8          # cores
NPC = N // NC   # 64 nodes per core
S = 64          # slot grid per node
EPC = E // NC   # 1024 edges per core (F stage)
D1 = NH * OUT   # 2048
N2 = N // 2     # 256
N3 = N // 4     # 128
JUMP = HID + D1 + OUT  # 2240

_cache = {}


def _ap(t, offset, dims):
    return bass.AP(tensor=t, offset=offset, ap=[list(d) for d in dims])


# ---------------------------------------------------------------- dispatch A
def build_A():
    nc = bacc.Bacc("TRN2", target_bir_lowering=False, debug=False, num_devices=NC)

    def inp(name, shape, dt=F32):
        return nc.dram_tensor(name, shape, dt, kind="ExternalInput").ap()

    def outp(name, shape, dt=F32):
        return nc.dram_tensor(name, shape, dt, kind="ExternalOutput").ap()

    featT = inp("featT", [HID, N])
    featTm = inp("featTm", [HID, NPC])
    W_sn = inp("W_sn", [HID, HID])
    a_sn = inp("a_sn", [HID, 1])
    Wg1 = inp("Wg1", [HID, 1])
    bg1 = inp("bg1", [1, 1])
    Wgat = inp("Wgat", [NH, HID, OUT])
    WgatT = inp("WgatT", [NH, OUT, HID])
    a12 = inp("a12", [NH, OUT, 2])
    a3t128 = inp("a3t128", [HID, 128])
    XP = inp("XP", [HID, NPC * S])
    gidx = inp("gidx", [128, 256], I16)
    adjmine = inp("adjmine", [NPC, N])
    selrep = inp("selrep", [NPC, NC * 128])
    selh2 = inp("selh2", [NH, 128])
    eaT = inp("eaT", [HID, EPC])
    Wegat = inp("Wegat", [NH, HID, OUT])
    a3oT = inp("a3oT", [OUT, NH])
    wp1ab = inp("wp1ab", [NH, OUT, 2])
    bp1 = inp("bp1", [1, 1])
    Wg2r = inp("Wg2r", [NH, OUT, 1])
    bg2 = inp("bg2", [1, 1])
    Wor = inp("Wor", [NH, OUT, OUT])
    ident = inp("ident", [128, 128])

    o_es = outp("o_es", [1, EPC])
    o_P = outp("o_P", [OUT, NH])
    o_Z = outp("o_Z", [1, 1])
    o_Wh2T = outp("o_Wh2T", [OUT, N2 // NC])
    o_hs0 = outp("o_hs0", [HID, 1])

    with tile.TileContext(nc) as tc, ExitStack() as ctx:
        sb = ctx.enter_context(tc.tile_pool(name="sb", bufs=1))
        sb2 = ctx.enter_context(tc.tile_pool(name="sb2", bufs=2))
        sb3 = ctx.enter_context(tc.tile_pool(name="sb3", bufs=3))
        psa = ctx.enter_context(tc.tile_pool(name="psa", bufs=1, space="PSUM"))
        psb = ctx.enter_context(tc.tile_pool(name="psb", bufs=2, space="PSUM"))
        dram = ctx.enter_context(tc.tile_pool(name="dram", bufs=1, space="DRAM"))

        def load(apx, shape, dt=F32, pool=sb, tag=None):
            t = pool.tile(shape, dt, tag=tag)
            nc.sync.dma_start(t[:], apx)
            return t

        featT_s = load(featT[:], [HID, N], tag="featT")
        featTm_s = load(featTm[:], [HID, NPC], tag="featTm")
        Wsn_s = load(W_sn[:], [HID, HID], tag="Wsn")
        asn_s = load(a_sn[:], [HID, 1], tag="asn")
        Wg1_s = load(Wg1[:], [HID, 1], tag="Wg1")
        bg1_s = load(bg1[:], [1, 1], tag="bg1")
        ident_s = load(ident[:], [128, 128], tag="ident")
        a3t_s = load(a3t128[:], [HID, 128], tag="a3t")
        XP_s = load(XP[:], [HID, NPC * S], tag="XP")
        gidx_s = load(gidx[:], [128, 256], I16, tag="gidx")
        adjm_s = load(adjmine[:], [NPC, N], tag="adjm")
        selh2_s = load(selh2[:], [NH, 128], tag="selh2")
        eaT_s = load(eaT[:], [HID, EPC], tag="eaT")
        selrep_s = load(selrep[:], [NPC, NC * 128], tag="selrep")
        a3oT_s = load(a3oT[:], [OUT, NH], tag="a3oT")
        bp1_s = load(bp1[:], [1, 1], tag="bp1")
        bg2_s = load(bg2[:], [1, 1], tag="bg2")

        ones1_128 = sb.tile([1, 128], F32, tag="ones1")
        nc.gpsimd.memset(ones1_128[:], 1.0)
        ones128 = sb.tile([128, 1], F32, tag="ones128")
        nc.gpsimd.memset(ones128[:], 1.0)

        def elu_inplace(src_ps, dst_sb, shape, pool=sb2, tagp="elu"):
            """dst = elu(src) where src is PSUM [p,f]; dst SBUF."""
            p, f = shape
            ex = pool.tile([p, f], F32, tag=tagp + "_ex")
            nc.scalar.activation(ex[:], src_ps, AF.Exp)
            rl = pool.tile([p, f], F32, tag=tagp + "_rl")
            nc.scalar.activation(rl[:], src_ps, AF.Relu)
            # dst = (min(ex,1) + rl) - 1
            nc.vector.scalar_tensor_tensor(dst_sb, ex[:], 1.0, rl[:],
                                           op0=ALU.min, op1=ALU.add)
            nc.vector.tensor_scalar(dst_sb, dst_sb, 1.0, None, op0=ALU.subtract)

        # ---------------- front: h = elu(sigmoid(lrelu(Wh0@a))*Wh0)
        def front(ft, width, tag):
            wh0_ps = psb.tile([HID, width], F32, tag="mm")
            nc.tensor.matmul(wh0_ps[:], Wsn_s[:], ft, start=True, stop=True)
            wh0 = sb.tile([HID, width], F32, tag="wh0_" + tag)
            nc.scalar.copy(wh0[:], wh0_ps[:])
            ga_ps = psb.tile([1, width], F32, tag="mm")
            nc.tensor.matmul(ga_ps[:], asn_s[:], wh0[:], start=True, stop=True)
            gl = sb.tile([1, width], F32, tag="gl_" + tag)
            nc.scalar.activation(gl[:], ga_ps[:], AF.Lrelu, alpha=0.2)
            gs = sb.tile([1, width], F32, tag="gs_" + tag)
            nc.scalar.activation(gs[:], gl[:], AF.Sigmoid)
            grep_ps = psb.tile([HID, width], F32, tag="mm")
            nc.tensor.matmul(grep_ps[:], ones1_128[:, :HID], gs[:], start=True, stop=True)
            hpre = sb.tile([HID, width], F32, tag="hpre_" + tag)
            nc.vector.tensor_tensor(hpre[:], wh0[:], grep_ps[:], ALU.mult)
            ht = sb.tile([HID, width], F32, tag="ht_" + tag)
            elu_inplace(hpre[:], ht[:], [HID, width], tagp="eluf_" + tag)
            return ht

        hT = front(featT_s[:], N, "full")          # [64, 512]
        hTm = front(featTm_s[:], NPC, "mine")      # [64, 64]

        # ---------------- gpool1 -> hs0
        g1_ps = psb.tile([1, N], F32, tag="mm")
        nc.tensor.matmul(g1_ps[:], Wg1_s[:], hT[:], start=True, stop=True)
        g1s = sb.tile([1, N], F32, tag="g1s")
        nc.scalar.activation(g1s[:], g1_ps[:], AF.Sigmoid, bias=bg1_s[:])
        nmax1 = sb.tile([1, 1], F32, tag="nmax1")
        nc.vector.tensor_reduce(nmax1[:], g1s[:], AX.X, ALU.max, negate=True)
        w1 = sb.tile([1, N], F32, tag="w1")
        z1 = sb.tile([1, 1], F32, tag="z1")
        nc.scalar.activation(w1[:], g1s[:], AF.Exp, bias=nmax1[:], accum_out=z1[:])
        iz1 = sb.tile([1, 1], F32, tag="iz1")
        nc.vector.reciprocal(iz1[:], z1[:])
        nc.vector.tensor_scalar(w1[:], w1[:], iz1[:], None, op0=ALU.mult)
        w1rep_ps = psb.tile([HID, N], F32, tag="mm")
        nc.tensor.matmul(w1rep_ps[:], ones1_128[:, :HID], w1[:], start=True, stop=True)
        hw = sb.tile([HID, N], F32, tag="hw")
        nc.vector.tensor_tensor(hw[:], hT[:], w1rep_ps[:], ALU.mult)
        hs0 = sb.tile([HID, 1], F32, tag="hs0")
        nc.vector.tensor_reduce(hs0[:], hw[:], AX.X, ALU.add)
        nc.sync.dma_start(o_hs0[:], hs0[:])

        # ---------------- v12 = WgatT[h] @ a12[h]  -> vall [64, 32]
        vall = sb.tile([HID, 2 * NH], F32, tag="vall")
        for h in range(NH):
            wgT_s = sb2.tile([OUT, HID], F32, tag="wgT")
            nc.sync.dma_start(wgT_s[:], WgatT[h])
            a12_s = sb2.tile([OUT, 2], F32, tag="a12s")
            nc.sync.dma_start(a12_s[:], a12[h])
            v_ps = psb.tile([HID, 2], F32, tag="mm")
            nc.tensor.matmul(v_ps[:], wgT_s[:], a12_s[:], start=True, stop=True)
            nc.vector.tensor_copy(vall[:, 2 * h:2 * h + 2], v_ps[:])

        # s1mine [16, 64] / s2all [16, 512]
        v1_ap = _ap(vall[:].tensor, 0, [[2 * NH, HID], [2, NH]])
        v2_ap = _ap(vall[:].tensor, 1, [[2 * NH, HID], [2, NH]])
        s1m_ps = psb.tile([NH, NPC], F32, tag="mm")
        nc.tensor.matmul(s1m_ps[:], v1_ap, hTm[:], start=True, stop=True)
        s1m = sb.tile([NH, NPC], F32, tag="s1m")
        nc.vector.tensor_copy(s1m[:], s1m_ps[:])
        s2a_ps = psb.tile([NH, N], F32, tag="mm")
        nc.tensor.matmul(s2a_ps[:], v2_ap, hT[:], start=True, stop=True)
        s2a = sb.tile([NH, N], F32, tag="s2a")
        nc.vector.tensor_copy(s2a[:], s2a_ps[:])
        # s2rep [128, 512]: row p -> s2a[p%16]
        s2rep_ps = psa.tile([128, N], F32, tag="s2rep")
        nc.tensor.matmul(s2rep_ps[:], selh2_s[:], s2a[:], start=True, stop=True)
        s2rep = sb.tile([128, N], F32, tag="s2repsb")
        nc.vector.tensor_copy(s2rep[:], s2rep_ps[:])

        # s1col [128, 8] via DRAM bounce: scratch [16, 64]
        scr = dram.tile([NH, NPC], F32, tag="scr")
        nc.sync.dma_start(scr[:], s1m[:])
        s1col = sb.tile([128, NC], F32, tag="s1col")
        with nc.allow_non_contiguous_dma(reason="s1col 4B gather"):
            for i in range(8):
                src_ap = _ap(scr[:].tensor, i, [[NPC, NH], [8, 8]])
                nc.sync.dma_start(s1col[16 * i:16 * (i + 1), :], src_ap)

        # ---------------- sc = a3-scores on slot grid, replicated rows
        sc_sb = sb.tile([128, NPC * S + 1], F32, tag="scsb")
        for q in range(8):
            scq_ps = psb.tile([128, 512], F32, tag="mm")
            nc.tensor.matmul(scq_ps[:], a3t_s[:], XP_s[:, 512 * q:512 * (q + 1)],
                             start=True, stop=True)
            nc.vector.tensor_copy(sc_sb[:, 512 * q:512 * (q + 1)], scq_ps[:])
        nc.gpsimd.memset(sc_sb[:, NPC * S:NPC * S + 1], 0.0)

        # ---------------- F stage: es over my 1024 edges
        esA_ps = psa.tile([1, 512], F32, tag="accA")
        esB_ps = psa.tile([1, 512], F32, tag="accB")
        sumo_ps = psa.tile([1, 1], F32, tag="accC")
        es_ps = [esA_ps, esB_ps]
        for h in range(NH):
            weg_s = sb2.tile([HID, OUT], F32, tag="weg")
            nc.sync.dma_start(weg_s[:], Wegat[h])
            st, sp = (h == 0), (h == NH - 1)
            for half in range(2):
                T_ps = psb.tile([128, 512], F32, tag="mm")
                nc.tensor.matmul(T_ps[:], weg_s[:], eaT_s[:, 512 * half:512 * (half + 1)],
                                 start=True, stop=True)
                ex = sb2.tile([128, 512], F32, tag="Fex")
                nc.scalar.activation(ex[:], T_ps[:], AF.Exp)
                rl = sb2.tile([128, 512], F32, tag="Frl")
                nc.scalar.activation(rl[:], T_ps[:], AF.Relu)
                eluP = sb2.tile([128, 512], F32, tag="eluP")
                nc.vector.scalar_tensor_tensor(eluP[:], ex[:], 1.0, rl[:],
                                               op0=ALU.min, op1=ALU.add)
                nc.tensor.matmul(es_ps[half][:], a3oT_s[:, h:h + 1], eluP[:],
                                 start=st, stop=sp)
            nc.tensor.matmul(sumo_ps[:], a3oT_s[:, h:h + 1], ones128[:], start=st, stop=sp)
        sumo = sb.tile([1, 1], F32, tag="sumosb")
        nc.vector.tensor_copy(sumo[:], sumo_ps[:])
        es_sb = sb.tile([1, EPC], F32, tag="essb")
        nc.vector.tensor_scalar(es_sb[:, :512], esA_ps[:], sumo[:], None, op0=ALU.subtract)
        nc.vector.tensor_scalar(es_sb[:, 512:], esB_ps[:], sumo[:], None, op0=ALU.subtract)
        nc.sync.dma_start(o_es[:], es_sb[:])

        # ---------------- e-stage: 8 tiles [128 (i*16+h), 512]
        att_tiles = []
        for t in range(8):
            e3g = sb2.tile([128, N], F32, tag="e3g")
            nc.gpsimd.ap_gather(e3g[:], sc_sb[:], gidx_s[:, 32 * t:32 * (t + 1)],
                                channels=128, num_elems=NPC * S + 1, d=1, num_idxs=N)
            e1 = sb2.tile([128, N], F32, tag="e1")
            nc.vector.tensor_tensor(e1[:], e3g[:], s2rep[:], ALU.add)
            lr = sb2.tile([128, N], F32, tag="lr")
            nc.scalar.activation(lr[:], e1[:], AF.Lrelu, bias=s1col[:, t:t + 1], alpha=0.2)
            adjrep_ps = psb.tile([128, N], F32, tag="mm")
            nc.tensor.matmul(adjrep_ps[:], selrep_s[:, 128 * t:128 * (t + 1)], adjm_s[:], start=True, stop=True)
            m1 = sb2.tile([128, N], F32, tag="m1")
            nc.vector.scalar_tensor_tensor(m1[:], lr[:], 1e9, adjrep_ps[:],
                                           op0=ALU.add, op1=ALU.mult)
            nmax = sb2.tile([128, 1], F32, tag="nmax")
            nc.vector.tensor_reduce(nmax[:], m1[:], AX.X, ALU.max, negate=True)
            pt = sb2.tile([128, N], F32, tag="pt")
            zt = sb2.tile([128, 1], F32, tag="zt")
            nc.scalar.activation(pt[:], m1[:], AF.Exp, bias=nmax[:], accum_out=zt[:])
            izt = sb2.tile([128, 1], F32, tag="izt")
            nc.vector.reciprocal(izt[:], zt[:])
            att = sb.tile([128, N], F32, tag=f"att{t}")
            nc.vector.tensor_scalar(att[:], pt[:], izt[:], None, op0=ALU.mult)
            att_tiles.append(att)

        # transposes -> attT[jc] [128, 1024] cols = t*128 + (i*16+h)
        attT = []
        for jc in range(4):
            bigt = sb.tile([128, 1024], F32, tag=f"attT{jc}")
            attT.append(bigt)
        for t in range(8):
            for jc in range(4):
                tp_ps = psb.tile([128, 128], F32, tag="mm")
                nc.tensor.transpose(tp_ps[:], att_tiles[t][:, 128 * jc:128 * (jc + 1)],
                                    ident_s[:])
                nc.vector.tensor_copy(attT[jc][:, 128 * t:128 * (t + 1)], tp_ps[:])

        # AV per head + elu
        hGelu = []
        for h in range(NH):
            wg_s = sb2.tile([HID, OUT], F32, tag="wgnat")
            nc.sync.dma_start(wg_s[:], Wgat[h])
            hg_ps = psa.tile([OUT, NPC], F32, tag="hg")
            for jc in range(4):
                wh_ps = psb.tile([128, OUT], F32, tag="mm")
                nc.tensor.matmul(wh_ps[:], hT[:, 128 * jc:128 * (jc + 1)], wg_s[:],
                                 start=True, stop=True)
                wh_sb = sb2.tile([128, OUT], F32, tag="whsb")
                nc.vector.tensor_copy(wh_sb[:], wh_ps[:])
                rhs = _ap(attT[jc][:].tensor, h, [[1024, 128], [128, 8], [16, 8]])
                nc.tensor.matmul(hg_ps[:], wh_sb[:], rhs, start=(jc == 0), stop=(jc == 3))
            hg = sb.tile([OUT, NPC], F32, tag=f"hgelu{h}")
            elu_inplace(hg_ps[:], hg[:], [OUT, NPC], tagp="elug")
            hGelu.append(hg)

        # pair gates
        dpa_ps = psa.tile([1, NPC], F32, tag="accA")
        dpb_ps = psa.tile([1, NPC], F32, tag="accB")
        for h in range(NH):
            wp_s = sb2.tile([OUT, 2], F32, tag="wps")
            nc.sync.dma_start(wp_s[:], wp1ab[h])
            st, sp = (h == 0), (h == NH - 1)
            nc.tensor.matmul(dpa_ps[:], wp_s[:, 0:1], hGelu[h][:], start=st, stop=sp)
            nc.tensor.matmul(dpb_ps[:], wp_s[:, 1:2], hGelu[h][:], start=st, stop=sp)
        dk = sb.tile([1, NPC // 2], F32, tag="dk")
        dasb = sb.tile([1, NPC], F32, tag="dasb")
        nc.vector.tensor_copy(dasb[:], dpa_ps[:])
        a_ap = _ap(dasb[:].tensor, 0, [[NPC, 1], [2, NPC // 2]])
        b_ap = _ap(dpb_ps[:].tensor, 1, [[NPC, 1], [2, NPC // 2]])
        nc.vector.tensor_tensor(dk[:], a_ap, b_ap, ALU.add)
        sgate = sb.tile([1, NPC // 2], F32, tag="sgate")
        nc.scalar.activation(sgate[:], dk[:], AF.Sigmoid, bias=bp1_s[:])
        srep_ps = psa.tile([128, NPC // 2], F32, tag="accC")
        nc.tensor.matmul(srep_ps[:], ones1_128[:], sgate[:], start=True, stop=True)

        h1T = []
        for h in range(NH):
            ev_ap = _ap(hGelu[h][:].tensor, 0, [[NPC, OUT], [2, NPC // 2]])
            od_ap = _ap(hGelu[h][:].tensor, 1, [[NPC, OUT], [2, NPC // 2]])
            t1 = sb2.tile([OUT, NPC // 2], F32, tag="pairsum")
            nc.vector.tensor_tensor(t1[:], ev_ap, od_ap, ALU.add)
            h1 = sb.tile([OUT, NPC // 2], F32, tag=f"h1T{h}")
            nc.vector.tensor_tensor(h1[:], t1[:], srep_ps[:], ALU.mult)
            h1T.append(h1)

        # g2 / u / Z / P
        g2_ps = psa.tile([1, NPC // 2], F32, tag="accA")
        for h in range(NH):
            wg2_s = sb2.tile([OUT, 1], F32, tag="wg2s")
            nc.sync.dma_start(wg2_s[:], Wg2r[h])
            nc.tensor.matmul(g2_ps[:], wg2_s[:], h1T[h][:],
                             start=(h == 0), stop=(h == NH - 1))
        sg2 = sb.tile([1, NPC // 2], F32, tag="sg2")
        nc.scalar.activation(sg2[:], g2_ps[:], AF.Sigmoid, bias=bg2_s[:])
        u = sb.tile([1, NPC // 2], F32, tag="u")
        nc.scalar.activation(u[:], sg2[:], AF.Exp)
        Zc = sb.tile([1, 1], F32, tag="Zc")
        nc.vector.tensor_reduce(Zc[:], u[:], AX.X, ALU.add)
        nc.sync.dma_start(o_Z[:], Zc[:])
        urep_ps = psa.tile([128, NPC // 2], F32, tag="accB")
        nc.tensor.matmul(urep_ps[:], ones1_128[:], u[:], start=True, stop=True)
        Pout = sb.tile([OUT, NH], F32, tag="Pout")
        for h in range(NH):
            pm = sb2.tile([OUT, NPC // 2], F32, tag="pm")
            nc.vector.tensor_tensor(pm[:], h1T[h][:], urep_ps[:OUT, :], ALU.mult)
            nc.vector.tensor_reduce(Pout[:, h:h + 1], pm[:], AX.X, ALU.add)
        nc.sync.dma_start(o_P[:], Pout[:])

        # Wh2T rows
        wh2_ps = psa.tile([OUT, NPC // 2], F32, tag="accC")
        for h in range(NH):
            wo_s = sb2.tile([OUT, OUT], F32, tag="wos")
            nc.sync.dma_start(wo_s[:], Wor[h])
            nc.tensor.matmul(wh2_ps[:], wo_s[:], h1T[h][:],
                             start=(h == 0), stop=(h == NH - 1))
        wh2 = sb.tile([OUT, NPC // 2], F32, tag="wh2sb")
        nc.vector.tensor_copy(wh2[:], wh2_ps[:])
        nc.sync.dma_start(o_Wh2T[:], wh2[:])

    nc.compile()
    return nc


# ---------------------------------------------------------------- dispatch B
def build_B():
    nc = bacc.Bacc("TRN2", target_bir_lowering=False, debug=False, num_devices=1)

    def inp(name, shape, dt=F32):
        return nc.dram_tensor(name, shape, dt, kind="ExternalInput").ap()

    adjm2 = inp("adjm2", [N2, N2])
    e3_2 = inp("e3_2", [N2, N2])
    Wh2T = inp("Wh2T", [OUT, N2])
    Wh2nat = inp("Wh2nat", [N2, OUT])
    a12o = inp("a12o", [OUT, 2])
    wp2ab = inp("wp2ab", [OUT, 2])
    bp2 = inp("bp2", [1, 1])
    Wg3 = inp("Wg3", [OUT, 1])
    bg3 = inp("bg3", [1, 1])
    fcWr = inp("fcWr", [2, LH, NCLS])
    fcb = inp("fcb", [1, NCLS])
    Pall = inp("Pall", [OUT, NC * NH])
    Zall = inp("Zall", [1, NC])
    hs0 = inp("hs0", [HID, 1])
    W0b = inp("W0b", [2, 18, 128, 4 * LH], BF16)   # row-chunked lhsT, bias row folded
    W1b = inp("W1b", [2, 3, 128, 4 * LH], BF16)
    ident = inp("ident", [128, 128])
    o_prob = nc.dram_tensor("o_prob", [1, NCLS], F32, kind="ExternalOutput").ap()

    with tile.TileContext(nc) as tc, ExitStack() as ctx:
        sb = ctx.enter_context(tc.tile_pool(name="sb", bufs=1))
        sb2 = ctx.enter_context(tc.tile_pool(name="sb2", bufs=2))
        psa = ctx.enter_context(tc.tile_pool(name="psa", bufs=1, space="PSUM"))
        psb = ctx.enter_context(tc.tile_pool(name="psb", bufs=2, space="PSUM"))

        def load(apx, shape, dt=F32, pool=sb, tag=None):
            t = pool.tile(shape, dt, tag=tag)
            nc.sync.dma_start(t[:], apx)
            return t

        ident_s = load(ident[:], [128, 128], tag="ident")
        ones1 = sb.tile([1, 128], F32, tag="ones1")
        nc.gpsimd.memset(ones1[:], 1.0)
        Pall_s = load(Pall[:], [OUT, NC * NH], tag="Pall")
        Zall_s = load(Zall[:], [1, NC], tag="Zall")
        hs0_s = load(hs0[:], [HID, 1], tag="hs0")
        Wh2T_s = load(Wh2T[:], [OUT, N2], tag="Wh2T")
        a12o_s = load(a12o[:], [OUT, 2], tag="a12o")
        wp2_s = load(wp2ab[:], [OUT, 2], tag="wp2")
        bp2_s = load(bp2[:], [1, 1], tag="bp2")
        Wg3_s = load(Wg3[:], [OUT, 1], tag="Wg3")
        bg3_s = load(bg3[:], [1, 1], tag="bg3")
        fcb_s = load(fcb[:], [1, NCLS], tag="fcb")

        # hs1 columns [128, 16] = sum_c Pall[:, c*16+h] / Z
        hs1c = sb.tile([OUT, NH], F32, tag="hs1c")
        src = _ap(Pall_s[:].tensor, 0, [[NC * NH, OUT], [1, NH], [NH, NC]])
        nc.vector.tensor_reduce(hs1c[:], src, AX.X, ALU.add)
        Zt = sb.tile([1, 1], F32, tag="Zt")
        nc.vector.tensor_reduce(Zt[:], Zall_s[:], AX.X, ALU.add)
        iZ = sb.tile([1, 1], F32, tag="iZ")
        nc.vector.reciprocal(iZ[:], Zt[:])
        izrep_ps = psa.tile([128, 1], F32, tag="r1")
        nc.tensor.matmul(izrep_ps[:], ones1[:], iZ[:], start=True, stop=True)
        izcol = sb.tile([128, 1], F32, tag="izcol")
        nc.vector.tensor_copy(izcol[:], izrep_ps[:])
        nc.vector.tensor_scalar(hs1c[:], hs1c[:], izcol[:OUT, :], None, op0=ALU.mult)

        # att2 scores
        s1o_ps = psa.tile([1, N2], F32, tag="r2")
        nc.tensor.matmul(s1o_ps[:], a12o_s[:, 0:1], Wh2T_s[:], start=True, stop=True)
        s2o_ps = psa.tile([1, N2], F32, tag="r3")
        nc.tensor.matmul(s2o_ps[:], a12o_s[:, 1:2], Wh2T_s[:], start=True, stop=True)
        s1o = sb.tile([1, N2], F32, tag="s1osb")
        nc.vector.tensor_copy(s1o[:], s1o_ps[:])
        s2o = sb.tile([1, N2], F32, tag="s2osb")
        nc.vector.tensor_copy(s2o[:], s2o_ps[:])
        s2orep_ps = psa.tile([128, N2], F32, tag="r4")
        nc.tensor.matmul(s2orep_ps[:], ones1[:], s2o[:], start=True, stop=True)

        att2 = []
        for t2 in range(2):
            s1c_ps = psb.tile([128, 1], F32, tag="mmB")
            nc.tensor.transpose(s1c_ps[:], s1o[:, 128 * t2:128 * (t2 + 1)], ident_s[0:1, 0:1])
            s1c = sb2.tile([128, 1], F32, tag="s1c")
            nc.vector.tensor_copy(s1c[:], s1c_ps[:])
            e3t = sb2.tile([128, N2], F32, tag="e3t")
            nc.sync.dma_start(e3t[:], e3_2[128 * t2:128 * (t2 + 1), :])
            adt = sb2.tile([128, N2], F32, tag="adt")
            nc.sync.dma_start(adt[:], adjm2[128 * t2:128 * (t2 + 1), :])
            e1 = sb2.tile([128, N2], F32, tag="e1b")
            nc.vector.tensor_tensor(e1[:], e3t[:], s2orep_ps[:], ALU.add)
            lr = sb2.tile([128, N2], F32, tag="lrb")
            nc.scalar.activation(lr[:], e1[:], AF.Lrelu, bias=s1c[:], alpha=0.2)
            m1 = sb2.tile([128, N2], F32, tag="m1b")
            nc.vector.scalar_tensor_tensor(m1[:], lr[:], 1e9, adt[:],
                                           op0=ALU.add, op1=ALU.mult)
            nmax = sb2.tile([128, 1], F32, tag="nmaxb")
            nc.vector.tensor_reduce(nmax[:], m1[:], AX.X, ALU.max, negate=True)
            pt = sb2.tile([128, N2], F32, tag="ptb")
            zt = sb2.tile([128, 1], F32, tag="ztb")
            nc.scalar.activation(pt[:], m1[:], AF.Exp, bias=nmax[:], accum_out=zt[:])
            izt = sb2.tile([128, 1], F32, tag="iztb")
            nc.vector.reciprocal(izt[:], zt[:])
            at = sb.tile([128, N2], F32, tag=f"att2_{t2}")
            nc.vector.tensor_scalar(at[:], pt[:], izt[:], None, op0=ALU.mult)
            att2.append(at)

        # att2T + h2T
        attT2 = []
        for lc in range(2):
            big = sb.tile([128, N2], F32, tag=f"attT2_{lc}")
            attT2.append(big)
        for t2 in range(2):
            for lc in range(2):
                tp_ps = psb.tile([128, 128], F32, tag="mmB")
                nc.tensor.transpose(tp_ps[:], att2[t2][:, 128 * lc:128 * (lc + 1)],
                                    ident_s[:])
                nc.vector.tensor_copy(attT2[lc][:, 128 * t2:128 * (t2 + 1)], tp_ps[:])
        h2_ps = psa.tile([OUT, N2], F32, tag="r5")
        for lc in range(2):
            w2n_s = sb2.tile([128, OUT], F32, tag="w2n")
            nc.sync.dma_start(w2n_s[:], Wh2nat[128 * lc:128 * (lc + 1), :])
            nc.tensor.matmul(h2_ps[:], w2n_s[:], attT2[lc][:],
                             start=(lc == 0), stop=(lc == 1))
        h2T = sb.tile([OUT, N2], F32, tag="h2T")
        nc.vector.tensor_copy(h2T[:], h2_ps[:])

        # edge pool 2
        dpa_ps = psa.tile([1, N2], F32, tag="r1")
        nc.tensor.matmul(dpa_ps[:], wp2_s[:, 0:1], h2T[:], start=True, stop=True)
        dpb_ps = psa.tile([1, N2], F32, tag="r2")
        nc.tensor.matmul(dpb_ps[:], wp2_s[:, 1:2], h2T[:], start=True, stop=True)
        dk2 = sb.tile([1, N3], F32, tag="dk2")
        dasb2 = sb.tile([1, N2], F32, tag="dasb2")
        nc.vector.tensor_copy(dasb2[:], dpa_ps[:])
        a_ap = _ap(dasb2[:].tensor, 0, [[N2, 1], [2, N3]])
        b_ap = _ap(dpb_ps[:].tensor, 1, [[N2, 1], [2, N3]])
        nc.vector.tensor_tensor(dk2[:], a_ap, b_ap, ALU.add)
        s2k = sb.tile([1, N3], F32, tag="s2k")
        nc.scalar.activation(s2k[:], dk2[:], AF.Sigmoid, bias=bp2_s[:])
        srep2_ps = psa.tile([128, N3], F32, tag="r3")
        nc.tensor.matmul(srep2_ps[:], ones1[:], s2k[:], start=True, stop=True)
        ev_ap = _ap(h2T[:].tensor, 0, [[N2, OUT], [2, N3]])
        od_ap = _ap(h2T[:].tensor, 1, [[N2, OUT], [2, N3]])
        t12 = sb.tile([OUT, N3], F32, tag="t12")
        nc.vector.tensor_tensor(t12[:], ev_ap, od_ap, ALU.add)
        h3T = sb.tile([OUT, N3], F32, tag="h3T")
        nc.vector.tensor_tensor(h3T[:], t12[:], srep2_ps[:OUT, :], ALU.mult)

        # gpool3 -> hs2 [128, 1]
        g3_ps = psa.tile([1, N3], F32, tag="r1")
        nc.tensor.matmul(g3_ps[:], Wg3_s[:], h3T[:], start=True, stop=True)
        g3s = sb.tile([1, N3], F32, tag="g3s")
        nc.scalar.activation(g3s[:], g3_ps[:], AF.Sigmoid, bias=bg3_s[:])
        nm3 = sb.tile([1, 1], F32, tag="nm3")
        nc.vector.tensor_reduce(nm3[:], g3s[:], AX.X, ALU.max, negate=True)
        w3 = sb.tile([1, N3], F32, tag="w3")
        z3 = sb.tile([1, 1], F32, tag="z3")
        nc.scalar.activation(w3[:], g3s[:], AF.Exp, bias=nm3[:], accum_out=z3[:])
        iz3 = sb.tile([1, 1], F32, tag="iz3")
        nc.vector.reciprocal(iz3[:], z3[:])
        nc.vector.tensor_scalar(w3[:], w3[:], iz3[:], None, op0=ALU.mult)
        w3rep_ps = psa.tile([128, N3], F32, tag="r2")
        nc.tensor.matmul(w3rep_ps[:], ones1[:], w3[:], start=True, stop=True)
        hw3 = sb.tile([OUT, N3], F32, tag="hw3")
        nc.vector.tensor_tensor(hw3[:], h3T[:], w3rep_ps[:OUT, :], ALU.mult)
        hs2 = sb.tile([OUT, 1], F32, tag="hs2")
        nc.vector.tensor_reduce(hs2[:], hw3[:], AX.X, ALU.add)

        # x chunks [128, 18] bf16: cols 0-15 hs1c, col16 [hs0; hs2[0:64]], col17 [hs2[64:]; 1]
        xc = sb.tile([128, 18], F32, tag="xc")
        nc.gpsimd.memset(xc[:], 0.0)
        nc.vector.tensor_copy(xc[:OUT, 0:NH], hs1c[:])
        nc.vector.tensor_copy(xc[:HID, 16:17], hs0_s[:])
        nc.sync.dma_start(xc[HID:128, 16:17], hs2[0:HID, :])
        nc.sync.dma_start(xc[0:HID, 17:18], hs2[HID:OUT, :])
        nc.gpsimd.memset(xc[HID:HID + 1, 17:18], 1.0)
        xcb = sb.tile([128, 18], BF16, tag="xcb")
        nc.vector.tensor_copy(xcb[:], xc[:])

        # LSTM layer 0 (M-orientation, skip f-gate m=1)
        h0 = []
        for d in range(2):
            g_ps = psa.tile([128, 4], F32, tag="gacc")
            for m in (0, 2, 3):
                for k in range(18):
                    rows = 65 if k == 17 else 128
                    w_s = sb2.tile([128, 128], BF16, tag="w0s")
                    nc.sync.dma_start(w_s[:rows, :], W0b[d, k, 0:rows, 128 * m:128 * (m + 1)])
                    nc.tensor.matmul(g_ps[:, m:m + 1], w_s[:rows, :], xcb[:rows, k:k + 1],
                                     start=(k == 0), stop=(k == 17))
            si = sb2.tile([128, 1], F32, tag="si")
            nc.scalar.activation(si[:], g_ps[:, 0:1], AF.Sigmoid)
            tg = sb2.tile([128, 1], F32, tag="tg")
            nc.scalar.activation(tg[:], g_ps[:, 2:3], AF.Tanh)
            so = sb2.tile([128, 1], F32, tag="so")
            nc.scalar.activation(so[:], g_ps[:, 3:4], AF.Sigmoid)
            c = sb2.tile([128, 1], F32, tag="c0")
            nc.vector.tensor_tensor(c[:], si[:], tg[:], ALU.mult)
            tc_ = sb2.tile([128, 1], F32, tag="tc0")
            nc.scalar.activation(tc_[:], c[:], AF.Tanh)
            hd = sb.tile([128, 1], F32, tag=f"h0_{d}")
            nc.vector.tensor_tensor(hd[:], so[:], tc_[:], ALU.mult)
            h0.append(hd)
        h0b_ = []
        for d in range(2):
            hb = sb.tile([128, 1], BF16, tag=f"h0b_{d}")
            nc.vector.tensor_copy(hb[:], h0[d][:])
            h0b_.append(hb)
        onesb = sb.tile([1, 1], BF16, tag="onesb")
        nc.gpsimd.memset(onesb[:], 1.0)

        # LSTM layer 1
        h1o = []
        for d in range(2):
            g_ps = psa.tile([128, 4], F32, tag="gacc")
            for m in (0, 2, 3):
                for k in range(3):
                    rows = 1 if k == 2 else 128
                    w_s = sb2.tile([128, 128], BF16, tag="w1s")
                    nc.sync.dma_start(w_s[:rows, :], W1b[d, k, 0:rows, 128 * m:128 * (m + 1)])
                    rhs = onesb[:] if k == 2 else h0b_[k][:]
                    nc.tensor.matmul(g_ps[:, m:m + 1], w_s[:rows, :], rhs,
                                     start=(k == 0), stop=(k == 2))
            si = sb2.tile([128, 1], F32, tag="si1")
            nc.scalar.activation(si[:], g_ps[:, 0:1], AF.Sigmoid)
            tg = sb2.tile([128, 1], F32, tag="tg1")
            nc.scalar.activation(tg[:], g_ps[:, 2:3], AF.Tanh)
            so = sb2.tile([128, 1], F32, tag="so1")
            nc.scalar.activation(so[:], g_ps[:, 3:4], AF.Sigmoid)
            c = sb2.tile([128, 1], F32, tag="c1")
            nc.vector.tensor_tensor(c[:], si[:], tg[:], ALU.mult)
            tc_ = sb2.tile([128, 1], F32, tag="tc1")
            nc.scalar.activation(tc_[:], c[:], AF.Tanh)
            hd = sb.tile([128, 1], F32, tag=f"h1_{d}")
            nc.vector.tensor_tensor(hd[:], so[:], tc_[:], ALU.mult)
            h1o.append(hd)

        # fc + softmax
        lg_ps = psa.tile([1, NCLS], F32, tag="r1")
        fcw0 = sb.tile([LH, NCLS], F32, tag="fcw0")
        nc.sync.dma_start(fcw0[:], fcWr[0])
        fcw1 = sb.tile([LH, NCLS], F32, tag="fcw1")
        nc.sync.dma_start(fcw1[:], fcWr[1])
        nc.tensor.matmul(lg_ps[:], h1o[0][:], fcw0[:], start=True, stop=False)
        nc.tensor.matmul(lg_ps[:], h1o[1][:], fcw1[:], start=False, stop=True)
        lg = sb.tile([1, NCLS], F32, tag="lg")
        nc.vector.tensor_tensor(lg[:], lg_ps[:], fcb_s[:], ALU.add)
        nmf = sb.tile([1, 1], F32, tag="nmf")
        nc.vector.tensor_reduce(nmf[:], lg[:], AX.X, ALU.max, negate=True)
        pf = sb.tile([1, NCLS], F32, tag="pf")
        zf = sb.tile([1, 1], F32, tag="zf")
        nc.scalar.activation(pf[:], lg[:], AF.Exp, bias=nmf[:], accum_out=zf[:])
        izf = sb.tile([1, 1], F32, tag="izf")
        nc.vector.reciprocal(izf[:], zf[:])
        prob = sb.tile([1, NCLS], F32, tag="prob")
        nc.vector.tensor_scalar(prob[:], pf[:], izf[:], None, op0=ALU.mult)
        nc.sync.dma_start(o_prob[:], prob[:])

    nc.compile()
    return nc


# ---------------------------------------------------------------- host prep
def _prep_A(inputs):
    """Build per-core input maps for dispatch A. Pure layout/indexing."""
    f32 = np.float32
    ei = np.asarray(inputs["edge_index"])
    feats = np.asarray(inputs["features"], f32)
    n2n = np.asarray(inputs["node2node_features"], f32)
    eattr = np.asarray(inputs["edgesAttr"], f32)
    adjacency = np.asarray(inputs["adjacency"], f32)

    src, dst = np.asarray(ei[0], np.int64), np.asarray(ei[1], np.int64)
    pairs = src * N + dst
    uniq = np.unique(pairs)
    us, ud = uniq // N, uniq % N
    # slot assignment per source node
    order = np.argsort(us, kind="stable")
    us, ud, uniq = us[order], ud[order], uniq[order]
    counts = np.bincount(us, minlength=N)
    assert counts.max() <= S, f"out-degree {counts.max()} > {S}"
    starts = np.zeros(N + 1, np.int64)
    np.cumsum(counts, out=starts[1:])
    slots = np.arange(len(us)) - starts[us]

    featT = np.ascontiguousarray(feats.T)
    eaT = np.ascontiguousarray(eattr.T)
    W_gat = np.asarray(inputs["W_gat"], f32)
    shared = {
        "featT": featT,
        "W_sn": np.asarray(inputs["W_sn"], f32),
        "a_sn": np.asarray(inputs["a_sn"], f32).reshape(HID, 1),
        "Wg1": np.asarray(inputs["Wg1"], f32).reshape(HID, 1),
        "bg1": np.asarray(inputs["bg1"], f32).reshape(1, 1),
        "Wgat": W_gat,
        "WgatT": np.ascontiguousarray(W_gat.transpose(0, 2, 1)),
        "a12": np.ascontiguousarray(np.stack(
            [np.asarray(inputs["a1_gat"], f32), np.asarray(inputs["a2_gat"], f32)], -1)),
        "a3t128": np.ascontiguousarray(np.tile(np.asarray(inputs["a3_gat"], f32).T, (1, 8))),
        "selh2": np.eye(NH, dtype=f32)[:, np.tile(np.arange(NH), 8)].reshape(NH, 128),
        "Wegat": np.asarray(inputs["We_gat"], f32),
        "a3oT": np.ascontiguousarray(np.asarray(inputs["a3_o"], f32).reshape(NH, OUT).T),
        "wp1ab": np.ascontiguousarray(np.stack([
            np.asarray(inputs["Wp1"], f32)[:D1, 0].reshape(NH, OUT),
            np.asarray(inputs["Wp1"], f32)[D1:, 0].reshape(NH, OUT)], -1)),
        "bp1": np.asarray(inputs["bp1"], f32).reshape(1, 1),
        "Wg2r": np.asarray(inputs["Wg2"], f32).reshape(NH, OUT, 1),
        "bg2": np.asarray(inputs["bg2"], f32).reshape(1, 1),
        "Wor": np.asarray(inputs["Wo"], f32).reshape(NH, OUT, OUT),
        "ident": np.eye(128, dtype=f32),
    }
    # selrep[t][r, p] = 1 iff r == 8t + p//16
    selrep = np.zeros((NPC, NC * 128), f32)
    for t in range(8):
        for p in range(128):
            selrep[8 * t + p // 16, 128 * t + p] = 1.0
    shared["selrep"] = selrep

    in_maps = []
    for c in range(NC):
        lo = c * NPC
        m = dict(shared)
        m["featTm"] = np.ascontiguousarray(featT[:, lo:lo + NPC])
        # slot grid XP [64, NPC*S] and gather idx
        mask = (us >= lo) & (us < lo + NPC)
        cs, cd, csl = us[mask] - lo, ud[mask], slots[mask]
        XP = np.zeros((NPC * S, HID), f32)
        XP[cs * S + csl] = n2n[uniq[mask]]
        m["XP"] = np.ascontiguousarray(XP.T)
        ptr = np.full((NPC, N), NPC * S, np.int64)
        ptr[cs, cd] = cs * S + csl
        g = np.zeros((128, 256), np.int16)
        for t in range(8):
            for gg in range(8):
                node = 8 * t + gg
                row = ptr[node]                      # [512]
                g[16 * gg:16 * gg + 16, 32 * t:32 * t + 32] = \
                    row.reshape(32, 16).T.astype(np.int16)
        m["gidx"] = g
        m["adjmine"] = np.ascontiguousarray(adjacency[lo:lo + NPC])
        m["eaT"] = np.ascontiguousarray(eaT[:, c * EPC:(c + 1) * EPC])
        in_maps.append(m)
    return in_maps, (src, dst)


def _prep_B(inputs, resA, ei_sd):
    f32 = np.float32
    src, dst = ei_sd
    es = np.concatenate([resA[c]["o_es"].reshape(-1) for c in range(NC)])
    s2, d2 = src // 2, dst // 2
    adj2 = np.zeros((N2, N2), f32)
    adj2[s2, d2] = 1.0
    e3_2 = np.zeros((N2, N2), f32)
    e3_2[s2, d2] = es  # numpy fancy assignment: last occurrence wins
    Wh2T = np.concatenate([resA[c]["o_Wh2T"] for c in range(NC)], axis=1)
    Pall = np.concatenate([resA[c]["o_P"] for c in range(NC)], axis=1)
    Zall = np.concatenate([resA[c]["o_Z"].reshape(1, 1) for c in range(NC)], axis=1)

    # LSTM weights: my-x order = [hs1(2048), hs0(64), hs2(128), bias(1)]
    perm = np.concatenate([np.arange(64, 2112), np.arange(0, 64), np.arange(2112, 2240)])
    W0 = np.zeros((2, 18, 128, 4 * LH), f32)
    for d in range(2):
        wt = np.asarray(inputs["Wih0"], f32)[d].T[perm]         # [2240, 512]
        wb = np.concatenate([wt, np.asarray(inputs["b0"], f32)[d][None, :]], 0)  # [2241,512]
        for k in range(18):
            rows = wb[128 * k:128 * (k + 1)]
            W0[d, k, :rows.shape[0], :] = rows
    W1 = np.zeros((2, 3, 128, 4 * LH), f32)
    for d in range(2):
        wt = np.asarray(inputs["Wih1"], f32)[d].T               # [256, 512]
        wb = np.concatenate([wt, np.asarray(inputs["b1"], f32)[d][None, :]], 0)
        for k in range(3):
            rows = wb[128 * k:128 * (k + 1)]
            W1[d, k, :rows.shape[0], :] = rows
    import ml_dtypes
    bf = ml_dtypes.bfloat16

    return {
        "adjm2": adj2,
        "e3_2": e3_2,
        "Wh2T": np.ascontiguousarray(Wh2T),
        "Wh2nat": np.ascontiguousarray(Wh2T.T),
        "a12o": np.ascontiguousarray(np.stack(
            [np.asarray(inputs["a1_o"], f32), np.asarray(inputs["a2_o"], f32)], -1)),
        "wp2ab": np.ascontiguousarray(np.stack(
            [np.asarray(inputs["Wp2"], f32)[:OUT, 0], np.asarray(inputs["Wp2"], f32)[OUT:, 0]], -1)),
        "bp2": np.asarray(inputs["bp2"], f32).reshape(1, 1),
        "Wg3": np.asarray(inputs["Wg3"], f32).reshape(OUT, 1),
        "bg3": np.asarray(inputs["bg3"], f32).reshape(1, 1),
        "fcWr": np.asarray(inputs["fc_W"], f32).reshape(2, LH, NCLS, order="C")
                  if False else np.stack([np.asarray(inputs["fc_W"], f32)[:LH],
                                          np.asarray(inputs["fc_W"], f32)[LH:]]),
        "fcb": np.asarray(inputs["fc_b"], f32).reshape(1, NCLS),
        "Pall": np.ascontiguousarray(Pall),
        "Zall": np.ascontiguousarray(Zall),
        "hs0": resA[0]["o_hs0"].reshape(HID, 1),
        "W0b": W0.astype(bf),
        "W1b": W1.astype(bf),
        "ident": np.eye(128, dtype=f32),
    }


# ------------------------------------------------------- cached SPMD runner
class _CachedRunner:
    """Like bass2jax.run_bass_via_pjrt but with the jitted callable built once."""

    def __init__(self, nc, n_cores):
        import jax
        from jax.sharding import Mesh, PartitionSpec
        from jax.experimental.shard_map import shard_map
        from concourse import bass2jax
        bass2jax.install_neuronx_cc_hook()
        self.n_cores = n_cores
        partition_name = nc.partition_id_tensor.name if nc.partition_id_tensor else None
        in_names, out_names, out_avals, zero_outs = [], [], [], []
        for alloc in nc.m.functions[0].allocations:
            if not isinstance(alloc, mybir.MemoryLocationSet):
                continue
            name = alloc.memorylocations[0].name
            if alloc.kind == "ExternalInput":
                if name != partition_name:
                    in_names.append(name)
            elif alloc.kind == "ExternalOutput":
                shape = tuple(alloc.tensor_shape)
                dtype = mybir.dt.np(alloc.dtype)
                out_names.append(name)
                out_avals.append(jax.core.ShapedArray(shape, dtype))
                zero_outs.append(np.zeros(shape, dtype))
        self.in_names, self.out_names = in_names, out_names
        self.out_avals, self.zero_outs = out_avals, zero_outs
        n_params, n_outs = len(in_names), len(out_names)
        all_names = in_names + out_names
        if partition_name is not None:
            all_names = all_names + [partition_name]
        donate = tuple(range(n_params, n_params + n_outs))

        def _body(*args):
            operands = list(args)
            if partition_name is not None:
                operands.append(bass2jax.partition_id_tensor())
            outs = bass2jax._bass_exec_p.bind(
                *operands,
                out_avals=tuple(out_avals),
                in_names=tuple(all_names),
                out_names=tuple(out_names),
                lowering_input_output_aliases=(),
                sim_require_finite=True,
                sim_require_nnan=True,
                nc=nc,
            )
            return tuple(outs)

        if n_cores == 1:
            self.fn = jax.jit(_body, donate_argnums=donate, keep_unused=True)
        else:
            devices = jax.devices()[:n_cores]
            mesh = Mesh(np.asarray(devices), ("core",))
            in_specs = (PartitionSpec("core"),) * (n_params + n_outs)
            out_specs = (PartitionSpec("core"),) * n_outs
            self.fn = jax.jit(
                shard_map(_body, mesh=mesh, in_specs=in_specs,
                          out_specs=out_specs, check_rep=False),
                donate_argnums=donate, keep_unused=True)

    def __call__(self, in_maps):
        nc_ = self.n_cores
        if nc_ == 1:
            out = self.fn(*[np.asarray(in_maps[0][n]) for n in self.in_names],
                          *self.zero_outs)
            return [{n: np.asarray(out[i]) for i, n in enumerate(self.out_names)}]
        concat_in = [np.concatenate([np.asarray(in_maps[c][n]) for c in range(nc_)], axis=0)
                     for n in self.in_names]
        concat_zeros = [np.zeros((nc_ * z.shape[0], *z.shape[1:]), z.dtype)
                        for z in self.zero_outs]
        out = self.fn(*concat_in, *concat_zeros)
        res = []
        for c in range(nc_):
            res.append({n: np.asarray(out[i]).reshape(nc_, *self.out_avals[i].shape)[c]
                        for i, n in enumerate(self.out_names)})
        return res


# ---------------------------------------------------------------- entrypoint
def kernel(**inputs):
    if "A" not in _cache:
        _cache["A"] = _CachedRunner(build_A(), NC)
    if "B" not in _cache:
        _cache["B"] = _CachedRunner(build_B(), 1)
    in_maps, ei_sd = _prep_A(inputs)
    resA = _cache["A"](in_maps)
    inB = _prep_B(inputs, resA, ei_sd)
    resB = _cache["B"]([inB])
    return resB[0]["o_prob"].reshape(NCLS).astype(np.float32)


# revision 9
# speedup vs baseline: 2.2273x; 1.5698x over previous
"""Trainium2 Bass kernel for nn_DefectDetection (GAT + pooling + LSTM head).

Self-contained: accepts FULL inputs, shards across 8 NeuronCores internally.

Strategy:
  Dispatch A (8 cores, SPMD):
    - replicated small front-end (node-attention layer, gpool1, GAT projections)
    - node-row-sharded dense [N,N] attention maps (64 rows x 16 heads / core),
      with the sparse node2node e3 term built from a host-packed slot grid via
      one matmul + gpsimd ap_gather (no 64MiB dense read)
    - edge-sharded edge-attr score reduction (es)
    - per-core outputs: es slice, gpool2 partials (P,Z), Wh2 rows, hs0
  Host in between: pure data movement (concat / scatter by precomputed indices).
  Dispatch B (1 core): pooled-graph attention (256 nodes), edge pool 2, gpool3,
    2-layer bi-LSTM (T=1) with bf16 weights, fc + softmax -> [2].
"""
import numpy as np
from contextlib import ExitStack

import concourse.bass as bass
import concourse.bacc as bacc
import concourse.tile as tile
import concourse.mybir as mybir
from concourse.bass_utils import run_bass_kernel_spmd

F32 = mybir.dt.float32
BF16 = mybir.dt.bfloat16
I16 = mybir.dt.int16
AF = mybir.ActivationFunctionType
ALU = mybir.AluOpType
AX = mybir.AxisListType

N, E, HID, NH, OUT, NCLS, LH = 512, 8192, 64, 16, 128, 2, 128
NC = 8          # cores
NPC = N // NC   # 64 nodes per core
S = 64          # slot grid per node
EPC = E // NC   # 1024 edges per core (F stage)
D1 = NH * OUT   # 2048
N2 = N // 2     # 256
N3 = N // 4     # 128
JUMP = HID + D1 + OUT  # 2240

_cache = {}


def _ap(t, offset, dims):
    return bass.AP(tensor=t, offset=offset, ap=[list(d) for d in dims])


# ---------------------------------------------------------------- dispatch A
def build_A():
    nc = bacc.Bacc("TRN2", target_bir_lowering=False, debug=False, num_devices=NC)

    def inp(name, shape, dt=F32):
        return nc.dram_tensor(name, shape, dt, kind="ExternalInput").ap()

    def outp(name, shape, dt=F32):
        return nc.dram_tensor(name, shape, dt, kind="ExternalOutput").ap()

    featT = inp("featT", [HID, N])
    featTm = inp("featTm", [HID, NPC])
    W_sn = inp("W_sn", [HID, HID])
    a_sn = inp("a_sn", [HID, 1])
    Wg1 = inp("Wg1", [HID, 1])
    bg1 = inp("bg1", [1, 1])
    Wgat = inp("Wgat", [NH, HID, OUT])
    a12 = inp("a12", [NH, OUT, 2])
    a3t128 = inp("a3t128", [HID, 128])
    XP = inp("XP", [HID, NPC * S])
    gidx = inp("gidx", [128, 256], I16)
    adjmine = inp("adjmine", [NPC, N])
    selrep = inp("selrep", [NPC, NC * 128])
    selh2 = inp("selh2", [NH, 128])
    eaT = inp("eaT", [HID, EPC])
    Wegat = inp("Wegat", [NH, HID, OUT])
    a3oT = inp("a3oT", [OUT, NH])
    wp1ab = inp("wp1ab", [NH, OUT, 2])
    bp1 = inp("bp1", [1, 1])
    Wg2r = inp("Wg2r", [NH, OUT, 1])
    bg2 = inp("bg2", [1, 1])
    Wor = inp("Wor", [NH, OUT, OUT])
    ident = inp("ident", [128, 128])

    o_all = outp("o_all", [128, 58])

    with tile.TileContext(nc) as tc, ExitStack() as ctx:
        sb = ctx.enter_context(tc.tile_pool(name="sb", bufs=1))
        sb2 = ctx.enter_context(tc.tile_pool(name="sb2", bufs=2))
        sb3 = ctx.enter_context(tc.tile_pool(name="sb3", bufs=3))
        psa = ctx.enter_context(tc.tile_pool(name="psa", bufs=1, space="PSUM"))
        psb = ctx.enter_context(tc.tile_pool(name="psb", bufs=2, space="PSUM"))
        dram = ctx.enter_context(tc.tile_pool(name="dram", bufs=1, space="DRAM"))

        def load(apx, shape, dt=F32, pool=sb, tag=None):
            t = pool.tile(shape, dt, tag=tag)
            nc.sync.dma_start(t[:], apx)
            return t

        featT_s = load(featT[:], [HID, N], tag="featT")
        featTm_s = load(featTm[:], [HID, NPC], tag="featTm")
        Wsn_s = load(W_sn[:], [HID, HID], tag="Wsn")
        asn_s = load(a_sn[:], [HID, 1], tag="asn")
        Wg1_s = load(Wg1[:], [HID, 1], tag="Wg1")
        bg1_s = load(bg1[:], [1, 1], tag="bg1")
        ident_s = load(ident[:], [128, 128], tag="ident")
        a3t_s = load(a3t128[:], [HID, 128], tag="a3t")
        XP_s = load(XP[:], [HID, NPC * S], tag="XP")
        gidx_s = load(gidx[:], [128, 256], I16, tag="gidx")
        adjm_s = load(adjmine[:], [NPC, N], tag="adjm")
        selh2_s = load(selh2[:], [NH, 128], tag="selh2")
        eaT_s = load(eaT[:], [HID, EPC], tag="eaT")
        selrep_s = load(selrep[:], [NPC, NC * 128], tag="selrep")
        a3oT_s = load(a3oT[:], [OUT, NH], tag="a3oT")
        bp1_s = load(bp1[:], [1, 1], tag="bp1")
        bg2_s = load(bg2[:], [1, 1], tag="bg2")

        ones1_128 = sb.tile([1, 128], F32, tag="ones1")
        nc.gpsimd.memset(ones1_128[:], 1.0)
        ones128 = sb.tile([128, 1], F32, tag="ones128")
        nc.gpsimd.memset(ones128[:], 1.0)

        def elu_inplace(src_ps, dst_sb, shape, pool=sb2, tagp="elu"):
            """dst = elu(src) where src is PSUM [p,f]; dst SBUF."""
            p, f = shape
            ex = pool.tile([p, f], F32, tag=tagp + "_ex")
            nc.scalar.activation(ex[:], src_ps, AF.Exp)
            rl = pool.tile([p, f], F32, tag=tagp + "_rl")
            nc.scalar.activation(rl[:], src_ps, AF.Relu)
            # dst = (min(ex,1) + rl) - 1
            nc.vector.scalar_tensor_tensor(dst_sb, ex[:], 1.0, rl[:],
                                           op0=ALU.min, op1=ALU.add)
            nc.vector.tensor_scalar(dst_sb, dst_sb, 1.0, None, op0=ALU.subtract)

        # ---------------- front: h = elu(sigmoid(lrelu(Wh0@a))*Wh0)
        def front(ft, width, tag):
            wh0_ps = psb.tile([HID, width], F32, tag="mm")
            nc.tensor.matmul(wh0_ps[:], Wsn_s[:], ft, start=True, stop=True)
            wh0 = sb.tile([HID, width], F32, tag="wh0_" + tag)
            nc.scalar.copy(wh0[:], wh0_ps[:])
            ga_ps = psb.tile([1, width], F32, tag="mm")
            nc.tensor.matmul(ga_ps[:], asn_s[:], wh0[:], start=True, stop=True)
            gl = sb.tile([1, width], F32, tag="gl_" + tag)
            nc.scalar.activation(gl[:], ga_ps[:], AF.Lrelu, alpha=0.2)
            gs = sb.tile([1, width], F32, tag="gs_" + tag)
            nc.scalar.activation(gs[:], gl[:], AF.Sigmoid)
            grep_ps = psb.tile([HID, width], F32, tag="mm")
            nc.tensor.matmul(grep_ps[:], ones1_128[:, :HID], gs[:], start=True, stop=True)
            hpre = sb.tile([HID, width], F32, tag="hpre_" + tag)
            nc.vector.tensor_tensor(hpre[:], wh0[:], grep_ps[:], ALU.mult)
            ht = sb.tile([HID, width], F32, tag="ht_" + tag)
            elu_inplace(hpre[:], ht[:], [HID, width], tagp="eluf_" + tag)
            return ht

        hT = front(featT_s[:], N, "full")          # [64, 512]
        hTm = front(featTm_s[:], NPC, "mine")      # [64, 64]

        # ---------------- gpool1 -> hs0
        g1_ps = psb.tile([1, N], F32, tag="mm")
        nc.tensor.matmul(g1_ps[:], Wg1_s[:], hT[:], start=True, stop=True)
        g1s = sb.tile([1, N], F32, tag="g1s")
        nc.scalar.activation(g1s[:], g1_ps[:], AF.Sigmoid, bias=bg1_s[:])
        nmax1 = sb.tile([1, 1], F32, tag="nmax1")
        nc.vector.tensor_reduce(nmax1[:], g1s[:], AX.X, ALU.max, negate=True)
        w1 = sb.tile([1, N], F32, tag="w1")
        z1 = sb.tile([1, 1], F32, tag="z1")
        nc.scalar.activation(w1[:], g1s[:], AF.Exp, bias=nmax1[:], accum_out=z1[:])
        iz1 = sb.tile([1, 1], F32, tag="iz1")
        nc.vector.reciprocal(iz1[:], z1[:])
        nc.vector.tensor_scalar(w1[:], w1[:], iz1[:], None, op0=ALU.mult)
        w1rep_ps = psb.tile([HID, N], F32, tag="mm")
        nc.tensor.matmul(w1rep_ps[:], ones1_128[:, :HID], w1[:], start=True, stop=True)
        hw = sb.tile([HID, N], F32, tag="hw")
        nc.vector.tensor_tensor(hw[:], hT[:], w1rep_ps[:], ALU.mult)
        hs0 = sb.tile([HID, 1], F32, tag="hs0")
        nc.vector.tensor_reduce(hs0[:], hw[:], AX.X, ALU.add)
        nc.sync.dma_start(o_all[0:HID, 48:49], hs0[:])

        # ---------------- v12 = WgatT[h] @ a12[h]  -> vall [64, 32]
        vall = sb.tile([HID, 2 * NH], F32, tag="vall")
        for h in range(NH):
            wg0_s = sb2.tile([HID, OUT], F32, tag="wgT0")
            nc.sync.dma_start(wg0_s[:], Wgat[h])
            wgT_ps = psb.tile([OUT, HID], F32, tag="mm")
            nc.tensor.transpose(wgT_ps[:], wg0_s[:], ident_s[0:HID, 0:HID])
            wgT_s = sb2.tile([OUT, HID], F32, tag="wgT")
            nc.vector.tensor_copy(wgT_s[:], wgT_ps[:])
            a12_s = sb2.tile([OUT, 2], F32, tag="a12s")
            nc.sync.dma_start(a12_s[:], a12[h])
            v_ps = psb.tile([HID, 2], F32, tag="mm")
            nc.tensor.matmul(v_ps[:], wgT_s[:], a12_s[:], start=True, stop=True)
            nc.vector.tensor_copy(vall[:, 2 * h:2 * h + 2], v_ps[:])

        # s1mine [16, 64] / s2all [16, 512]
        v1_ap = _ap(vall[:].tensor, 0, [[2 * NH, HID], [2, NH]])
        v2_ap = _ap(vall[:].tensor, 1, [[2 * NH, HID], [2, NH]])
        s1m_ps = psb.tile([NH, NPC], F32, tag="mm")
        nc.tensor.matmul(s1m_ps[:], v1_ap, hTm[:], start=True, stop=True)
        s1m = sb.tile([NH, NPC], F32, tag="s1m")
        nc.vector.tensor_copy(s1m[:], s1m_ps[:])
        s2a_ps = psb.tile([NH, N], F32, tag="mm")
        nc.tensor.matmul(s2a_ps[:], v2_ap, hT[:], start=True, stop=True)
        s2a = sb.tile([NH, N], F32, tag="s2a")
        nc.vector.tensor_copy(s2a[:], s2a_ps[:])
        # s2rep [128, 512]: row p -> s2a[p%16]
        s2rep_ps = psa.tile([128, N], F32, tag="s2rep")
        nc.tensor.matmul(s2rep_ps[:], selh2_s[:], s2a[:], start=True, stop=True)
        s2rep = sb.tile([128, N], F32, tag="s2repsb")
        nc.vector.tensor_copy(s2rep[:], s2rep_ps[:])

        # s1col [128, 8] via DRAM bounce: scratch [16, 64]
        scr = dram.tile([NH, NPC], F32, tag="scr")
        nc.sync.dma_start(scr[:], s1m[:])
        s1col = sb.tile([128, NC], F32, tag="s1col")
        with nc.allow_non_contiguous_dma(reason="s1col 4B gather"):
            for i in range(8):
                src_ap = _ap(scr[:].tensor, i, [[NPC, NH], [8, 8]])
                nc.sync.dma_start(s1col[16 * i:16 * (i + 1), :], src_ap)

        # ---------------- sc = a3-scores on slot grid, replicated rows
        sc_sb = sb.tile([128, NPC * S + 1], F32, tag="scsb")
        for q in range(8):
            scq_ps = psb.tile([128, 512], F32, tag="mm")
            nc.tensor.matmul(scq_ps[:], a3t_s[:], XP_s[:, 512 * q:512 * (q + 1)],
                             start=True, stop=True)
            nc.vector.tensor_copy(sc_sb[:, 512 * q:512 * (q + 1)], scq_ps[:])
        nc.gpsimd.memset(sc_sb[:, NPC * S:NPC * S + 1], 0.0)

        # ---------------- F stage: es over my 1024 edges
        esA_ps = psa.tile([1, 512], F32, tag="accA")
        esB_ps = psa.tile([1, 512], F32, tag="accB")
        sumo_ps = psa.tile([1, 1], F32, tag="accC")
        es_ps = [esA_ps, esB_ps]
        for h in range(NH):
            weg_s = sb2.tile([HID, OUT], F32, tag="weg")
            nc.sync.dma_start(weg_s[:], Wegat[h])
            st, sp = (h == 0), (h == NH - 1)
            for half in range(2):
                T_ps = psb.tile([128, 512], F32, tag="mm")
                nc.tensor.matmul(T_ps[:], weg_s[:], eaT_s[:, 512 * half:512 * (half + 1)],
                                 start=True, stop=True)
                ex = sb2.tile([128, 512], F32, tag="Fex")
                nc.scalar.activation(ex[:], T_ps[:], AF.Exp)
                rl = sb2.tile([128, 512], F32, tag="Frl")
                nc.scalar.activation(rl[:], T_ps[:], AF.Relu)
                eluP = sb2.tile([128, 512], F32, tag="eluP")
                nc.vector.scalar_tensor_tensor(eluP[:], ex[:], 1.0, rl[:],
                                               op0=ALU.min, op1=ALU.add)
                nc.tensor.matmul(es_ps[half][:], a3oT_s[:, h:h + 1], eluP[:],
                                 start=st, stop=sp)
            nc.tensor.matmul(sumo_ps[:], a3oT_s[:, h:h + 1], ones128[:], start=st, stop=sp)
        sumo = sb.tile([1, 1], F32, tag="sumosb")
        nc.vector.tensor_copy(sumo[:], sumo_ps[:])
        es_sb = sb.tile([1, EPC], F32, tag="essb")
        nc.vector.tensor_scalar(es_sb[:, :512], esA_ps[:], sumo[:], None, op0=ALU.subtract)
        nc.vector.tensor_scalar(es_sb[:, 512:], esB_ps[:], sumo[:], None, op0=ALU.subtract)
        dst_es = _ap(o_all.tensor, 50, [[58, 128], [1, 8]])
        nc.sync.dma_start(dst_es, es_sb[:])

        # ---------------- e-stage: 8 tiles [128 (i*16+h), 512]
        att_tiles = []
        for t in range(8):
            e3g = sb2.tile([128, N], F32, tag="e3g")
            nc.gpsimd.ap_gather(e3g[:], sc_sb[:], gidx_s[:, 32 * t:32 * (t + 1)],
                                channels=128, num_elems=NPC * S + 1, d=1, num_idxs=N)
            e1 = sb2.tile([128, N], F32, tag="e1")
            nc.vector.tensor_tensor(e1[:], e3g[:], s2rep[:], ALU.add)
            lr = sb2.tile([128, N], F32, tag="lr")
            nc.scalar.activation(lr[:], e1[:], AF.Lrelu, bias=s1col[:, t:t + 1], alpha=0.2)
            adjrep_ps = psb.tile([128, N], F32, tag="mm")
            nc.tensor.matmul(adjrep_ps[:], selrep_s[:, 128 * t:128 * (t + 1)], adjm_s[:], start=True, stop=True)
            m1 = sb2.tile([128, N], F32, tag="m1")
            nc.vector.scalar_tensor_tensor(m1[:], lr[:], 1e9, adjrep_ps[:],
                                           op0=ALU.add, op1=ALU.mult)
            nmax = sb2.tile([128, 1], F32, tag="nmax")
            nc.vector.tensor_reduce(nmax[:], m1[:], AX.X, ALU.max, negate=True)
            pt = sb2.tile([128, N], F32, tag="pt")
            zt = sb2.tile([128, 1], F32, tag="zt")
            nc.scalar.activation(pt[:], m1[:], AF.Exp, bias=nmax[:], accum_out=zt[:])
            izt = sb2.tile([128, 1], F32, tag="izt")
            nc.vector.reciprocal(izt[:], zt[:])
            att = sb.tile([128, N], F32, tag=f"att{t}")
            nc.vector.tensor_scalar(att[:], pt[:], izt[:], None, op0=ALU.mult)
            att_tiles.append(att)

        # transposes -> attT[jc] [128, 1024] cols = t*128 + (i*16+h)
        attT = []
        for jc in range(4):
            bigt = sb.tile([128, 1024], F32, tag=f"attT{jc}")
            attT.append(bigt)
        for t in range(8):
            for jc in range(4):
                tp_ps = psb.tile([128, 128], F32, tag="mm")
                nc.tensor.transpose(tp_ps[:], att_tiles[t][:, 128 * jc:128 * (jc + 1)],
                                    ident_s[:])
                nc.vector.tensor_copy(attT[jc][:, 128 * t:128 * (t + 1)], tp_ps[:])

        # AV per head + elu
        hGelu = []
        for h in range(NH):
            wg_s = sb2.tile([HID, OUT], F32, tag="wgnat")
            nc.sync.dma_start(wg_s[:], Wgat[h])
            hg_ps = psa.tile([OUT, NPC], F32, tag="hg")
            for jc in range(4):
                wh_ps = psb.tile([128, OUT], F32, tag="mm")
                nc.tensor.matmul(wh_ps[:], hT[:, 128 * jc:128 * (jc + 1)], wg_s[:],
                                 start=True, stop=True)
                wh_sb = sb2.tile([128, OUT], F32, tag="whsb")
                nc.vector.tensor_copy(wh_sb[:], wh_ps[:])
                rhs = _ap(attT[jc][:].tensor, h, [[1024, 128], [128, 8], [16, 8]])
                nc.tensor.matmul(hg_ps[:], wh_sb[:], rhs, start=(jc == 0), stop=(jc == 3))
            hg = sb.tile([OUT, NPC], F32, tag=f"hgelu{h}")
            elu_inplace(hg_ps[:], hg[:], [OUT, NPC], tagp="elug")
            hGelu.append(hg)

        # pair gates
        dpa_ps = psa.tile([1, NPC], F32, tag="accA")
        dpb_ps = psa.tile([1, NPC], F32, tag="accB")
        for h in range(NH):
            wp_s = sb2.tile([OUT, 2], F32, tag="wps")
            nc.sync.dma_start(wp_s[:], wp1ab[h])
            st, sp = (h == 0), (h == NH - 1)
            nc.tensor.matmul(dpa_ps[:], wp_s[:, 0:1], hGelu[h][:], start=st, stop=sp)
            nc.tensor.matmul(dpb_ps[:], wp_s[:, 1:2], hGelu[h][:], start=st, stop=sp)
        dk = sb.tile([1, NPC // 2], F32, tag="dk")
        dasb = sb.tile([1, NPC], F32, tag="dasb")
        nc.vector.tensor_copy(dasb[:], dpa_ps[:])
        a_ap = _ap(dasb[:].tensor, 0, [[NPC, 1], [2, NPC // 2]])
        b_ap = _ap(dpb_ps[:].tensor, 1, [[NPC, 1], [2, NPC // 2]])
        nc.vector.tensor_tensor(dk[:], a_ap, b_ap, ALU.add)
        sgate = sb.tile([1, NPC // 2], F32, tag="sgate")
        nc.scalar.activation(sgate[:], dk[:], AF.Sigmoid, bias=bp1_s[:])
        srep_ps = psa.tile([128, NPC // 2], F32, tag="accC")
        nc.tensor.matmul(srep_ps[:], ones1_128[:], sgate[:], start=True, stop=True)

        h1T = []
        for h in range(NH):
            ev_ap = _ap(hGelu[h][:].tensor, 0, [[NPC, OUT], [2, NPC // 2]])
            od_ap = _ap(hGelu[h][:].tensor, 1, [[NPC, OUT], [2, NPC // 2]])
            t1 = sb2.tile([OUT, NPC // 2], F32, tag="pairsum")
            nc.vector.tensor_tensor(t1[:], ev_ap, od_ap, ALU.add)
            h1 = sb.tile([OUT, NPC // 2], F32, tag=f"h1T{h}")
            nc.vector.tensor_tensor(h1[:], t1[:], srep_ps[:], ALU.mult)
            h1T.append(h1)

        # g2 / u / Z / P
        g2_ps = psa.tile([1, NPC // 2], F32, tag="accA")
        for h in range(NH):
            wg2_s = sb2.tile([OUT, 1], F32, tag="wg2s")
            nc.sync.dma_start(wg2_s[:], Wg2r[h])
            nc.tensor.matmul(g2_ps[:], wg2_s[:], h1T[h][:],
                             start=(h == 0), stop=(h == NH - 1))
        sg2 = sb.tile([1, NPC // 2], F32, tag="sg2")
        nc.scalar.activation(sg2[:], g2_ps[:], AF.Sigmoid, bias=bg2_s[:])
        u = sb.tile([1, NPC // 2], F32, tag="u")
        nc.scalar.activation(u[:], sg2[:], AF.Exp)
        Zc = sb.tile([1, 1], F32, tag="Zc")
        nc.vector.tensor_reduce(Zc[:], u[:], AX.X, ALU.add)
        nc.sync.dma_start(o_all[0:1, 49:50], Zc[:])
        urep_ps = psa.tile([128, NPC // 2], F32, tag="accB")
        nc.tensor.matmul(urep_ps[:], ones1_128[:], u[:], start=True, stop=True)
        Pout = sb.tile([OUT, NH], F32, tag="Pout")
        for h in range(NH):
            pm = sb2.tile([OUT, NPC // 2], F32, tag="pm")
            nc.vector.tensor_tensor(pm[:], h1T[h][:], urep_ps[:OUT, :], ALU.mult)
            nc.vector.tensor_reduce(Pout[:, h:h + 1], pm[:], AX.X, ALU.add)
        nc.sync.dma_start(o_all[:, 0:16], Pout[:])

        # Wh2T rows
        wh2_ps = psa.tile([OUT, NPC // 2], F32, tag="accC")
        for h in range(NH):
            wo_s = sb2.tile([OUT, OUT], F32, tag="wos")
            nc.sync.dma_start(wo_s[:], Wor[h])
            nc.tensor.matmul(wh2_ps[:], wo_s[:], h1T[h][:],
                             start=(h == 0), stop=(h == NH - 1))
        wh2 = sb.tile([OUT, NPC // 2], F32, tag="wh2sb")
        nc.vector.tensor_copy(wh2[:], wh2_ps[:])
        nc.sync.dma_start(o_all[:, 16:48], wh2[:])

    nc.compile()
    return nc


# ---------------------------------------------------------------- dispatch B
def build_B():
    nc = bacc.Bacc("TRN2", target_bir_lowering=False, debug=False, num_devices=1)

    def inp(name, shape, dt=F32):
        return nc.dram_tensor(name, shape, dt, kind="ExternalInput").ap()

    adjm2 = inp("adjm2", [N2, N2])
    e3_2 = inp("e3_2", [N2, N2])
    Wh2T = inp("Wh2T", [OUT, N2])
    Wh2nat = inp("Wh2nat", [N2, OUT])
    a12o = inp("a12o", [OUT, 2])
    wp2ab = inp("wp2ab", [OUT, 2])
    bp2 = inp("bp2", [1, 1])
    Wg3 = inp("Wg3", [OUT, 1])
    bg3 = inp("bg3", [1, 1])
    fcWr = inp("fcWr", [2, LH, NCLS])
    fcb = inp("fcb", [1, NCLS])
    Pall = inp("Pall", [OUT, NC * NH])
    Zall = inp("Zall", [1, NC])
    hs0 = inp("hs0", [HID, 1])
    W0b = inp("W0b", [2, 18, 128, 4 * LH], BF16)   # row-chunked lhsT, bias row folded
    W1b = inp("W1b", [2, 3, 128, 4 * LH], BF16)
    ident = inp("ident", [128, 128])
    o_prob = nc.dram_tensor("o_prob", [1, NCLS], F32, kind="ExternalOutput").ap()

    with tile.TileContext(nc) as tc, ExitStack() as ctx:
        sb = ctx.enter_context(tc.tile_pool(name="sb", bufs=1))
        sb2 = ctx.enter_context(tc.tile_pool(name="sb2", bufs=2))
        psa = ctx.enter_context(tc.tile_pool(name="psa", bufs=1, space="PSUM"))
        psb = ctx.enter_context(tc.tile_pool(name="psb", bufs=2, space="PSUM"))

        def load(apx, shape, dt=F32, pool=sb, tag=None):
            t = pool.tile(shape, dt, tag=tag)
            nc.sync.dma_start(t[:], apx)
            return t

        ident_s = load(ident[:], [128, 128], tag="ident")
        ones1 = sb.tile([1, 128], F32, tag="ones1")
        nc.gpsimd.memset(ones1[:], 1.0)
        Pall_s = load(Pall[:], [OUT, NC * NH], tag="Pall")
        Zall_s = load(Zall[:], [1, NC], tag="Zall")
        hs0_s = load(hs0[:], [HID, 1], tag="hs0")
        Wh2T_s = load(Wh2T[:], [OUT, N2], tag="Wh2T")
        a12o_s = load(a12o[:], [OUT, 2], tag="a12o")
        wp2_s = load(wp2ab[:], [OUT, 2], tag="wp2")
        bp2_s = load(bp2[:], [1, 1], tag="bp2")
        Wg3_s = load(Wg3[:], [OUT, 1], tag="Wg3")
        bg3_s = load(bg3[:], [1, 1], tag="bg3")
        fcb_s = load(fcb[:], [1, NCLS], tag="fcb")

        # hs1 columns [128, 16] = sum_c Pall[:, c*16+h] / Z
        hs1c = sb.tile([OUT, NH], F32, tag="hs1c")
        src = _ap(Pall_s[:].tensor, 0, [[NC * NH, OUT], [1, NH], [NH, NC]])
        nc.vector.tensor_reduce(hs1c[:], src, AX.X, ALU.add)
        Zt = sb.tile([1, 1], F32, tag="Zt")
        nc.vector.tensor_reduce(Zt[:], Zall_s[:], AX.X, ALU.add)
        iZ = sb.tile([1, 1], F32, tag="iZ")
        nc.vector.reciprocal(iZ[:], Zt[:])
        izrep_ps = psa.tile([128, 1], F32, tag="r1")
        nc.tensor.matmul(izrep_ps[:], ones1[:], iZ[:], start=True, stop=True)
        izcol = sb.tile([128, 1], F32, tag="izcol")
        nc.vector.tensor_copy(izcol[:], izrep_ps[:])
        nc.vector.tensor_scalar(hs1c[:], hs1c[:], izcol[:OUT, :], None, op0=ALU.mult)

        # att2 scores
        s1o_ps = psa.tile([1, N2], F32, tag="r2")
        nc.tensor.matmul(s1o_ps[:], a12o_s[:, 0:1], Wh2T_s[:], start=True, stop=True)
        s2o_ps = psa.tile([1, N2], F32, tag="r3")
        nc.tensor.matmul(s2o_ps[:], a12o_s[:, 1:2], Wh2T_s[:], start=True, stop=True)
        s1o = sb.tile([1, N2], F32, tag="s1osb")
        nc.vector.tensor_copy(s1o[:], s1o_ps[:])
        s2o = sb.tile([1, N2], F32, tag="s2osb")
        nc.vector.tensor_copy(s2o[:], s2o_ps[:])
        s2orep_ps = psa.tile([128, N2], F32, tag="r4")
        nc.tensor.matmul(s2orep_ps[:], ones1[:], s2o[:], start=True, stop=True)

        att2 = []
        for t2 in range(2):
            s1c_ps = psb.tile([128, 1], F32, tag="mmB")
            nc.tensor.transpose(s1c_ps[:], s1o[:, 128 * t2:128 * (t2 + 1)], ident_s[0:1, 0:1])
            s1c = sb2.tile([128, 1], F32, tag="s1c")
            nc.vector.tensor_copy(s1c[:], s1c_ps[:])
            e3t = sb2.tile([128, N2], F32, tag="e3t")
            nc.sync.dma_start(e3t[:], e3_2[128 * t2:128 * (t2 + 1), :])
            adt = sb2.tile([128, N2], F32, tag="adt")
            nc.sync.dma_start(adt[:], adjm2[128 * t2:128 * (t2 + 1), :])
            e1 = sb2.tile([128, N2], F32, tag="e1b")
            nc.vector.tensor_tensor(e1[:], e3t[:], s2orep_ps[:], ALU.add)
            lr = sb2.tile([128, N2], F32, tag="lrb")
            nc.scalar.activation(lr[:], e1[:], AF.Lrelu, bias=s1c[:], alpha=0.2)
            m1 = sb2.tile([128, N2], F32, tag="m1b")
            nc.vector.scalar_tensor_tensor(m1[:], lr[:], 1e9, adt[:],
                                           op0=ALU.add, op1=ALU.mult)
            nmax = sb2.tile([128, 1], F32, tag="nmaxb")
            nc.vector.tensor_reduce(nmax[:], m1[:], AX.X, ALU.max, negate=True)
            pt = sb2.tile([128, N2], F32, tag="ptb")
            zt = sb2.tile([128, 1], F32, tag="ztb")
            nc.scalar.activation(pt[:], m1[:], AF.Exp, bias=nmax[:], accum_out=zt[:])
            izt = sb2.tile([128, 1], F32, tag="iztb")
            nc.vector.reciprocal(izt[:], zt[:])
            at = sb.tile([128, N2], F32, tag=f"att2_{t2}")
            nc.vector.tensor_scalar(at[:], pt[:], izt[:], None, op0=ALU.mult)
            att2.append(at)

        # att2T + h2T
        attT2 = []
        for lc in range(2):
            big = sb.tile([128, N2], F32, tag=f"attT2_{lc}")
            attT2.append(big)
        for t2 in range(2):
            for lc in range(2):
                tp_ps = psb.tile([128, 128], F32, tag="mmB")
                nc.tensor.transpose(tp_ps[:], att2[t2][:, 128 * lc:128 * (lc + 1)],
                                    ident_s[:])
                nc.vector.tensor_copy(attT2[lc][:, 128 * t2:128 * (t2 + 1)], tp_ps[:])
        h2_ps = psa.tile([OUT, N2], F32, tag="r5")
        for lc in range(2):
            w2n_s = sb2.tile([128, OUT], F32, tag="w2n")
            nc.sync.dma_start(w2n_s[:], Wh2nat[128 * lc:128 * (lc + 1), :])
            nc.tensor.matmul(h2_ps[:], w2n_s[:], attT2[lc][:],
                             start=(lc == 0), stop=(lc == 1))
        h2T = sb.tile([OUT, N2], F32, tag="h2T")
        nc.vector.tensor_copy(h2T[:], h2_ps[:])

        # edge pool 2
        dpa_ps = psa.tile([1, N2], F32, tag="r1")
        nc.tensor.matmul(dpa_ps[:], wp2_s[:, 0:1], h2T[:], start=True, stop=True)
        dpb_ps = psa.tile([1, N2], F32, tag="r2")
        nc.tensor.matmul(dpb_ps[:], wp2_s[:, 1:2], h2T[:], start=True, stop=True)
        dk2 = sb.tile([1, N3], F32, tag="dk2")
        dasb2 = sb.tile([1, N2], F32, tag="dasb2")
        nc.vector.tensor_copy(dasb2[:], dpa_ps[:])
        a_ap = _ap(dasb2[:].tensor, 0, [[N2, 1], [2, N3]])
        b_ap = _ap(dpb_ps[:].tensor, 1, [[N2, 1], [2, N3]])
        nc.vector.tensor_tensor(dk2[:], a_ap, b_ap, ALU.add)
        s2k = sb.tile([1, N3], F32, tag="s2k")
        nc.scalar.activation(s2k[:], dk2[:], AF.Sigmoid, bias=bp2_s[:])
        srep2_ps = psa.tile([128, N3], F32, tag="r3")
        nc.tensor.matmul(srep2_ps[:], ones1[:], s2k[:], start=True, stop=True)
        ev_ap = _ap(h2T[:].tensor, 0, [[N2, OUT], [2, N3]])
        od_ap = _ap(h2T[:].tensor, 1, [[N2, OUT], [2, N3]])
        t12 = sb.tile([OUT, N3], F32, tag="t12")
        nc.vector.tensor_tensor(t12[:], ev_ap, od_ap, ALU.add)
        h3T = sb.tile([OUT, N3], F32, tag="h3T")
        nc.vector.tensor_tensor(h3T[:], t12[:], srep2_ps[:OUT, :], ALU.mult)

        # gpool3 -> hs2 [128, 1]
        g3_ps = psa.tile([1, N3], F32, tag="r1")
        nc.tensor.matmul(g3_ps[:], Wg3_s[:], h3T[:], start=True, stop=True)
        g3s = sb.tile([1, N3], F32, tag="g3s")
        nc.scalar.activation(g3s[:], g3_ps[:], AF.Sigmoid, bias=bg3_s[:])
        nm3 = sb.tile([1, 1], F32, tag="nm3")
        nc.vector.tensor_reduce(nm3[:], g3s[:], AX.X, ALU.max, negate=True)
        w3 = sb.tile([1, N3], F32, tag="w3")
        z3 = sb.tile([1, 1], F32, tag="z3")
        nc.scalar.activation(w3[:], g3s[:], AF.Exp, bias=nm3[:], accum_out=z3[:])
        iz3 = sb.tile([1, 1], F32, tag="iz3")
        nc.vector.reciprocal(iz3[:], z3[:])
        nc.vector.tensor_scalar(w3[:], w3[:], iz3[:], None, op0=ALU.mult)
        w3rep_ps = psa.tile([128, N3], F32, tag="r2")
        nc.tensor.matmul(w3rep_ps[:], ones1[:], w3[:], start=True, stop=True)
        hw3 = sb.tile([OUT, N3], F32, tag="hw3")
        nc.vector.tensor_tensor(hw3[:], h3T[:], w3rep_ps[:OUT, :], ALU.mult)
        hs2 = sb.tile([OUT, 1], F32, tag="hs2")
        nc.vector.tensor_reduce(hs2[:], hw3[:], AX.X, ALU.add)

        # x chunks [128, 18] bf16: cols 0-15 hs1c, col16 [hs0; hs2[0:64]], col17 [hs2[64:]; 1]
        xc = sb.tile([128, 18], F32, tag="xc")
        nc.gpsimd.memset(xc[:], 0.0)
        nc.vector.tensor_copy(xc[:OUT, 0:NH], hs1c[:])
        nc.vector.tensor_copy(xc[:HID, 16:17], hs0_s[:])
        nc.sync.dma_start(xc[HID:128, 16:17], hs2[0:HID, :])
        nc.sync.dma_start(xc[0:HID, 17:18], hs2[HID:OUT, :])
        nc.gpsimd.memset(xc[HID:HID + 1, 17:18], 1.0)
        xcb = sb.tile([128, 18], BF16, tag="xcb")
        nc.vector.tensor_copy(xcb[:], xc[:])

        # LSTM layer 0 (M-orientation, skip f-gate m=1)
        h0 = []
        for d in range(2):
            g_ps = psa.tile([128, 4], F32, tag="gacc")
            for m in (0, 2, 3):
                for k in range(18):
                    rows = 65 if k == 17 else 128
                    w_s = sb2.tile([128, 128], BF16, tag="w0s")
                    nc.sync.dma_start(w_s[:rows, :], W0b[d, k, 0:rows, 128 * m:128 * (m + 1)])
                    nc.tensor.matmul(g_ps[:, m:m + 1], w_s[:rows, :], xcb[:rows, k:k + 1],
                                     start=(k == 0), stop=(k == 17))
            si = sb2.tile([128, 1], F32, tag="si")
            nc.scalar.activation(si[:], g_ps[:, 0:1], AF.Sigmoid)
            tg = sb2.tile([128, 1], F32, tag="tg")
            nc.scalar.activation(tg[:], g_ps[:, 2:3], AF.Tanh)
            so = sb2.tile([128, 1], F32, tag="so")
            nc.scalar.activation(so[:], g_ps[:, 3:4], AF.Sigmoid)
            c = sb2.tile([128, 1], F32, tag="c0")
            nc.vector.tensor_tensor(c[:], si[:], tg[:], ALU.mult)
            tc_ = sb2.tile([128, 1], F32, tag="tc0")
            nc.scalar.activation(tc_[:], c[:], AF.Tanh)
            hd = sb.tile([128, 1], F32, tag=f"h0_{d}")
            nc.vector.tensor_tensor(hd[:], so[:], tc_[:], ALU.mult)
            h0.append(hd)
        h0b_ = []
        for d in range(2):
            hb = sb.tile([128, 1], BF16, tag=f"h0b_{d}")
            nc.vector.tensor_copy(hb[:], h0[d][:])
            h0b_.append(hb)
        onesb = sb.tile([1, 1], BF16, tag="onesb")
        nc.gpsimd.memset(onesb[:], 1.0)

        # LSTM layer 1
        h1o = []
        for d in range(2):
            g_ps = psa.tile([128, 4], F32, tag="gacc")
            for m in (0, 2, 3):
                for k in range(3):
                    rows = 1 if k == 2 else 128
                    w_s = sb2.tile([128, 128], BF16, tag="w1s")
                    nc.sync.dma_start(w_s[:rows, :], W1b[d, k, 0:rows, 128 * m:128 * (m + 1)])
                    rhs = onesb[:] if k == 2 else h0b_[k][:]
                    nc.tensor.matmul(g_ps[:, m:m + 1], w_s[:rows, :], rhs,
                                     start=(k == 0), stop=(k == 2))
            si = sb2.tile([128, 1], F32, tag="si1")
            nc.scalar.activation(si[:], g_ps[:, 0:1], AF.Sigmoid)
            tg = sb2.tile([128, 1], F32, tag="tg1")
            nc.scalar.activation(tg[:], g_ps[:, 2:3], AF.Tanh)
            so = sb2.tile([128, 1], F32, tag="so1")
            nc.scalar.activation(so[:], g_ps[:, 3:4], AF.Sigmoid)
            c = sb2.tile([128, 1], F32, tag="c1")
            nc.vector.tensor_tensor(c[:], si[:], tg[:], ALU.mult)
            tc_ = sb2.tile([128, 1], F32, tag="tc1")
            nc.scalar.activation(tc_[:], c[:], AF.Tanh)
            hd = sb.tile([128, 1], F32, tag=f"h1_{d}")
            nc.vector.tensor_tensor(hd[:], so[:], tc_[:], ALU.mult)
            h1o.append(hd)

        # fc + softmax
        lg_ps = psa.tile([1, NCLS], F32, tag="r1")
        fcw0 = sb.tile([LH, NCLS], F32, tag="fcw0")
        nc.sync.dma_start(fcw0[:], fcWr[0])
        fcw1 = sb.tile([LH, NCLS], F32, tag="fcw1")
        nc.sync.dma_start(fcw1[:], fcWr[1])
        nc.tensor.matmul(lg_ps[:], h1o[0][:], fcw0[:], start=True, stop=False)
        nc.tensor.matmul(lg_ps[:], h1o[1][:], fcw1[:], start=False, stop=True)
        lg = sb.tile([1, NCLS], F32, tag="lg")
        nc.vector.tensor_tensor(lg[:], lg_ps[:], fcb_s[:], ALU.add)
        nmf = sb.tile([1, 1], F32, tag="nmf")
        nc.vector.tensor_reduce(nmf[:], lg[:], AX.X, ALU.max, negate=True)
        pf = sb.tile([1, NCLS], F32, tag="pf")
        zf = sb.tile([1, 1], F32, tag="zf")
        nc.scalar.activation(pf[:], lg[:], AF.Exp, bias=nmf[:], accum_out=zf[:])
        izf = sb.tile([1, 1], F32, tag="izf")
        nc.vector.reciprocal(izf[:], zf[:])
        prob = sb.tile([1, NCLS], F32, tag="prob")
        nc.vector.tensor_scalar(prob[:], pf[:], izf[:], None, op0=ALU.mult)
        nc.sync.dma_start(o_prob[:], prob[:])

    nc.compile()
    return nc


# ---------------------------------------------------------------- host prep
def _prep_A(inputs):
    """Build per-core input maps for dispatch A. Pure layout/indexing."""
    f32 = np.float32
    ei = np.asarray(inputs["edge_index"])
    feats = np.asarray(inputs["features"], f32)
    n2n = np.asarray(inputs["node2node_features"], f32)
    eattr = np.asarray(inputs["edgesAttr"], f32)
    adjacency = np.asarray(inputs["adjacency"], f32)

    src, dst = np.asarray(ei[0], np.int64), np.asarray(ei[1], np.int64)
    pairs = src * N + dst
    uniq = np.unique(pairs)
    us, ud = uniq // N, uniq % N
    # slot assignment per source node
    order = np.argsort(us, kind="stable")
    us, ud, uniq = us[order], ud[order], uniq[order]
    counts = np.bincount(us, minlength=N)
    assert counts.max() <= S, f"out-degree {counts.max()} > {S}"
    starts = np.zeros(N + 1, np.int64)
    np.cumsum(counts, out=starts[1:])
    slots = np.arange(len(us)) - starts[us]

    featT = np.ascontiguousarray(feats.T)
    eaT = np.ascontiguousarray(eattr.T)
    W_gat = np.asarray(inputs["W_gat"], f32)
    shared = {
        "featT": featT,
        "W_sn": np.asarray(inputs["W_sn"], f32),
        "a_sn": np.asarray(inputs["a_sn"], f32).reshape(HID, 1),
        "Wg1": np.asarray(inputs["Wg1"], f32).reshape(HID, 1),
        "bg1": np.asarray(inputs["bg1"], f32).reshape(1, 1),
        "Wgat": W_gat,
        "a12": np.ascontiguousarray(np.stack(
            [np.asarray(inputs["a1_gat"], f32), np.asarray(inputs["a2_gat"], f32)], -1)),
        "a3t128": np.ascontiguousarray(np.tile(np.asarray(inputs["a3_gat"], f32).T, (1, 8))),
        "selh2": np.eye(NH, dtype=f32)[:, np.tile(np.arange(NH), 8)].reshape(NH, 128),
        "Wegat": np.asarray(inputs["We_gat"], f32),
        "a3oT": np.ascontiguousarray(np.asarray(inputs["a3_o"], f32).reshape(NH, OUT).T),
        "wp1ab": np.ascontiguousarray(np.stack([
            np.asarray(inputs["Wp1"], f32)[:D1, 0].reshape(NH, OUT),
            np.asarray(inputs["Wp1"], f32)[D1:, 0].reshape(NH, OUT)], -1)),
        "bp1": np.asarray(inputs["bp1"], f32).reshape(1, 1),
        "Wg2r": np.asarray(inputs["Wg2"], f32).reshape(NH, OUT, 1),
        "bg2": np.asarray(inputs["bg2"], f32).reshape(1, 1),
        "Wor": np.asarray(inputs["Wo"], f32).reshape(NH, OUT, OUT),
        "ident": np.eye(128, dtype=f32),
    }
    # selrep[t][r, p] = 1 iff r == 8t + p//16
    selrep = np.zeros((NPC, NC * 128), f32)
    for t in range(8):
        for p in range(128):
            selrep[8 * t + p // 16, 128 * t + p] = 1.0
    shared["selrep"] = selrep

    in_maps = []
    for c in range(NC):
        lo = c * NPC
        m = dict(shared)
        m["featTm"] = np.ascontiguousarray(featT[:, lo:lo + NPC])
        # slot grid XP [64, NPC*S] and gather idx
        mask = (us >= lo) & (us < lo + NPC)
        cs, cd, csl = us[mask] - lo, ud[mask], slots[mask]
        XP = np.zeros((NPC * S, HID), f32)
        XP[cs * S + csl] = n2n[uniq[mask]]
        m["XP"] = np.ascontiguousarray(XP.T)
        ptr = np.full((NPC, N), NPC * S, np.int64)
        ptr[cs, cd] = cs * S + csl
        g = np.zeros((128, 256), np.int16)
        for t in range(8):
            for gg in range(8):
                node = 8 * t + gg
                row = ptr[node]                      # [512]
                g[16 * gg:16 * gg + 16, 32 * t:32 * t + 32] = \
                    row.reshape(32, 16).T.astype(np.int16)
        m["gidx"] = g
        m["adjmine"] = np.ascontiguousarray(adjacency[lo:lo + NPC])
        m["eaT"] = np.ascontiguousarray(eaT[:, c * EPC:(c + 1) * EPC])
        in_maps.append(m)
    return in_maps, (src, dst)


def _prep_B(inputs, resA, ei_sd):
    f32 = np.float32
    src, dst = ei_sd
    unp = []
    for c in range(NC):
        o = resA[c]["o_all"]
        unp.append({"o_P": o[:, 0:16], "o_Wh2T": o[:, 16:48],
                    "o_hs0": o[0:HID, 48:49], "o_Z": o[0:1, 49:50],
                    "o_es": o[:, 50:58].reshape(-1)})
    resA = unp
    es = np.concatenate([resA[c]["o_es"].reshape(-1) for c in range(NC)])
    s2, d2 = src // 2, dst // 2
    adj2 = np.zeros((N2, N2), f32)
    adj2[s2, d2] = 1.0
    e3_2 = np.zeros((N2, N2), f32)
    e3_2[s2, d2] = es  # numpy fancy assignment: last occurrence wins
    Wh2T = np.concatenate([resA[c]["o_Wh2T"] for c in range(NC)], axis=1)
    Pall = np.concatenate([resA[c]["o_P"] for c in range(NC)], axis=1)
    Zall = np.concatenate([resA[c]["o_Z"].reshape(1, 1) for c in range(NC)], axis=1)

    # LSTM weights: my-x order = [hs1(2048), hs0(64), hs2(128), bias(1)]
    perm = np.concatenate([np.arange(64, 2112), np.arange(0, 64), np.arange(2112, 2240)])
    W0 = np.zeros((2, 18, 128, 4 * LH), f32)
    for d in range(2):
        wt = np.asarray(inputs["Wih0"], f32)[d].T[perm]         # [2240, 512]
        wb = np.concatenate([wt, np.asarray(inputs["b0"], f32)[d][None, :]], 0)  # [2241,512]
        for k in range(18):
            rows = wb[128 * k:128 * (k + 1)]
            W0[d, k, :rows.shape[0], :] = rows
    W1 = np.zeros((2, 3, 128, 4 * LH), f32)
    for d in range(2):
        wt = np.asarray(inputs["Wih1"], f32)[d].T               # [256, 512]
        wb = np.concatenate([wt, np.asarray(inputs["b1"], f32)[d][None, :]], 0)
        for k in range(3):
            rows = wb[128 * k:128 * (k + 1)]
            W1[d, k, :rows.shape[0], :] = rows
    import ml_dtypes
    bf = ml_dtypes.bfloat16

    return {
        "adjm2": adj2,
        "e3_2": e3_2,
        "Wh2T": np.ascontiguousarray(Wh2T),
        "Wh2nat": np.ascontiguousarray(Wh2T.T),
        "a12o": np.ascontiguousarray(np.stack(
            [np.asarray(inputs["a1_o"], f32), np.asarray(inputs["a2_o"], f32)], -1)),
        "wp2ab": np.ascontiguousarray(np.stack(
            [np.asarray(inputs["Wp2"], f32)[:OUT, 0], np.asarray(inputs["Wp2"], f32)[OUT:, 0]], -1)),
        "bp2": np.asarray(inputs["bp2"], f32).reshape(1, 1),
        "Wg3": np.asarray(inputs["Wg3"], f32).reshape(OUT, 1),
        "bg3": np.asarray(inputs["bg3"], f32).reshape(1, 1),
        "fcWr": np.asarray(inputs["fc_W"], f32).reshape(2, LH, NCLS, order="C")
                  if False else np.stack([np.asarray(inputs["fc_W"], f32)[:LH],
                                          np.asarray(inputs["fc_W"], f32)[LH:]]),
        "fcb": np.asarray(inputs["fc_b"], f32).reshape(1, NCLS),
        "Pall": np.ascontiguousarray(Pall),
        "Zall": np.ascontiguousarray(Zall),
        "hs0": resA[0]["o_hs0"].reshape(HID, 1),
        "W0b": W0.astype(bf),
        "W1b": W1.astype(bf),
        "ident": np.eye(128, dtype=f32),
    }


# ------------------------------------------------------- cached SPMD runner
class _CachedRunner:
    """Like bass2jax.run_bass_via_pjrt but with the jitted callable built once."""

    def __init__(self, nc, n_cores):
        import jax
        from jax.sharding import Mesh, PartitionSpec
        from jax.experimental.shard_map import shard_map
        from concourse import bass2jax
        bass2jax.install_neuronx_cc_hook()
        self.n_cores = n_cores
        partition_name = nc.partition_id_tensor.name if nc.partition_id_tensor else None
        in_names, out_names, out_avals, zero_outs = [], [], [], []
        for alloc in nc.m.functions[0].allocations:
            if not isinstance(alloc, mybir.MemoryLocationSet):
                continue
            name = alloc.memorylocations[0].name
            if alloc.kind == "ExternalInput":
                if name != partition_name:
                    in_names.append(name)
            elif alloc.kind == "ExternalOutput":
                shape = tuple(alloc.tensor_shape)
                dtype = mybir.dt.np(alloc.dtype)
                out_names.append(name)
                out_avals.append(jax.core.ShapedArray(shape, dtype))
                zero_outs.append(np.zeros(shape, dtype))
        self.in_names, self.out_names = in_names, out_names
        self.out_avals, self.zero_outs = out_avals, zero_outs
        n_params, n_outs = len(in_names), len(out_names)
        all_names = in_names + out_names
        if partition_name is not None:
            all_names = all_names + [partition_name]
        donate = tuple(range(n_params, n_params + n_outs))

        def _body(*args):
            operands = list(args)
            if partition_name is not None:
                operands.append(bass2jax.partition_id_tensor())
            outs = bass2jax._bass_exec_p.bind(
                *operands,
                out_avals=tuple(out_avals),
                in_names=tuple(all_names),
                out_names=tuple(out_names),
                lowering_input_output_aliases=(),
                sim_require_finite=True,
                sim_require_nnan=True,
                nc=nc,
            )
            return tuple(outs)

        if n_cores == 1:
            self.fn = jax.jit(_body, donate_argnums=donate, keep_unused=True)
        else:
            devices = jax.devices()[:n_cores]
            mesh = Mesh(np.asarray(devices), ("core",))
            in_specs = (PartitionSpec("core"),) * (n_params + n_outs)
            out_specs = (PartitionSpec("core"),) * n_outs
            self.fn = jax.jit(
                shard_map(_body, mesh=mesh, in_specs=in_specs,
                          out_specs=out_specs, check_rep=False),
                donate_argnums=donate, keep_unused=True)

    def __call__(self, in_maps):
        nc_ = self.n_cores
        if nc_ == 1:
            out = self.fn(*[np.asarray(in_maps[0][n]) for n in self.in_names],
                          *self.zero_outs)
            return [{n: np.asarray(out[i]) for i, n in enumerate(self.out_names)}]
        concat_in = [np.concatenate([np.asarray(in_maps[c][n]) for c in range(nc_)], axis=0)
                     for n in self.in_names]
        concat_zeros = [np.zeros((nc_ * z.shape[0], *z.shape[1:]), z.dtype)
                        for z in self.zero_outs]
        out = self.fn(*concat_in, *concat_zeros)
        res = []
        for c in range(nc_):
            res.append({n: np.asarray(out[i]).reshape(nc_, *self.out_avals[i].shape)[c]
                        for i, n in enumerate(self.out_names)})
        return res


# ---------------------------------------------------------------- entrypoint
def kernel(**inputs):
    if "A" not in _cache:
        _cache["A"] = _CachedRunner(build_A(), NC)
    if "B" not in _cache:
        _cache["B"] = _CachedRunner(build_B(), 1)
    in_maps, ei_sd = _prep_A(inputs)
    resA = _cache["A"](in_maps)
    inB = _prep_B(inputs, resA, ei_sd)
    resB = _cache["B"]([inB])
    return resB[0]["o_prob"].reshape(NCLS).astype(np.float32)


# revision 10
# speedup vs baseline: 2.4039x; 1.0793x over previous
"""Trainium2 Bass kernel for nn_DefectDetection (GAT + pooling + LSTM head).

Self-contained: accepts FULL inputs, shards across 8 NeuronCores internally.

Strategy:
  Dispatch A (8 cores, SPMD):
    - replicated small front-end (node-attention layer, gpool1, GAT projections)
    - node-row-sharded dense [N,N] attention maps (64 rows x 16 heads / core),
      with the sparse node2node e3 term built from a host-packed slot grid via
      one matmul + gpsimd ap_gather (no 64MiB dense read)
    - edge-sharded edge-attr score reduction (es)
    - per-core outputs: es slice, gpool2 partials (P,Z), Wh2 rows, hs0
  Host in between: pure data movement (concat / scatter by precomputed indices).
  Dispatch B (1 core): pooled-graph attention (256 nodes), edge pool 2, gpool3,
    2-layer bi-LSTM (T=1) with bf16 weights, fc + softmax -> [2].
"""
import numpy as np
from contextlib import ExitStack

import concourse.bass as bass
import concourse.bacc as bacc
import concourse.tile as tile
import concourse.mybir as mybir
from concourse.bass_utils import run_bass_kernel_spmd

F32 = mybir.dt.float32
BF16 = mybir.dt.bfloat16
I16 = mybir.dt.int16
AF = mybir.ActivationFunctionType
ALU = mybir.AluOpType
AX = mybir.AxisListType

N, E, HID, NH, OUT, NCLS, LH = 512, 8192, 64, 16, 128, 2, 128
NC = 8          # cores
NPC = N // NC   # 64 nodes per core
S = 64          # slot grid per node
EPC = E // NC   # 1024 edges per core (F stage)
D1 = NH * OUT   # 2048
N2 = N // 2     # 256
N3 = N // 4     # 128
JUMP = HID + D1 + OUT  # 2240

_cache = {}


def _ap(t, offset, dims):
    return bass.AP(tensor=t, offset=offset, ap=[list(d) for d in dims])


# ---------------------------------------------------------------- dispatch A
def build_A():
    nc = bacc.Bacc("TRN2", target_bir_lowering=False, debug=False, num_devices=NC)

    def inp(name, shape, dt=F32):
        return nc.dram_tensor(name, shape, dt, kind="ExternalInput").ap()

    def outp(name, shape, dt=F32):
        return nc.dram_tensor(name, shape, dt, kind="ExternalOutput").ap()

    featT = inp("featT", [HID, N])
    featTm = inp("featTm", [HID, NPC])
    W_sn = inp("W_sn", [HID, HID])
    a_sn = inp("a_sn", [HID, 1])
    Wg1 = inp("Wg1", [HID, 1])
    bg1 = inp("bg1", [1, 1])
    Wgat = inp("Wgat", [NH, HID, OUT])
    a12 = inp("a12", [NH, OUT, 2])
    a3t128 = inp("a3t128", [HID, 128])
    XP = inp("XP", [HID, NPC * S])
    gidx = inp("gidx", [128, 256], I16)
    adjmine = inp("adjmine", [NPC, N])
    selrep = inp("selrep", [NPC, NC * 128])
    selh2 = inp("selh2", [NH, 128])
    eaT = inp("eaT", [HID, EPC])
    Wegat = inp("Wegat", [NH, HID, OUT])
    a3oT = inp("a3oT", [OUT, NH])
    wp1ab = inp("wp1ab", [NH, OUT, 2])
    bp1 = inp("bp1", [1, 1])
    Wg2r = inp("Wg2r", [NH, OUT, 1])
    bg2 = inp("bg2", [1, 1])
    Wor = inp("Wor", [NH, OUT, OUT])
    ident = inp("ident", [128, 128])

    o_all = outp("o_all", [128, 58])

    with tile.TileContext(nc) as tc, ExitStack() as ctx:
        sb = ctx.enter_context(tc.tile_pool(name="sb", bufs=1))
        sb2 = ctx.enter_context(tc.tile_pool(name="sb2", bufs=2))
        sb3 = ctx.enter_context(tc.tile_pool(name="sb3", bufs=3))
        psa = ctx.enter_context(tc.tile_pool(name="psa", bufs=1, space="PSUM"))
        psb = ctx.enter_context(tc.tile_pool(name="psb", bufs=2, space="PSUM"))
        dram = ctx.enter_context(tc.tile_pool(name="dram", bufs=1, space="DRAM"))

        def load(apx, shape, dt=F32, pool=sb, tag=None):
            t = pool.tile(shape, dt, tag=tag)
            nc.sync.dma_start(t[:], apx)
            return t

        featT_s = load(featT[:], [HID, N], tag="featT")
        featTm_s = load(featTm[:], [HID, NPC], tag="featTm")
        Wsn_s = load(W_sn[:], [HID, HID], tag="Wsn")
        asn_s = load(a_sn[:], [HID, 1], tag="asn")
        Wg1_s = load(Wg1[:], [HID, 1], tag="Wg1")
        bg1_s = load(bg1[:], [1, 1], tag="bg1")
        ident_s = load(ident[:], [128, 128], tag="ident")
        a3t_s = load(a3t128[:], [HID, 128], tag="a3t")
        XP_s = load(XP[:], [HID, NPC * S], tag="XP")
        gidx_s = load(gidx[:], [128, 256], I16, tag="gidx")
        adjm_s = load(adjmine[:], [NPC, N], tag="adjm")
        selh2_s = load(selh2[:], [NH, 128], tag="selh2")
        eaT_s = load(eaT[:], [HID, EPC], tag="eaT")
        selrep_s = load(selrep[:], [NPC, NC * 128], tag="selrep")
        a3oT_s = load(a3oT[:], [OUT, NH], tag="a3oT")
        bp1_s = load(bp1[:], [1, 1], tag="bp1")
        bg2_s = load(bg2[:], [1, 1], tag="bg2")

        ones1_128 = sb.tile([1, 128], F32, tag="ones1")
        nc.gpsimd.memset(ones1_128[:], 1.0)
        ones128 = sb.tile([128, 1], F32, tag="ones128")
        nc.gpsimd.memset(ones128[:], 1.0)

        def elu_inplace(src_ps, dst_sb, shape, pool=sb2, tagp="elu"):
            """dst = elu(src) where src is PSUM [p,f]; dst SBUF."""
            p, f = shape
            ex = pool.tile([p, f], F32, tag=tagp + "_ex")
            nc.scalar.activation(ex[:], src_ps, AF.Exp)
            rl = pool.tile([p, f], F32, tag=tagp + "_rl")
            nc.scalar.activation(rl[:], src_ps, AF.Relu)
            # dst = (min(ex,1) + rl) - 1
            nc.vector.scalar_tensor_tensor(dst_sb, ex[:], 1.0, rl[:],
                                           op0=ALU.min, op1=ALU.add)
            nc.vector.tensor_scalar(dst_sb, dst_sb, 1.0, None, op0=ALU.subtract)

        # ---------------- front: h = elu(sigmoid(lrelu(Wh0@a))*Wh0)
        def front(ft, width, tag):
            wh0_ps = psb.tile([HID, width], F32, tag="mm")
            nc.tensor.matmul(wh0_ps[:], Wsn_s[:], ft, start=True, stop=True)
            wh0 = sb.tile([HID, width], F32, tag="wh0_" + tag)
            nc.scalar.copy(wh0[:], wh0_ps[:])
            ga_ps = psb.tile([1, width], F32, tag="mm")
            nc.tensor.matmul(ga_ps[:], asn_s[:], wh0[:], start=True, stop=True)
            gl = sb.tile([1, width], F32, tag="gl_" + tag)
            nc.scalar.activation(gl[:], ga_ps[:], AF.Lrelu, alpha=0.2)
            gs = sb.tile([1, width], F32, tag="gs_" + tag)
            nc.scalar.activation(gs[:], gl[:], AF.Sigmoid)
            grep_ps = psb.tile([HID, width], F32, tag="mm")
            nc.tensor.matmul(grep_ps[:], ones1_128[:, :HID], gs[:], start=True, stop=True)
            hpre = sb.tile([HID, width], F32, tag="hpre_" + tag)
            nc.vector.tensor_tensor(hpre[:], wh0[:], grep_ps[:], ALU.mult)
            ht = sb.tile([HID, width], F32, tag="ht_" + tag)
            elu_inplace(hpre[:], ht[:], [HID, width], tagp="eluf_" + tag)
            return ht

        hT = front(featT_s[:], N, "full")          # [64, 512]
        hTm = front(featTm_s[:], NPC, "mine")      # [64, 64]

        # ---------------- gpool1 -> hs0
        g1_ps = psb.tile([1, N], F32, tag="mm")
        nc.tensor.matmul(g1_ps[:], Wg1_s[:], hT[:], start=True, stop=True)
        g1s = sb.tile([1, N], F32, tag="g1s")
        nc.scalar.activation(g1s[:], g1_ps[:], AF.Sigmoid, bias=bg1_s[:])
        nmax1 = sb.tile([1, 1], F32, tag="nmax1")
        nc.vector.tensor_reduce(nmax1[:], g1s[:], AX.X, ALU.max, negate=True)
        w1 = sb.tile([1, N], F32, tag="w1")
        z1 = sb.tile([1, 1], F32, tag="z1")
        nc.scalar.activation(w1[:], g1s[:], AF.Exp, bias=nmax1[:], accum_out=z1[:])
        iz1 = sb.tile([1, 1], F32, tag="iz1")
        nc.vector.reciprocal(iz1[:], z1[:])
        nc.vector.tensor_scalar(w1[:], w1[:], iz1[:], None, op0=ALU.mult)
        w1rep_ps = psb.tile([HID, N], F32, tag="mm")
        nc.tensor.matmul(w1rep_ps[:], ones1_128[:, :HID], w1[:], start=True, stop=True)
        hw = sb.tile([HID, N], F32, tag="hw")
        nc.vector.tensor_tensor(hw[:], hT[:], w1rep_ps[:], ALU.mult)
        hs0 = sb.tile([HID, 1], F32, tag="hs0")
        nc.vector.tensor_reduce(hs0[:], hw[:], AX.X, ALU.add)
        nc.sync.dma_start(o_all[0:HID, 48:49], hs0[:])

        # ---------------- v12 = WgatT[h] @ a12[h]  -> vall [64, 32]
        vall = sb.tile([HID, 2 * NH], F32, tag="vall")
        for h in range(NH):
            wg0_s = sb2.tile([HID, OUT], F32, tag="wgT0")
            nc.sync.dma_start(wg0_s[:], Wgat[h])
            wgT_ps = psb.tile([OUT, HID], F32, tag="mm")
            nc.tensor.transpose(wgT_ps[:], wg0_s[:], ident_s[0:HID, 0:HID])
            wgT_s = sb2.tile([OUT, HID], F32, tag="wgT")
            nc.vector.tensor_copy(wgT_s[:], wgT_ps[:])
            a12_s = sb2.tile([OUT, 2], F32, tag="a12s")
            nc.sync.dma_start(a12_s[:], a12[h])
            v_ps = psb.tile([HID, 2], F32, tag="mm")
            nc.tensor.matmul(v_ps[:], wgT_s[:], a12_s[:], start=True, stop=True)
            nc.vector.tensor_copy(vall[:, 2 * h:2 * h + 2], v_ps[:])

        # s1mine [16, 64] / s2all [16, 512]
        v1_ap = _ap(vall[:].tensor, 0, [[2 * NH, HID], [2, NH]])
        v2_ap = _ap(vall[:].tensor, 1, [[2 * NH, HID], [2, NH]])
        s1m_ps = psb.tile([NH, NPC], F32, tag="mm")
        nc.tensor.matmul(s1m_ps[:], v1_ap, hTm[:], start=True, stop=True)
        s1m = sb.tile([NH, NPC], F32, tag="s1m")
        nc.vector.tensor_copy(s1m[:], s1m_ps[:])
        s2a_ps = psb.tile([NH, N], F32, tag="mm")
        nc.tensor.matmul(s2a_ps[:], v2_ap, hT[:], start=True, stop=True)
        s2a = sb.tile([NH, N], F32, tag="s2a")
        nc.vector.tensor_copy(s2a[:], s2a_ps[:])
        # s2rep [128, 512]: row p -> s2a[p%16]
        s2rep_ps = psa.tile([128, N], F32, tag="s2rep")
        nc.tensor.matmul(s2rep_ps[:], selh2_s[:], s2a[:], start=True, stop=True)
        s2rep = sb.tile([128, N], F32, tag="s2repsb")
        nc.vector.tensor_copy(s2rep[:], s2rep_ps[:])

        # s1col [128, 8] via DRAM bounce: scratch [16, 64]
        scr = dram.tile([NH, NPC], F32, tag="scr")
        nc.sync.dma_start(scr[:], s1m[:])
        s1col = sb.tile([128, NC], F32, tag="s1col")
        with nc.allow_non_contiguous_dma(reason="s1col 4B gather"):
            for i in range(8):
                src_ap = _ap(scr[:].tensor, i, [[NPC, NH], [8, 8]])
                nc.sync.dma_start(s1col[16 * i:16 * (i + 1), :], src_ap)

        # ---------------- sc = a3-scores on slot grid, replicated rows
        sc_sb = sb.tile([128, NPC * S + 1], F32, tag="scsb")
        for q in range(8):
            scq_ps = psb.tile([128, 512], F32, tag="mm")
            nc.tensor.matmul(scq_ps[:], a3t_s[:], XP_s[:, 512 * q:512 * (q + 1)],
                             start=True, stop=True)
            nc.vector.tensor_copy(sc_sb[:, 512 * q:512 * (q + 1)], scq_ps[:])
        nc.gpsimd.memset(sc_sb[:, NPC * S:NPC * S + 1], 0.0)

        # ---------------- F stage: es over my 1024 edges
        esA_ps = psa.tile([1, 512], F32, tag="accA")
        esB_ps = psa.tile([1, 512], F32, tag="accB")
        sumo_ps = psa.tile([1, 1], F32, tag="accC")
        es_ps = [esA_ps, esB_ps]
        for h in range(NH):
            weg_s = sb2.tile([HID, OUT], F32, tag="weg")
            nc.sync.dma_start(weg_s[:], Wegat[h])
            st, sp = (h == 0), (h == NH - 1)
            for half in range(2):
                T_ps = psb.tile([128, 512], F32, tag="mm")
                nc.tensor.matmul(T_ps[:], weg_s[:], eaT_s[:, 512 * half:512 * (half + 1)],
                                 start=True, stop=True)
                ex = sb2.tile([128, 512], F32, tag="Fex")
                nc.scalar.activation(ex[:], T_ps[:], AF.Exp)
                rl = sb2.tile([128, 512], F32, tag="Frl")
                nc.scalar.activation(rl[:], T_ps[:], AF.Relu)
                eluP = sb2.tile([128, 512], F32, tag="eluP")
                nc.vector.scalar_tensor_tensor(eluP[:], ex[:], 1.0, rl[:],
                                               op0=ALU.min, op1=ALU.add)
                nc.tensor.matmul(es_ps[half][:], a3oT_s[:, h:h + 1], eluP[:],
                                 start=st, stop=sp)
            nc.tensor.matmul(sumo_ps[:], a3oT_s[:, h:h + 1], ones128[:], start=st, stop=sp)
        sumo = sb.tile([1, 1], F32, tag="sumosb")
        nc.vector.tensor_copy(sumo[:], sumo_ps[:])
        es_sb = sb.tile([1, EPC], F32, tag="essb")
        nc.vector.tensor_scalar(es_sb[:, :512], esA_ps[:], sumo[:], None, op0=ALU.subtract)
        nc.vector.tensor_scalar(es_sb[:, 512:], esB_ps[:], sumo[:], None, op0=ALU.subtract)
        dst_es = _ap(o_all.tensor, 50, [[58, 128], [1, 8]])
        nc.sync.dma_start(dst_es, es_sb[:])

        # ---------------- e-stage: 8 tiles [128 (i*16+h), 512]
        att_tiles = []
        for t in range(8):
            e3g = sb2.tile([128, N], F32, tag="e3g")
            nc.gpsimd.ap_gather(e3g[:], sc_sb[:], gidx_s[:, 32 * t:32 * (t + 1)],
                                channels=128, num_elems=NPC * S + 1, d=1, num_idxs=N)
            e1 = sb2.tile([128, N], F32, tag="e1")
            nc.vector.tensor_tensor(e1[:], e3g[:], s2rep[:], ALU.add)
            lr = sb2.tile([128, N], F32, tag="lr")
            nc.scalar.activation(lr[:], e1[:], AF.Lrelu, bias=s1col[:, t:t + 1], alpha=0.2)
            adjrep_ps = psb.tile([128, N], F32, tag="mm")
            nc.tensor.matmul(adjrep_ps[:], selrep_s[:, 128 * t:128 * (t + 1)], adjm_s[:], start=True, stop=True)
            m1 = sb2.tile([128, N], F32, tag="m1")
            nc.vector.scalar_tensor_tensor(m1[:], lr[:], 1e9, adjrep_ps[:],
                                           op0=ALU.add, op1=ALU.mult)
            nmax = sb2.tile([128, 1], F32, tag="nmax")
            nc.vector.tensor_reduce(nmax[:], m1[:], AX.X, ALU.max, negate=True)
            pt = sb2.tile([128, N], F32, tag="pt")
            zt = sb2.tile([128, 1], F32, tag="zt")
            nc.scalar.activation(pt[:], m1[:], AF.Exp, bias=nmax[:], accum_out=zt[:])
            izt = sb2.tile([128, 1], F32, tag="izt")
            nc.vector.reciprocal(izt[:], zt[:])
            att = sb.tile([128, N], F32, tag=f"att{t}")
            nc.vector.tensor_scalar(att[:], pt[:], izt[:], None, op0=ALU.mult)
            att_tiles.append(att)

        # transposes -> attT[jc] [128, 1024] cols = t*128 + (i*16+h)
        attT = []
        for jc in range(4):
            bigt = sb.tile([128, 1024], F32, tag=f"attT{jc}")
            attT.append(bigt)
        for t in range(8):
            for jc in range(4):
                tp_ps = psb.tile([128, 128], F32, tag="mm")
                nc.tensor.transpose(tp_ps[:], att_tiles[t][:, 128 * jc:128 * (jc + 1)],
                                    ident_s[:])
                nc.vector.tensor_copy(attT[jc][:, 128 * t:128 * (t + 1)], tp_ps[:])

        # AV per head + elu
        hGelu = []
        for h in range(NH):
            wg_s = sb2.tile([HID, OUT], F32, tag="wgnat")
            nc.sync.dma_start(wg_s[:], Wgat[h])
            hg_ps = psa.tile([OUT, NPC], F32, tag="hg")
            for jc in range(4):
                wh_ps = psb.tile([128, OUT], F32, tag="mm")
                nc.tensor.matmul(wh_ps[:], hT[:, 128 * jc:128 * (jc + 1)], wg_s[:],
                                 start=True, stop=True)
                wh_sb = sb2.tile([128, OUT], F32, tag="whsb")
                nc.vector.tensor_copy(wh_sb[:], wh_ps[:])
                rhs = _ap(attT[jc][:].tensor, h, [[1024, 128], [128, 8], [16, 8]])
                nc.tensor.matmul(hg_ps[:], wh_sb[:], rhs, start=(jc == 0), stop=(jc == 3))
            hg = sb.tile([OUT, NPC], F32, tag=f"hgelu{h}")
            elu_inplace(hg_ps[:], hg[:], [OUT, NPC], tagp="elug")
            hGelu.append(hg)

        # pair gates
        dpa_ps = psa.tile([1, NPC], F32, tag="accA")
        dpb_ps = psa.tile([1, NPC], F32, tag="accB")
        for h in range(NH):
            wp_s = sb2.tile([OUT, 2], F32, tag="wps")
            nc.sync.dma_start(wp_s[:], wp1ab[h])
            st, sp = (h == 0), (h == NH - 1)
            nc.tensor.matmul(dpa_ps[:], wp_s[:, 0:1], hGelu[h][:], start=st, stop=sp)
            nc.tensor.matmul(dpb_ps[:], wp_s[:, 1:2], hGelu[h][:], start=st, stop=sp)
        dk = sb.tile([1, NPC // 2], F32, tag="dk")
        dasb = sb.tile([1, NPC], F32, tag="dasb")
        nc.vector.tensor_copy(dasb[:], dpa_ps[:])
        a_ap = _ap(dasb[:].tensor, 0, [[NPC, 1], [2, NPC // 2]])
        b_ap = _ap(dpb_ps[:].tensor, 1, [[NPC, 1], [2, NPC // 2]])
        nc.vector.tensor_tensor(dk[:], a_ap, b_ap, ALU.add)
        sgate = sb.tile([1, NPC // 2], F32, tag="sgate")
        nc.scalar.activation(sgate[:], dk[:], AF.Sigmoid, bias=bp1_s[:])
        srep_ps = psa.tile([128, NPC // 2], F32, tag="accC")
        nc.tensor.matmul(srep_ps[:], ones1_128[:], sgate[:], start=True, stop=True)

        h1T = []
        for h in range(NH):
            ev_ap = _ap(hGelu[h][:].tensor, 0, [[NPC, OUT], [2, NPC // 2]])
            od_ap = _ap(hGelu[h][:].tensor, 1, [[NPC, OUT], [2, NPC // 2]])
            t1 = sb2.tile([OUT, NPC // 2], F32, tag="pairsum")
            nc.vector.tensor_tensor(t1[:], ev_ap, od_ap, ALU.add)
            h1 = sb.tile([OUT, NPC // 2], F32, tag=f"h1T{h}")
            nc.vector.tensor_tensor(h1[:], t1[:], srep_ps[:], ALU.mult)
            h1T.append(h1)

        # g2 / u / Z / P
        g2_ps = psa.tile([1, NPC // 2], F32, tag="accA")
        for h in range(NH):
            wg2_s = sb2.tile([OUT, 1], F32, tag="wg2s")
            nc.sync.dma_start(wg2_s[:], Wg2r[h])
            nc.tensor.matmul(g2_ps[:], wg2_s[:], h1T[h][:],
                             start=(h == 0), stop=(h == NH - 1))
        sg2 = sb.tile([1, NPC // 2], F32, tag="sg2")
        nc.scalar.activation(sg2[:], g2_ps[:], AF.Sigmoid, bias=bg2_s[:])
        u = sb.tile([1, NPC // 2], F32, tag="u")
        nc.scalar.activation(u[:], sg2[:], AF.Exp)
        Zc = sb.tile([1, 1], F32, tag="Zc")
        nc.vector.tensor_reduce(Zc[:], u[:], AX.X, ALU.add)
        nc.sync.dma_start(o_all[0:1, 49:50], Zc[:])
        urep_ps = psa.tile([128, NPC // 2], F32, tag="accB")
        nc.tensor.matmul(urep_ps[:], ones1_128[:], u[:], start=True, stop=True)
        Pout = sb.tile([OUT, NH], F32, tag="Pout")
        for h in range(NH):
            pm = sb2.tile([OUT, NPC // 2], F32, tag="pm")
            nc.vector.tensor_tensor(pm[:], h1T[h][:], urep_ps[:OUT, :], ALU.mult)
            nc.vector.tensor_reduce(Pout[:, h:h + 1], pm[:], AX.X, ALU.add)
        nc.sync.dma_start(o_all[:, 0:16], Pout[:])

        # Wh2T rows
        wh2_ps = psa.tile([OUT, NPC // 2], F32, tag="accC")
        for h in range(NH):
            wo_s = sb2.tile([OUT, OUT], F32, tag="wos")
            nc.sync.dma_start(wo_s[:], Wor[h])
            nc.tensor.matmul(wh2_ps[:], wo_s[:], h1T[h][:],
                             start=(h == 0), stop=(h == NH - 1))
        wh2 = sb.tile([OUT, NPC // 2], F32, tag="wh2sb")
        nc.vector.tensor_copy(wh2[:], wh2_ps[:])
        nc.sync.dma_start(o_all[:, 16:48], wh2[:])

    nc.compile()
    return nc


# ---------------------------------------------------------------- dispatch B
def build_B():
    nc = bacc.Bacc("TRN2", target_bir_lowering=False, debug=False, num_devices=1)

    def inp(name, shape, dt=F32):
        return nc.dram_tensor(name, shape, dt, kind="ExternalInput").ap()

    adjm2 = inp("adjm2", [N2, N2])
    e3_2 = inp("e3_2", [N2, N2])
    Wh2T = inp("Wh2T", [OUT, N2])
    Wh2nat = inp("Wh2nat", [N2, OUT])
    a12o = inp("a12o", [OUT, 2])
    wp2ab = inp("wp2ab", [OUT, 2])
    bp2 = inp("bp2", [1, 1])
    Wg3 = inp("Wg3", [OUT, 1])
    bg3 = inp("bg3", [1, 1])
    fcWr = inp("fcWr", [2, LH, NCLS])
    fcb = inp("fcb", [1, NCLS])
    Pall = inp("Pall", [OUT, NC * NH])
    Zall = inp("Zall", [1, NC])
    hs0 = inp("hs0", [HID, 1])
    W0b = inp("W0b", [2, 18, 128, 4 * LH], BF16)   # row-chunked lhsT, bias row folded
    W1b = inp("W1b", [2, 3, 128, 4 * LH], BF16)
    ident = inp("ident", [128, 128])
    o_prob = nc.dram_tensor("o_prob", [1, NCLS], F32, kind="ExternalOutput").ap()

    with tile.TileContext(nc) as tc, ExitStack() as ctx:
        sb = ctx.enter_context(tc.tile_pool(name="sb", bufs=1))
        sb2 = ctx.enter_context(tc.tile_pool(name="sb2", bufs=2))
        psa = ctx.enter_context(tc.tile_pool(name="psa", bufs=1, space="PSUM"))
        psb = ctx.enter_context(tc.tile_pool(name="psb", bufs=2, space="PSUM"))

        def load(apx, shape, dt=F32, pool=sb, tag=None):
            t = pool.tile(shape, dt, tag=tag)
            nc.sync.dma_start(t[:], apx)
            return t

        ident_s = load(ident[:], [128, 128], tag="ident")
        ones1 = sb.tile([1, 128], F32, tag="ones1")
        nc.gpsimd.memset(ones1[:], 1.0)
        Pall_s = load(Pall[:], [OUT, NC * NH], tag="Pall")
        Zall_s = load(Zall[:], [1, NC], tag="Zall")
        hs0_s = load(hs0[:], [HID, 1], tag="hs0")
        Wh2T_s = load(Wh2T[:], [OUT, N2], tag="Wh2T")
        a12o_s = load(a12o[:], [OUT, 2], tag="a12o")
        wp2_s = load(wp2ab[:], [OUT, 2], tag="wp2")
        bp2_s = load(bp2[:], [1, 1], tag="bp2")
        Wg3_s = load(Wg3[:], [OUT, 1], tag="Wg3")
        bg3_s = load(bg3[:], [1, 1], tag="bg3")
        fcb_s = load(fcb[:], [1, NCLS], tag="fcb")

        # hs1 columns [128, 16] = sum_c Pall[:, c*16+h] / Z
        hs1c = sb.tile([OUT, NH], F32, tag="hs1c")
        src = _ap(Pall_s[:].tensor, 0, [[NC * NH, OUT], [1, NH], [NH, NC]])
        nc.vector.tensor_reduce(hs1c[:], src, AX.X, ALU.add)
        Zt = sb.tile([1, 1], F32, tag="Zt")
        nc.vector.tensor_reduce(Zt[:], Zall_s[:], AX.X, ALU.add)
        iZ = sb.tile([1, 1], F32, tag="iZ")
        nc.vector.reciprocal(iZ[:], Zt[:])
        izrep_ps = psa.tile([128, 1], F32, tag="r1")
        nc.tensor.matmul(izrep_ps[:], ones1[:], iZ[:], start=True, stop=True)
        izcol = sb.tile([128, 1], F32, tag="izcol")
        nc.vector.tensor_copy(izcol[:], izrep_ps[:])
        nc.vector.tensor_scalar(hs1c[:], hs1c[:], izcol[:OUT, :], None, op0=ALU.mult)

        # att2 scores
        s1o_ps = psa.tile([1, N2], F32, tag="r2")
        nc.tensor.matmul(s1o_ps[:], a12o_s[:, 0:1], Wh2T_s[:], start=True, stop=True)
        s2o_ps = psa.tile([1, N2], F32, tag="r3")
        nc.tensor.matmul(s2o_ps[:], a12o_s[:, 1:2], Wh2T_s[:], start=True, stop=True)
        s1o = sb.tile([1, N2], F32, tag="s1osb")
        nc.vector.tensor_copy(s1o[:], s1o_ps[:])
        s2o = sb.tile([1, N2], F32, tag="s2osb")
        nc.vector.tensor_copy(s2o[:], s2o_ps[:])
        s2orep_ps = psa.tile([128, N2], F32, tag="r4")
        nc.tensor.matmul(s2orep_ps[:], ones1[:], s2o[:], start=True, stop=True)

        att2 = []
        for t2 in range(2):
            s1c_ps = psb.tile([128, 1], F32, tag="mmB")
            nc.tensor.transpose(s1c_ps[:], s1o[:, 128 * t2:128 * (t2 + 1)], ident_s[0:1, 0:1])
            s1c = sb2.tile([128, 1], F32, tag="s1c")
            nc.vector.tensor_copy(s1c[:], s1c_ps[:])
            e3t = sb2.tile([128, N2], F32, tag="e3t")
            nc.sync.dma_start(e3t[:], e3_2[128 * t2:128 * (t2 + 1), :])
            adt = sb2.tile([128, N2], F32, tag="adt")
            nc.sync.dma_start(adt[:], adjm2[128 * t2:128 * (t2 + 1), :])
            e1 = sb2.tile([128, N2], F32, tag="e1b")
            nc.vector.tensor_tensor(e1[:], e3t[:], s2orep_ps[:], ALU.add)
            lr = sb2.tile([128, N2], F32, tag="lrb")
            nc.scalar.activation(lr[:], e1[:], AF.Lrelu, bias=s1c[:], alpha=0.2)
            m1 = sb2.tile([128, N2], F32, tag="m1b")
            nc.vector.scalar_tensor_tensor(m1[:], lr[:], 1e9, adt[:],
                                           op0=ALU.add, op1=ALU.mult)
            nmax = sb2.tile([128, 1], F32, tag="nmaxb")
            nc.vector.tensor_reduce(nmax[:], m1[:], AX.X, ALU.max, negate=True)
            pt = sb2.tile([128, N2], F32, tag="ptb")
            zt = sb2.tile([128, 1], F32, tag="ztb")
            nc.scalar.activation(pt[:], m1[:], AF.Exp, bias=nmax[:], accum_out=zt[:])
            izt = sb2.tile([128, 1], F32, tag="iztb")
            nc.vector.reciprocal(izt[:], zt[:])
            at = sb.tile([128, N2], F32, tag=f"att2_{t2}")
            nc.vector.tensor_scalar(at[:], pt[:], izt[:], None, op0=ALU.mult)
            att2.append(at)

        # att2T + h2T
        attT2 = []
        for lc in range(2):
            big = sb.tile([128, N2], F32, tag=f"attT2_{lc}")
            attT2.append(big)
        for t2 in range(2):
            for lc in range(2):
                tp_ps = psb.tile([128, 128], F32, tag="mmB")
                nc.tensor.transpose(tp_ps[:], att2[t2][:, 128 * lc:128 * (lc + 1)],
                                    ident_s[:])
                nc.vector.tensor_copy(attT2[lc][:, 128 * t2:128 * (t2 + 1)], tp_ps[:])
        h2_ps = psa.tile([OUT, N2], F32, tag="r5")
        for lc in range(2):
            w2n_s = sb2.tile([128, OUT], F32, tag="w2n")
            nc.sync.dma_start(w2n_s[:], Wh2nat[128 * lc:128 * (lc + 1), :])
            nc.tensor.matmul(h2_ps[:], w2n_s[:], attT2[lc][:],
                             start=(lc == 0), stop=(lc == 1))
        h2T = sb.tile([OUT, N2], F32, tag="h2T")
        nc.vector.tensor_copy(h2T[:], h2_ps[:])

        # edge pool 2
        dpa_ps = psa.tile([1, N2], F32, tag="r1")
        nc.tensor.matmul(dpa_ps[:], wp2_s[:, 0:1], h2T[:], start=True, stop=True)
        dpb_ps = psa.tile([1, N2], F32, tag="r2")
        nc.tensor.matmul(dpb_ps[:], wp2_s[:, 1:2], h2T[:], start=True, stop=True)
        dk2 = sb.tile([1, N3], F32, tag="dk2")
        dasb2 = sb.tile([1, N2], F32, tag="dasb2")
        nc.vector.tensor_copy(dasb2[:], dpa_ps[:])
        a_ap = _ap(dasb2[:].tensor, 0, [[N2, 1], [2, N3]])
        b_ap = _ap(dpb_ps[:].tensor, 1, [[N2, 1], [2, N3]])
        nc.vector.tensor_tensor(dk2[:], a_ap, b_ap, ALU.add)
        s2k = sb.tile([1, N3], F32, tag="s2k")
        nc.scalar.activation(s2k[:], dk2[:], AF.Sigmoid, bias=bp2_s[:])
        srep2_ps = psa.tile([128, N3], F32, tag="r3")
        nc.tensor.matmul(srep2_ps[:], ones1[:], s2k[:], start=True, stop=True)
        ev_ap = _ap(h2T[:].tensor, 0, [[N2, OUT], [2, N3]])
        od_ap = _ap(h2T[:].tensor, 1, [[N2, OUT], [2, N3]])
        t12 = sb.tile([OUT, N3], F32, tag="t12")
        nc.vector.tensor_tensor(t12[:], ev_ap, od_ap, ALU.add)
        h3T = sb.tile([OUT, N3], F32, tag="h3T")
        nc.vector.tensor_tensor(h3T[:], t12[:], srep2_ps[:OUT, :], ALU.mult)

        # gpool3 -> hs2 [128, 1]
        g3_ps = psa.tile([1, N3], F32, tag="r1")
        nc.tensor.matmul(g3_ps[:], Wg3_s[:], h3T[:], start=True, stop=True)
        g3s = sb.tile([1, N3], F32, tag="g3s")
        nc.scalar.activation(g3s[:], g3_ps[:], AF.Sigmoid, bias=bg3_s[:])
        nm3 = sb.tile([1, 1], F32, tag="nm3")
        nc.vector.tensor_reduce(nm3[:], g3s[:], AX.X, ALU.max, negate=True)
        w3 = sb.tile([1, N3], F32, tag="w3")
        z3 = sb.tile([1, 1], F32, tag="z3")
        nc.scalar.activation(w3[:], g3s[:], AF.Exp, bias=nm3[:], accum_out=z3[:])
        iz3 = sb.tile([1, 1], F32, tag="iz3")
        nc.vector.reciprocal(iz3[:], z3[:])
        nc.vector.tensor_scalar(w3[:], w3[:], iz3[:], None, op0=ALU.mult)
        w3rep_ps = psa.tile([128, N3], F32, tag="r2")
        nc.tensor.matmul(w3rep_ps[:], ones1[:], w3[:], start=True, stop=True)
        hw3 = sb.tile([OUT, N3], F32, tag="hw3")
        nc.vector.tensor_tensor(hw3[:], h3T[:], w3rep_ps[:OUT, :], ALU.mult)
        hs2 = sb.tile([OUT, 1], F32, tag="hs2")
        nc.vector.tensor_reduce(hs2[:], hw3[:], AX.X, ALU.add)

        # x chunks [128, 18] bf16: cols 0-15 hs1c, col16 [hs0; hs2[0:64]], col17 [hs2[64:]; 1]
        xc = sb.tile([128, 18], F32, tag="xc")
        nc.gpsimd.memset(xc[:], 0.0)
        nc.vector.tensor_copy(xc[:OUT, 0:NH], hs1c[:])
        nc.vector.tensor_copy(xc[:HID, 16:17], hs0_s[:])
        nc.sync.dma_start(xc[HID:128, 16:17], hs2[0:HID, :])
        nc.sync.dma_start(xc[0:HID, 17:18], hs2[HID:OUT, :])
        nc.gpsimd.memset(xc[HID:HID + 1, 17:18], 1.0)
        xcb = sb.tile([128, 18], BF16, tag="xcb")
        nc.vector.tensor_copy(xcb[:], xc[:])

        # LSTM layer 0 (M-orientation, skip f-gate m=1)
        h0 = []
        for d in range(2):
            g_ps = psa.tile([128, 4], F32, tag="gacc")
            for m in (0, 2, 3):
                for k in range(18):
                    rows = 65 if k == 17 else 128
                    w_s = sb2.tile([128, 128], BF16, tag="w0s")
                    nc.sync.dma_start(w_s[:rows, :], W0b[d, k, 0:rows, 128 * m:128 * (m + 1)])
                    nc.tensor.matmul(g_ps[:, m:m + 1], w_s[:rows, :], xcb[:rows, k:k + 1],
                                     start=(k == 0), stop=(k == 17))
            si = sb2.tile([128, 1], F32, tag="si")
            nc.scalar.activation(si[:], g_ps[:, 0:1], AF.Sigmoid)
            tg = sb2.tile([128, 1], F32, tag="tg")
            nc.scalar.activation(tg[:], g_ps[:, 2:3], AF.Tanh)
            so = sb2.tile([128, 1], F32, tag="so")
            nc.scalar.activation(so[:], g_ps[:, 3:4], AF.Sigmoid)
            c = sb2.tile([128, 1], F32, tag="c0")
            nc.vector.tensor_tensor(c[:], si[:], tg[:], ALU.mult)
            tc_ = sb2.tile([128, 1], F32, tag="tc0")
            nc.scalar.activation(tc_[:], c[:], AF.Tanh)
            hd = sb.tile([128, 1], F32, tag=f"h0_{d}")
            nc.vector.tensor_tensor(hd[:], so[:], tc_[:], ALU.mult)
            h0.append(hd)
        h0b_ = []
        for d in range(2):
            hb = sb.tile([128, 1], BF16, tag=f"h0b_{d}")
            nc.vector.tensor_copy(hb[:], h0[d][:])
            h0b_.append(hb)
        onesb = sb.tile([1, 1], BF16, tag="onesb")
        nc.gpsimd.memset(onesb[:], 1.0)

        # LSTM layer 1
        h1o = []
        for d in range(2):
            g_ps = psa.tile([128, 4], F32, tag="gacc")
            for m in (0, 2, 3):
                for k in range(3):
                    rows = 1 if k == 2 else 128
                    w_s = sb2.tile([128, 128], BF16, tag="w1s")
                    nc.sync.dma_start(w_s[:rows, :], W1b[d, k, 0:rows, 128 * m:128 * (m + 1)])
                    rhs = onesb[:] if k == 2 else h0b_[k][:]
                    nc.tensor.matmul(g_ps[:, m:m + 1], w_s[:rows, :], rhs,
                                     start=(k == 0), stop=(k == 2))
            si = sb2.tile([128, 1], F32, tag="si1")
            nc.scalar.activation(si[:], g_ps[:, 0:1], AF.Sigmoid)
            tg = sb2.tile([128, 1], F32, tag="tg1")
            nc.scalar.activation(tg[:], g_ps[:, 2:3], AF.Tanh)
            so = sb2.tile([128, 1], F32, tag="so1")
            nc.scalar.activation(so[:], g_ps[:, 3:4], AF.Sigmoid)
            c = sb2.tile([128, 1], F32, tag="c1")
            nc.vector.tensor_tensor(c[:], si[:], tg[:], ALU.mult)
            tc_ = sb2.tile([128, 1], F32, tag="tc1")
            nc.scalar.activation(tc_[:], c[:], AF.Tanh)
            hd = sb.tile([128, 1], F32, tag=f"h1_{d}")
            nc.vector.tensor_tensor(hd[:], so[:], tc_[:], ALU.mult)
            h1o.append(hd)

        # fc + softmax
        lg_ps = psa.tile([1, NCLS], F32, tag="r1")
        fcw0 = sb.tile([LH, NCLS], F32, tag="fcw0")
        nc.sync.dma_start(fcw0[:], fcWr[0])
        fcw1 = sb.tile([LH, NCLS], F32, tag="fcw1")
        nc.sync.dma_start(fcw1[:], fcWr[1])
        nc.tensor.matmul(lg_ps[:], h1o[0][:], fcw0[:], start=True, stop=False)
        nc.tensor.matmul(lg_ps[:], h1o[1][:], fcw1[:], start=False, stop=True)
        lg = sb.tile([1, NCLS], F32, tag="lg")
        nc.vector.tensor_tensor(lg[:], lg_ps[:], fcb_s[:], ALU.add)
        nmf = sb.tile([1, 1], F32, tag="nmf")
        nc.vector.tensor_reduce(nmf[:], lg[:], AX.X, ALU.max, negate=True)
        pf = sb.tile([1, NCLS], F32, tag="pf")
        zf = sb.tile([1, 1], F32, tag="zf")
        nc.scalar.activation(pf[:], lg[:], AF.Exp, bias=nmf[:], accum_out=zf[:])
        izf = sb.tile([1, 1], F32, tag="izf")
        nc.vector.reciprocal(izf[:], zf[:])
        prob = sb.tile([1, NCLS], F32, tag="prob")
        nc.vector.tensor_scalar(prob[:], pf[:], izf[:], None, op0=ALU.mult)
        nc.sync.dma_start(o_prob[:], prob[:])

    nc.compile()
    return nc


# ---------------------------------------------------------------- host prep
def _prep_A(inputs):
    """Build per-core input maps for dispatch A. Pure layout/indexing."""
    f32 = np.float32
    ei = np.asarray(inputs["edge_index"])
    feats = np.asarray(inputs["features"], f32)
    n2n = np.asarray(inputs["node2node_features"], f32)
    eattr = np.asarray(inputs["edgesAttr"], f32)
    adjacency = np.asarray(inputs["adjacency"], f32)

    src, dst = np.asarray(ei[0], np.int64), np.asarray(ei[1], np.int64)
    pairs = src * N + dst
    uniq = np.unique(pairs)
    us, ud = uniq // N, uniq % N
    # slot assignment per source node
    order = np.argsort(us, kind="stable")
    us, ud, uniq = us[order], ud[order], uniq[order]
    counts = np.bincount(us, minlength=N)
    assert counts.max() <= S, f"out-degree {counts.max()} > {S}"
    starts = np.zeros(N + 1, np.int64)
    np.cumsum(counts, out=starts[1:])
    slots = np.arange(len(us)) - starts[us]

    featT = np.ascontiguousarray(feats.T)
    eaT = np.ascontiguousarray(eattr.T)
    W_gat = np.asarray(inputs["W_gat"], f32)
    shared = {
        "featT": featT,
        "W_sn": np.asarray(inputs["W_sn"], f32),
        "a_sn": np.asarray(inputs["a_sn"], f32).reshape(HID, 1),
        "Wg1": np.asarray(inputs["Wg1"], f32).reshape(HID, 1),
        "bg1": np.asarray(inputs["bg1"], f32).reshape(1, 1),
        "Wgat": W_gat,
        "a12": np.ascontiguousarray(np.stack(
            [np.asarray(inputs["a1_gat"], f32), np.asarray(inputs["a2_gat"], f32)], -1)),
        "a3t128": np.ascontiguousarray(np.tile(np.asarray(inputs["a3_gat"], f32).T, (1, 8))),
        "selh2": np.eye(NH, dtype=f32)[:, np.tile(np.arange(NH), 8)].reshape(NH, 128),
        "Wegat": np.asarray(inputs["We_gat"], f32),
        "a3oT": np.ascontiguousarray(np.asarray(inputs["a3_o"], f32).reshape(NH, OUT).T),
        "wp1ab": np.ascontiguousarray(np.stack([
            np.asarray(inputs["Wp1"], f32)[:D1, 0].reshape(NH, OUT),
            np.asarray(inputs["Wp1"], f32)[D1:, 0].reshape(NH, OUT)], -1)),
        "bp1": np.asarray(inputs["bp1"], f32).reshape(1, 1),
        "Wg2r": np.asarray(inputs["Wg2"], f32).reshape(NH, OUT, 1),
        "bg2": np.asarray(inputs["bg2"], f32).reshape(1, 1),
        "Wor": np.asarray(inputs["Wo"], f32).reshape(NH, OUT, OUT),
        "ident": np.eye(128, dtype=f32),
    }
    # selrep[t][r, p] = 1 iff r == 8t + p//16
    selrep = np.zeros((NPC, NC * 128), f32)
    for t in range(8):
        for p in range(128):
            selrep[8 * t + p // 16, 128 * t + p] = 1.0
    shared["selrep"] = selrep

    in_maps = []
    for c in range(NC):
        lo = c * NPC
        m = dict(shared)
        m["featTm"] = np.ascontiguousarray(featT[:, lo:lo + NPC])
        # slot grid XP [64, NPC*S] and gather idx
        mask = (us >= lo) & (us < lo + NPC)
        cs, cd, csl = us[mask] - lo, ud[mask], slots[mask]
        XP = np.zeros((NPC * S, HID), f32)
        XP[cs * S + csl] = n2n[uniq[mask]]
        m["XP"] = np.ascontiguousarray(XP.T)
        ptr = np.full((NPC, N), NPC * S, np.int64)
        ptr[cs, cd] = cs * S + csl
        g = np.zeros((128, 256), np.int16)
        for t in range(8):
            for gg in range(8):
                node = 8 * t + gg
                row = ptr[node]                      # [512]
                g[16 * gg:16 * gg + 16, 32 * t:32 * t + 32] = \
                    row.reshape(32, 16).T.astype(np.int16)
        m["gidx"] = g
        m["adjmine"] = np.ascontiguousarray(adjacency[lo:lo + NPC])
        m["eaT"] = np.ascontiguousarray(eaT[:, c * EPC:(c + 1) * EPC])
        in_maps.append(m)
    return in_maps, (src, dst)


def _prep_B(inputs, resA, ei_sd):
    f32 = np.float32
    src, dst = ei_sd
    unp = []
    for c in range(NC):
        o = resA[c]["o_all"]
        unp.append({"o_P": o[:, 0:16], "o_Wh2T": o[:, 16:48],
                    "o_hs0": o[0:HID, 48:49], "o_Z": o[0:1, 49:50],
                    "o_es": o[:, 50:58].reshape(-1)})
    resA = unp
    es = np.concatenate([resA[c]["o_es"].reshape(-1) for c in range(NC)])
    s2, d2 = src // 2, dst // 2
    adj2 = np.zeros((N2, N2), f32)
    adj2[s2, d2] = 1.0
    e3_2 = np.zeros((N2, N2), f32)
    e3_2[s2, d2] = es  # numpy fancy assignment: last occurrence wins
    Wh2T = np.concatenate([resA[c]["o_Wh2T"] for c in range(NC)], axis=1)
    Pall = np.concatenate([resA[c]["o_P"] for c in range(NC)], axis=1)
    Zall = np.concatenate([resA[c]["o_Z"].reshape(1, 1) for c in range(NC)], axis=1)

    # LSTM weights: my-x order = [hs1(2048), hs0(64), hs2(128), bias(1)]
    perm = np.concatenate([np.arange(64, 2112), np.arange(0, 64), np.arange(2112, 2240)])
    W0 = np.zeros((2, 18, 128, 4 * LH), f32)
    for d in range(2):
        wt = np.asarray(inputs["Wih0"], f32)[d].T[perm]         # [2240, 512]
        wb = np.concatenate([wt, np.asarray(inputs["b0"], f32)[d][None, :]], 0)  # [2241,512]
        for k in range(18):
            rows = wb[128 * k:128 * (k + 1)]
            W0[d, k, :rows.shape[0], :] = rows
    W1 = np.zeros((2, 3, 128, 4 * LH), f32)
    for d in range(2):
        wt = np.asarray(inputs["Wih1"], f32)[d].T               # [256, 512]
        wb = np.concatenate([wt, np.asarray(inputs["b1"], f32)[d][None, :]], 0)
        for k in range(3):
            rows = wb[128 * k:128 * (k + 1)]
            W1[d, k, :rows.shape[0], :] = rows
    import ml_dtypes
    bf = ml_dtypes.bfloat16

    return {
        "adjm2": adj2,
        "e3_2": e3_2,
        "Wh2T": np.ascontiguousarray(Wh2T),
        "Wh2nat": np.ascontiguousarray(Wh2T.T),
        "a12o": np.ascontiguousarray(np.stack(
            [np.asarray(inputs["a1_o"], f32), np.asarray(inputs["a2_o"], f32)], -1)),
        "wp2ab": np.ascontiguousarray(np.stack(
            [np.asarray(inputs["Wp2"], f32)[:OUT, 0], np.asarray(inputs["Wp2"], f32)[OUT:, 0]], -1)),
        "bp2": np.asarray(inputs["bp2"], f32).reshape(1, 1),
        "Wg3": np.asarray(inputs["Wg3"], f32).reshape(OUT, 1),
        "bg3": np.asarray(inputs["bg3"], f32).reshape(1, 1),
        "fcWr": np.asarray(inputs["fc_W"], f32).reshape(2, LH, NCLS, order="C")
                  if False else np.stack([np.asarray(inputs["fc_W"], f32)[:LH],
                                          np.asarray(inputs["fc_W"], f32)[LH:]]),
        "fcb": np.asarray(inputs["fc_b"], f32).reshape(1, NCLS),
        "Pall": np.ascontiguousarray(Pall),
        "Zall": np.ascontiguousarray(Zall),
        "hs0": resA[0]["o_hs0"].reshape(HID, 1),
        "W0b": W0.astype(bf),
        "W1b": W1.astype(bf),
        "ident": np.eye(128, dtype=f32),
    }


# ------------------------------------------------------- cached SPMD runner
class _CachedRunner:
    """Like bass2jax.run_bass_via_pjrt but with the jitted callable built once."""

    def __init__(self, nc, n_cores):
        import jax
        from jax.sharding import Mesh, PartitionSpec
        from jax.experimental.shard_map import shard_map
        from concourse import bass2jax
        bass2jax.install_neuronx_cc_hook()
        self.n_cores = n_cores
        partition_name = nc.partition_id_tensor.name if nc.partition_id_tensor else None
        in_names, out_names, out_avals, zero_outs = [], [], [], []
        for alloc in nc.m.functions[0].allocations:
            if not isinstance(alloc, mybir.MemoryLocationSet):
                continue
            name = alloc.memorylocations[0].name
            if alloc.kind == "ExternalInput":
                if name != partition_name:
                    in_names.append(name)
            elif alloc.kind == "ExternalOutput":
                shape = tuple(alloc.tensor_shape)
                dtype = mybir.dt.np(alloc.dtype)
                out_names.append(name)
                out_avals.append(jax.core.ShapedArray(shape, dtype))
                zero_outs.append(np.zeros(shape, dtype))
        self.in_names, self.out_names = in_names, out_names
        self.out_avals, self.zero_outs = out_avals, zero_outs
        n_params, n_outs = len(in_names), len(out_names)
        all_names = in_names + out_names
        if partition_name is not None:
            all_names = all_names + [partition_name]
        donate = tuple(range(n_params, n_params + n_outs))

        def _body(*args):
            operands = list(args)
            if partition_name is not None:
                operands.append(bass2jax.partition_id_tensor())
            outs = bass2jax._bass_exec_p.bind(
                *operands,
                out_avals=tuple(out_avals),
                in_names=tuple(all_names),
                out_names=tuple(out_names),
                lowering_input_output_aliases=(),
                sim_require_finite=True,
                sim_require_nnan=True,
                nc=nc,
            )
            return tuple(outs)

        self._body = _body
        self._jax = jax
        self._Mesh, self._P, self._shard_map = Mesh, PartitionSpec, shard_map
        self.donate = donate
        self.n_params, self.n_outs = n_params, n_outs
        self.fn = None
        if n_cores == 1:
            self.fn = jax.jit(_body, donate_argnums=donate, keep_unused=True)

    def _build_multi(self, shared_flags):
        jax = self._jax
        devices = jax.devices()[:self.n_cores]
        mesh = self._Mesh(np.asarray(devices), ("core",))
        self.shared_flags = shared_flags
        in_specs = tuple(self._P() if f else self._P("core") for f in shared_flags) \
            + (self._P("core"),) * self.n_outs
        out_specs = (self._P("core"),) * self.n_outs
        self.fn = jax.jit(
            self._shard_map(self._body, mesh=mesh, in_specs=in_specs,
                            out_specs=out_specs, check_rep=False),
            donate_argnums=self.donate, keep_unused=True)

    def __call__(self, in_maps):
        nc_ = self.n_cores
        if nc_ == 1:
            out = self.fn(*[np.asarray(in_maps[0][n]) for n in self.in_names],
                          *self.zero_outs)
            return [{n: np.asarray(out[i]) for i, n in enumerate(self.out_names)}]
        if self.fn is None:
            flags = [all(in_maps[c][n] is in_maps[0][n] for c in range(nc_))
                     for n in self.in_names]
            self._build_multi(flags)
        args = []
        for i, n in enumerate(self.in_names):
            if self.shared_flags[i]:
                args.append(np.asarray(in_maps[0][n]))
            else:
                args.append(np.concatenate(
                    [np.asarray(in_maps[c][n]) for c in range(nc_)], axis=0))
        concat_zeros = [np.zeros((nc_ * z.shape[0], *z.shape[1:]), z.dtype)
                        for z in self.zero_outs]
        out = self.fn(*args, *concat_zeros)
        res = []
        for c in range(nc_):
            res.append({n: np.asarray(out[i]).reshape(nc_, *self.out_avals[i].shape)[c]
                        for i, n in enumerate(self.out_names)})
        return res


# ---------------------------------------------------------------- entrypoint
def kernel(**inputs):
    if "A" not in _cache:
        _cache["A"] = _CachedRunner(build_A(), NC)
    if "B" not in _cache:
        _cache["B"] = _CachedRunner(build_B(), 1)
    in_maps, ei_sd = _prep_A(inputs)
    resA = _cache["A"](in_maps)
    inB = _prep_B(inputs, resA, ei_sd)
    resB = _cache["B"]([inB])
    return resB[0]["o_prob"].reshape(NCLS).astype(np.float32)


# revision 12
# speedup vs baseline: 3.9097x; 1.6264x over previous
"""Trainium2 Bass kernel for nn_DefectDetection (GAT + pooling + LSTM head).

Self-contained: accepts FULL inputs, shards across 8 NeuronCores internally.

Strategy:
  Dispatch A (8 cores, SPMD):
    - replicated small front-end (node-attention layer, gpool1, GAT projections)
    - node-row-sharded dense [N,N] attention maps (64 rows x 16 heads / core),
      with the sparse node2node e3 term built from a host-packed slot grid via
      one matmul + gpsimd ap_gather (no 64MiB dense read)
    - edge-sharded edge-attr score reduction (es)
    - per-core outputs: es slice, gpool2 partials (P,Z), Wh2 rows, hs0
  Host in between: pure data movement (concat / scatter by precomputed indices).
  Dispatch B (1 core): pooled-graph attention (256 nodes), edge pool 2, gpool3,
    2-layer bi-LSTM (T=1) with bf16 weights, fc + softmax -> [2].
"""
import numpy as np
from contextlib import ExitStack

import concourse.bass as bass
import concourse.bacc as bacc
import concourse.tile as tile
import concourse.mybir as mybir
from concourse.bass_utils import run_bass_kernel_spmd

F32 = mybir.dt.float32
BF16 = mybir.dt.bfloat16
I16 = mybir.dt.int16
AF = mybir.ActivationFunctionType
ALU = mybir.AluOpType
AX = mybir.AxisListType

N, E, HID, NH, OUT, NCLS, LH = 512, 8192, 64, 16, 128, 2, 128
NC = 8          # cores
NPC = N // NC   # 64 nodes per core
S = 64          # slot grid per node
EPC = E // NC   # 1024 edges per core (F stage)
D1 = NH * OUT   # 2048
N2 = N // 2     # 256
N3 = N // 4     # 128
JUMP = HID + D1 + OUT  # 2240

_cache = {}

# blob layouts: (name, shape) -> row-major at running offset
SPEC32 = [
    ("featT", (HID, N)), ("featTm", (HID, NPC)), ("W_sn", (HID, HID)),
    ("a_sn", (HID, 1)), ("Wg1", (HID, 1)), ("bg1", (1, 1)),
    ("a12", (NH, OUT, 2)), ("a3t128", (HID, 128)), ("a3oT", (OUT, NH)),
    ("wp1ab", (NH, OUT, 2)), ("Wg2r", (NH, OUT, 1)), ("bp1", (1, 1)),
    ("bg2", (1, 1)), ("selh2", (NH, 128)), ("gidxbits", (128, 128)),
]
SPEC16 = [
    ("Wgat", (NH, HID, OUT)), ("Wegat", (NH, HID, OUT)), ("Wor", (NH, OUT, OUT)),
    ("XP", (HID, NPC * S)), ("eaT", (HID, EPC)), ("adjmine", (NPC, N)),
    ("selrep", (NPC, NC * 128)), ("ident", (128, 128)),
]


def _offsets(spec):
    out, off = {}, 0
    for name, shape in spec:
        n = int(np.prod(shape))
        out[name] = (off, shape)
        off += n
    return out, off

OFF32, LEN32 = _offsets(SPEC32)
OFF16, LEN16 = _offsets(SPEC16)



def _ap(t, offset, dims):
    return bass.AP(tensor=t, offset=offset, ap=[list(d) for d in dims])


# ---------------------------------------------------------------- dispatch A
def build_A():
    nc = bacc.Bacc("TRN2", target_bir_lowering=False, debug=False, num_devices=NC)

    def inp(name, shape, dt=F32):
        return nc.dram_tensor(name, shape, dt, kind="ExternalInput").ap()

    def outp(name, shape, dt=F32):
        return nc.dram_tensor(name, shape, dt, kind="ExternalOutput").ap()

    blob32 = inp("blob32", [LEN32])
    blob16 = inp("blob16", [LEN16], BF16)

    def b32(name, head=None):
        off, shape = OFF32[name]
        if head is not None:
            per = int(np.prod(shape[1:]))
            off, shape = off + head * per, shape[1:]
        rows, cols = (shape[0], int(np.prod(shape[1:]))) if len(shape) > 1 else (1, shape[0])
        return _ap(blob32.tensor, off, [[cols, rows], [1, cols]])

    def b16(name, head=None):
        off, shape = OFF16[name]
        if head is not None:
            per = int(np.prod(shape[1:]))
            off, shape = off + head * per, shape[1:]
        rows, cols = (shape[0], int(np.prod(shape[1:]))) if len(shape) > 1 else (1, shape[0])
        return _ap(blob16.tensor, off, [[cols, rows], [1, cols]])

    o_all = outp("o_all", [128, 58])

    with tile.TileContext(nc) as tc, ExitStack() as ctx:
        sb = ctx.enter_context(tc.tile_pool(name="sb", bufs=1))
        sb2 = ctx.enter_context(tc.tile_pool(name="sb2", bufs=2))
        sb3 = ctx.enter_context(tc.tile_pool(name="sb3", bufs=3))
        psa = ctx.enter_context(tc.tile_pool(name="psa", bufs=1, space="PSUM"))
        psb = ctx.enter_context(tc.tile_pool(name="psb", bufs=2, space="PSUM"))
        dram = ctx.enter_context(tc.tile_pool(name="dram", bufs=1, space="DRAM"))

        def load(apx, shape, dt=F32, pool=sb, tag=None):
            t = pool.tile(shape, dt, tag=tag)
            nc.sync.dma_start(t[:], apx)
            return t

        def load16(name, shape, tag):
            t = sb.tile(shape, F32, tag=tag)
            nc.gpsimd.dma_start(t[:], b16(name))
            return t

        featT_s = load(b32("featT"), [HID, N], tag="featT")
        featTm_s = load(b32("featTm"), [HID, NPC], tag="featTm")
        Wsn_s = load(b32("W_sn"), [HID, HID], tag="Wsn")
        asn_s = load(b32("a_sn"), [HID, 1], tag="asn")
        Wg1_s = load(b32("Wg1"), [HID, 1], tag="Wg1")
        bg1_s = load(b32("bg1"), [1, 1], tag="bg1")
        ident_s = load16("ident", [128, 128], tag="ident")
        a3t_s = load(b32("a3t128"), [HID, 128], tag="a3t")
        XP_s = load16("XP", [HID, NPC * S], tag="XP")
        gidxf_s = load(b32("gidxbits"), [128, 128], tag="gidx")
        gidx_s = None
        adjm_s = load16("adjmine", [NPC, N], tag="adjm")
        selh2_s = load(b32("selh2"), [NH, 128], tag="selh2")
        eaT_s = load16("eaT", [HID, EPC], tag="eaT")
        selrep_s = load16("selrep", [NPC, NC * 128], tag="selrep")
        a3oT_s = load(b32("a3oT"), [OUT, NH], tag="a3oT")
        bp1_s = load(b32("bp1"), [1, 1], tag="bp1")
        bg2_s = load(b32("bg2"), [1, 1], tag="bg2")

        ones1_128 = sb.tile([1, 128], F32, tag="ones1")
        nc.gpsimd.memset(ones1_128[:], 1.0)
        ones128 = sb.tile([128, 1], F32, tag="ones128")
        nc.gpsimd.memset(ones128[:], 1.0)

        def elu_inplace(src_ps, dst_sb, shape, pool=sb2, tagp="elu"):
            """dst = elu(src) where src is PSUM [p,f]; dst SBUF."""
            p, f = shape
            ex = pool.tile([p, f], F32, tag=tagp + "_ex")
            nc.scalar.activation(ex[:], src_ps, AF.Exp)
            rl = pool.tile([p, f], F32, tag=tagp + "_rl")
            nc.scalar.activation(rl[:], src_ps, AF.Relu)
            # dst = (min(ex,1) + rl) - 1
            nc.vector.scalar_tensor_tensor(dst_sb, ex[:], 1.0, rl[:],
                                           op0=ALU.min, op1=ALU.add)
            nc.vector.tensor_scalar(dst_sb, dst_sb, 1.0, None, op0=ALU.subtract)

        # ---------------- front: h = elu(sigmoid(lrelu(Wh0@a))*Wh0)
        def front(ft, width, tag):
            wh0_ps = psb.tile([HID, width], F32, tag="mm")
            nc.tensor.matmul(wh0_ps[:], Wsn_s[:], ft, start=True, stop=True)
            wh0 = sb.tile([HID, width], F32, tag="wh0_" + tag)
            nc.scalar.copy(wh0[:], wh0_ps[:])
            ga_ps = psb.tile([1, width], F32, tag="mm")
            nc.tensor.matmul(ga_ps[:], asn_s[:], wh0[:], start=True, stop=True)
            gl = sb.tile([1, width], F32, tag="gl_" + tag)
            nc.scalar.activation(gl[:], ga_ps[:], AF.Lrelu, alpha=0.2)
            gs = sb.tile([1, width], F32, tag="gs_" + tag)
            nc.scalar.activation(gs[:], gl[:], AF.Sigmoid)
            grep_ps = psb.tile([HID, width], F32, tag="mm")
            nc.tensor.matmul(grep_ps[:], ones1_128[:, :HID], gs[:], start=True, stop=True)
            hpre = sb.tile([HID, width], F32, tag="hpre_" + tag)
            nc.vector.tensor_tensor(hpre[:], wh0[:], grep_ps[:], ALU.mult)
            ht = sb.tile([HID, width], F32, tag="ht_" + tag)
            elu_inplace(hpre[:], ht[:], [HID, width], tagp="eluf_" + tag)
            return ht

        hT = front(featT_s[:], N, "full")          # [64, 512]
        hTm = front(featTm_s[:], NPC, "mine")      # [64, 64]

        # ---------------- gpool1 -> hs0
        g1_ps = psb.tile([1, N], F32, tag="mm")
        nc.tensor.matmul(g1_ps[:], Wg1_s[:], hT[:], start=True, stop=True)
        g1s = sb.tile([1, N], F32, tag="g1s")
        nc.scalar.activation(g1s[:], g1_ps[:], AF.Sigmoid, bias=bg1_s[:])
        nmax1 = sb.tile([1, 1], F32, tag="nmax1")
        nc.vector.tensor_reduce(nmax1[:], g1s[:], AX.X, ALU.max, negate=True)
        w1 = sb.tile([1, N], F32, tag="w1")
        z1 = sb.tile([1, 1], F32, tag="z1")
        nc.scalar.activation(w1[:], g1s[:], AF.Exp, bias=nmax1[:], accum_out=z1[:])
        iz1 = sb.tile([1, 1], F32, tag="iz1")
        nc.vector.reciprocal(iz1[:], z1[:])
        nc.vector.tensor_scalar(w1[:], w1[:], iz1[:], None, op0=ALU.mult)
        w1rep_ps = psb.tile([HID, N], F32, tag="mm")
        nc.tensor.matmul(w1rep_ps[:], ones1_128[:, :HID], w1[:], start=True, stop=True)
        hw = sb.tile([HID, N], F32, tag="hw")
        nc.vector.tensor_tensor(hw[:], hT[:], w1rep_ps[:], ALU.mult)
        hs0 = sb.tile([HID, 1], F32, tag="hs0")
        nc.vector.tensor_reduce(hs0[:], hw[:], AX.X, ALU.add)
        nc.sync.dma_start(o_all[0:HID, 48:49], hs0[:])

        # ---------------- v12 = WgatT[h] @ a12[h]  -> vall [64, 32]
        vall = sb.tile([HID, 2 * NH], F32, tag="vall")
        for h in range(NH):
            wg0_s = sb2.tile([HID, OUT], F32, tag="wgT0")
            nc.gpsimd.dma_start(wg0_s[:], b16('Wgat', h))
            wgT_ps = psb.tile([OUT, HID], F32, tag="mm")
            nc.tensor.transpose(wgT_ps[:], wg0_s[:], ident_s[0:HID, 0:HID])
            wgT_s = sb2.tile([OUT, HID], F32, tag="wgT")
            nc.vector.tensor_copy(wgT_s[:], wgT_ps[:])
            a12_s = sb2.tile([OUT, 2], F32, tag="a12s")
            nc.sync.dma_start(a12_s[:], b32('a12', h))
            v_ps = psb.tile([HID, 2], F32, tag="mm")
            nc.tensor.matmul(v_ps[:], wgT_s[:], a12_s[:], start=True, stop=True)
            nc.vector.tensor_copy(vall[:, 2 * h:2 * h + 2], v_ps[:])

        # s1mine [16, 64] / s2all [16, 512]
        v1_ap = _ap(vall[:].tensor, 0, [[2 * NH, HID], [2, NH]])
        v2_ap = _ap(vall[:].tensor, 1, [[2 * NH, HID], [2, NH]])
        s1m_ps = psb.tile([NH, NPC], F32, tag="mm")
        nc.tensor.matmul(s1m_ps[:], v1_ap, hTm[:], start=True, stop=True)
        s1m = sb.tile([NH, NPC], F32, tag="s1m")
        nc.vector.tensor_copy(s1m[:], s1m_ps[:])
        s2a_ps = psb.tile([NH, N], F32, tag="mm")
        nc.tensor.matmul(s2a_ps[:], v2_ap, hT[:], start=True, stop=True)
        s2a = sb.tile([NH, N], F32, tag="s2a")
        nc.vector.tensor_copy(s2a[:], s2a_ps[:])
        # s2rep [128, 512]: row p -> s2a[p%16]
        s2rep_ps = psa.tile([128, N], F32, tag="s2rep")
        nc.tensor.matmul(s2rep_ps[:], selh2_s[:], s2a[:], start=True, stop=True)
        s2rep = sb.tile([128, N], F32, tag="s2repsb")
        nc.vector.tensor_copy(s2rep[:], s2rep_ps[:])

        # s1col [128, 8] via DRAM bounce: scratch [16, 64]
        scr = dram.tile([NH, NPC], F32, tag="scr")
        nc.sync.dma_start(scr[:], s1m[:])
        s1col = sb.tile([128, NC], F32, tag="s1col")
        with nc.allow_non_contiguous_dma(reason="s1col 4B gather"):
            for i in range(8):
                src_ap = _ap(scr[:].tensor, i, [[NPC, NH], [8, 8]])
                nc.sync.dma_start(s1col[16 * i:16 * (i + 1), :], src_ap)

        # ---------------- sc = a3-scores on slot grid, replicated rows
        sc_sb = sb.tile([128, NPC * S + 1], F32, tag="scsb")
        for q in range(8):
            scq_ps = psb.tile([128, 512], F32, tag="mm")
            nc.tensor.matmul(scq_ps[:], a3t_s[:], XP_s[:, 512 * q:512 * (q + 1)],
                             start=True, stop=True)
            nc.vector.tensor_copy(sc_sb[:, 512 * q:512 * (q + 1)], scq_ps[:])
        nc.gpsimd.memset(sc_sb[:, NPC * S:NPC * S + 1], 0.0)

        # ---------------- F stage: es over my 1024 edges
        esA_ps = psa.tile([1, 512], F32, tag="accA")
        esB_ps = psa.tile([1, 512], F32, tag="accB")
        sumo_ps = psa.tile([1, 1], F32, tag="accC")
        es_ps = [esA_ps, esB_ps]
        for h in range(NH):
            weg_s = sb2.tile([HID, OUT], F32, tag="weg")
            nc.gpsimd.dma_start(weg_s[:], b16('Wegat', h))
            st, sp = (h == 0), (h == NH - 1)
            for half in range(2):
                T_ps = psb.tile([128, 512], F32, tag="mm")
                nc.tensor.matmul(T_ps[:], weg_s[:], eaT_s[:, 512 * half:512 * (half + 1)],
                                 start=True, stop=True)
                ex = sb2.tile([128, 512], F32, tag="Fex")
                nc.scalar.activation(ex[:], T_ps[:], AF.Exp)
                rl = sb2.tile([128, 512], F32, tag="Frl")
                nc.scalar.activation(rl[:], T_ps[:], AF.Relu)
                eluP = sb2.tile([128, 512], F32, tag="eluP")
                nc.vector.scalar_tensor_tensor(eluP[:], ex[:], 1.0, rl[:],
                                               op0=ALU.min, op1=ALU.add)
                nc.tensor.matmul(es_ps[half][:], a3oT_s[:, h:h + 1], eluP[:],
                                 start=st, stop=sp)
            nc.tensor.matmul(sumo_ps[:], a3oT_s[:, h:h + 1], ones128[:], start=st, stop=sp)
        sumo = sb.tile([1, 1], F32, tag="sumosb")
        nc.vector.tensor_copy(sumo[:], sumo_ps[:])
        es_sb = sb.tile([1, EPC], F32, tag="essb")
        nc.vector.tensor_scalar(es_sb[:, :512], esA_ps[:], sumo[:], None, op0=ALU.subtract)
        nc.vector.tensor_scalar(es_sb[:, 512:], esB_ps[:], sumo[:], None, op0=ALU.subtract)
        dst_es = _ap(o_all.tensor, 50, [[58, 128], [1, 8]])
        nc.sync.dma_start(dst_es, es_sb[:])

        # ---------------- e-stage: 8 tiles [128 (i*16+h), 512]
        att_tiles = []
        for t in range(8):
            e3g = sb2.tile([128, N], F32, tag="e3g")
            nc.gpsimd.ap_gather(e3g[:], sc_sb[:], gidxf_s[:].bitcast(I16)[:, 32 * t:32 * (t + 1)],
                                channels=128, num_elems=NPC * S + 1, d=1, num_idxs=N)
            e1 = sb2.tile([128, N], F32, tag="e1")
            nc.vector.tensor_tensor(e1[:], e3g[:], s2rep[:], ALU.add)
            lr = sb2.tile([128, N], F32, tag="lr")
            nc.scalar.activation(lr[:], e1[:], AF.Lrelu, bias=s1col[:, t:t + 1], alpha=0.2)
            adjrep_ps = psb.tile([128, N], F32, tag="mm")
            nc.tensor.matmul(adjrep_ps[:], selrep_s[:, 128 * t:128 * (t + 1)], adjm_s[:], start=True, stop=True)
            m1 = sb2.tile([128, N], F32, tag="m1")
            nc.vector.scalar_tensor_tensor(m1[:], lr[:], 1e9, adjrep_ps[:],
                                           op0=ALU.add, op1=ALU.mult)
            nmax = sb2.tile([128, 1], F32, tag="nmax")
            nc.vector.tensor_reduce(nmax[:], m1[:], AX.X, ALU.max, negate=True)
            pt = sb2.tile([128, N], F32, tag="pt")
            zt = sb2.tile([128, 1], F32, tag="zt")
            nc.scalar.activation(pt[:], m1[:], AF.Exp, bias=nmax[:], accum_out=zt[:])
            izt = sb2.tile([128, 1], F32, tag="izt")
            nc.vector.reciprocal(izt[:], zt[:])
            att = sb.tile([128, N], F32, tag=f"att{t}")
            nc.vector.tensor_scalar(att[:], pt[:], izt[:], None, op0=ALU.mult)
            att_tiles.append(att)

        # transposes -> attT[jc] [128, 1024] cols = t*128 + (i*16+h)
        attT = []
        for jc in range(4):
            bigt = sb.tile([128, 1024], F32, tag=f"attT{jc}")
            attT.append(bigt)
        for t in range(8):
            for jc in range(4):
                tp_ps = psb.tile([128, 128], F32, tag="mm")
                nc.tensor.transpose(tp_ps[:], att_tiles[t][:, 128 * jc:128 * (jc + 1)],
                                    ident_s[:])
                nc.vector.tensor_copy(attT[jc][:, 128 * t:128 * (t + 1)], tp_ps[:])

        # AV per head + elu
        hGelu = []
        for h in range(NH):
            wg_s = sb2.tile([HID, OUT], F32, tag="wgnat")
            nc.gpsimd.dma_start(wg_s[:], b16('Wgat', h))
            hg_ps = psa.tile([OUT, NPC], F32, tag="hg")
            for jc in range(4):
                wh_ps = psb.tile([128, OUT], F32, tag="mm")
                nc.tensor.matmul(wh_ps[:], hT[:, 128 * jc:128 * (jc + 1)], wg_s[:],
                                 start=True, stop=True)
                wh_sb = sb2.tile([128, OUT], F32, tag="whsb")
                nc.vector.tensor_copy(wh_sb[:], wh_ps[:])
                rhs = _ap(attT[jc][:].tensor, h, [[1024, 128], [128, 8], [16, 8]])
                nc.tensor.matmul(hg_ps[:], wh_sb[:], rhs, start=(jc == 0), stop=(jc == 3))
            hg = sb.tile([OUT, NPC], F32, tag=f"hgelu{h}")
            elu_inplace(hg_ps[:], hg[:], [OUT, NPC], tagp="elug")
            hGelu.append(hg)

        # pair gates
        dpa_ps = psa.tile([1, NPC], F32, tag="accA")
        dpb_ps = psa.tile([1, NPC], F32, tag="accB")
        for h in range(NH):
            wp_s = sb2.tile([OUT, 2], F32, tag="wps")
            nc.sync.dma_start(wp_s[:], b32('wp1ab', h))
            st, sp = (h == 0), (h == NH - 1)
            nc.tensor.matmul(dpa_ps[:], wp_s[:, 0:1], hGelu[h][:], start=st, stop=sp)
            nc.tensor.matmul(dpb_ps[:], wp_s[:, 1:2], hGelu[h][:], start=st, stop=sp)
        dk = sb.tile([1, NPC // 2], F32, tag="dk")
        dasb = sb.tile([1, NPC], F32, tag="dasb")
        nc.vector.tensor_copy(dasb[:], dpa_ps[:])
        a_ap = _ap(dasb[:].tensor, 0, [[NPC, 1], [2, NPC // 2]])
        b_ap = _ap(dpb_ps[:].tensor, 1, [[NPC, 1], [2, NPC // 2]])
        nc.vector.tensor_tensor(dk[:], a_ap, b_ap, ALU.add)
        sgate = sb.tile([1, NPC // 2], F32, tag="sgate")
        nc.scalar.activation(sgate[:], dk[:], AF.Sigmoid, bias=bp1_s[:])
        srep_ps = psa.tile([128, NPC // 2], F32, tag="accC")
        nc.tensor.matmul(srep_ps[:], ones1_128[:], sgate[:], start=True, stop=True)

        h1T = []
        for h in range(NH):
            ev_ap = _ap(hGelu[h][:].tensor, 0, [[NPC, OUT], [2, NPC // 2]])
            od_ap = _ap(hGelu[h][:].tensor, 1, [[NPC, OUT], [2, NPC // 2]])
            t1 = sb2.tile([OUT, NPC // 2], F32, tag="pairsum")
            nc.vector.tensor_tensor(t1[:], ev_ap, od_ap, ALU.add)
            h1 = sb.tile([OUT, NPC // 2], F32, tag=f"h1T{h}")
            nc.vector.tensor_tensor(h1[:], t1[:], srep_ps[:], ALU.mult)
            h1T.append(h1)

        # g2 / u / Z / P
        g2_ps = psa.tile([1, NPC // 2], F32, tag="accA")
        for h in range(NH):
            wg2_s = sb2.tile([OUT, 1], F32, tag="wg2s")
            nc.sync.dma_start(wg2_s[:], b32('Wg2r', h))
            nc.tensor.matmul(g2_ps[:], wg2_s[:], h1T[h][:],
                             start=(h == 0), stop=(h == NH - 1))
        sg2 = sb.tile([1, NPC // 2], F32, tag="sg2")
        nc.scalar.activation(sg2[:], g2_ps[:], AF.Sigmoid, bias=bg2_s[:])
        u = sb.tile([1, NPC // 2], F32, tag="u")
        nc.scalar.activation(u[:], sg2[:], AF.Exp)
        Zc = sb.tile([1, 1], F32, tag="Zc")
        nc.vector.tensor_reduce(Zc[:], u[:], AX.X, ALU.add)
        nc.sync.dma_start(o_all[0:1, 49:50], Zc[:])
        urep_ps = psa.tile([128, NPC // 2], F32, tag="accB")
        nc.tensor.matmul(urep_ps[:], ones1_128[:], u[:], start=True, stop=True)
        Pout = sb.tile([OUT, NH], F32, tag="Pout")
        for h in range(NH):
            pm = sb2.tile([OUT, NPC // 2], F32, tag="pm")
            nc.vector.tensor_tensor(pm[:], h1T[h][:], urep_ps[:OUT, :], ALU.mult)
            nc.vector.tensor_reduce(Pout[:, h:h + 1], pm[:], AX.X, ALU.add)
        nc.sync.dma_start(o_all[:, 0:16], Pout[:])

        # Wh2T rows
        wh2_ps = psa.tile([OUT, NPC // 2], F32, tag="accC")
        for h in range(NH):
            wo_s = sb2.tile([OUT, OUT], F32, tag="wos")
            nc.gpsimd.dma_start(wo_s[:], b16('Wor', h))
            nc.tensor.matmul(wh2_ps[:], wo_s[:], h1T[h][:],
                             start=(h == 0), stop=(h == NH - 1))
        wh2 = sb.tile([OUT, NPC // 2], F32, tag="wh2sb")
        nc.vector.tensor_copy(wh2[:], wh2_ps[:])
        nc.sync.dma_start(o_all[:, 16:48], wh2[:])

    nc.compile()
    return nc


# ---------------------------------------------------------------- dispatch B
def build_B():
    nc = bacc.Bacc("TRN2", target_bir_lowering=False, debug=False, num_devices=1)

    def inp(name, shape, dt=F32):
        return nc.dram_tensor(name, shape, dt, kind="ExternalInput").ap()

    adjm2 = inp("adjm2", [N2, N2])
    e3_2 = inp("e3_2", [N2, N2])
    Wh2T = inp("Wh2T", [OUT, N2])
    Wh2nat = inp("Wh2nat", [N2, OUT])
    a12o = inp("a12o", [OUT, 2])
    wp2ab = inp("wp2ab", [OUT, 2])
    bp2 = inp("bp2", [1, 1])
    Wg3 = inp("Wg3", [OUT, 1])
    bg3 = inp("bg3", [1, 1])
    fcWr = inp("fcWr", [2, LH, NCLS])
    fcb = inp("fcb", [1, NCLS])
    Pall = inp("Pall", [OUT, NC * NH])
    Zall = inp("Zall", [1, NC])
    hs0 = inp("hs0", [HID, 1])
    W0b = inp("W0b", [2, 18, 128, 4 * LH], BF16)   # row-chunked lhsT, bias row folded
    W1b = inp("W1b", [2, 3, 128, 4 * LH], BF16)
    ident = inp("ident", [128, 128])
    o_prob = nc.dram_tensor("o_prob", [1, NCLS], F32, kind="ExternalOutput").ap()

    with tile.TileContext(nc) as tc, ExitStack() as ctx:
        sb = ctx.enter_context(tc.tile_pool(name="sb", bufs=1))
        sb2 = ctx.enter_context(tc.tile_pool(name="sb2", bufs=2))
        psa = ctx.enter_context(tc.tile_pool(name="psa", bufs=1, space="PSUM"))
        psb = ctx.enter_context(tc.tile_pool(name="psb", bufs=2, space="PSUM"))

        def load(apx, shape, dt=F32, pool=sb, tag=None):
            t = pool.tile(shape, dt, tag=tag)
            nc.sync.dma_start(t[:], apx)
            return t

        ident_s = load(ident[:], [128, 128], tag="ident")
        ones1 = sb.tile([1, 128], F32, tag="ones1")
        nc.gpsimd.memset(ones1[:], 1.0)
        Pall_s = load(Pall[:], [OUT, NC * NH], tag="Pall")
        Zall_s = load(Zall[:], [1, NC], tag="Zall")
        hs0_s = load(hs0[:], [HID, 1], tag="hs0")
        Wh2T_s = load(Wh2T[:], [OUT, N2], tag="Wh2T")
        a12o_s = load(a12o[:], [OUT, 2], tag="a12o")
        wp2_s = load(wp2ab[:], [OUT, 2], tag="wp2")
        bp2_s = load(bp2[:], [1, 1], tag="bp2")
        Wg3_s = load(Wg3[:], [OUT, 1], tag="Wg3")
        bg3_s = load(bg3[:], [1, 1], tag="bg3")
        fcb_s = load(fcb[:], [1, NCLS], tag="fcb")

        # hs1 columns [128, 16] = sum_c Pall[:, c*16+h] / Z
        hs1c = sb.tile([OUT, NH], F32, tag="hs1c")
        src = _ap(Pall_s[:].tensor, 0, [[NC * NH, OUT], [1, NH], [NH, NC]])
        nc.vector.tensor_reduce(hs1c[:], src, AX.X, ALU.add)
        Zt = sb.tile([1, 1], F32, tag="Zt")
        nc.vector.tensor_reduce(Zt[:], Zall_s[:], AX.X, ALU.add)
        iZ = sb.tile([1, 1], F32, tag="iZ")
        nc.vector.reciprocal(iZ[:], Zt[:])
        izrep_ps = psa.tile([128, 1], F32, tag="r1")
        nc.tensor.matmul(izrep_ps[:], ones1[:], iZ[:], start=True, stop=True)
        izcol = sb.tile([128, 1], F32, tag="izcol")
        nc.vector.tensor_copy(izcol[:], izrep_ps[:])
        nc.vector.tensor_scalar(hs1c[:], hs1c[:], izcol[:OUT, :], None, op0=ALU.mult)

        # att2 scores
        s1o_ps = psa.tile([1, N2], F32, tag="r2")
        nc.tensor.matmul(s1o_ps[:], a12o_s[:, 0:1], Wh2T_s[:], start=True, stop=True)
        s2o_ps = psa.tile([1, N2], F32, tag="r3")
        nc.tensor.matmul(s2o_ps[:], a12o_s[:, 1:2], Wh2T_s[:], start=True, stop=True)
        s1o = sb.tile([1, N2], F32, tag="s1osb")
        nc.vector.tensor_copy(s1o[:], s1o_ps[:])
        s2o = sb.tile([1, N2], F32, tag="s2osb")
        nc.vector.tensor_copy(s2o[:], s2o_ps[:])
        s2orep_ps = psa.tile([128, N2], F32, tag="r4")
        nc.tensor.matmul(s2orep_ps[:], ones1[:], s2o[:], start=True, stop=True)

        att2 = []
        for t2 in range(2):
            s1c_ps = psb.tile([128, 1], F32, tag="mmB")
            nc.tensor.transpose(s1c_ps[:], s1o[:, 128 * t2:128 * (t2 + 1)], ident_s[0:1, 0:1])
            s1c = sb2.tile([128, 1], F32, tag="s1c")
            nc.vector.tensor_copy(s1c[:], s1c_ps[:])
            e3t = sb2.tile([128, N2], F32, tag="e3t")
            nc.sync.dma_start(e3t[:], e3_2[128 * t2:128 * (t2 + 1), :])
            adt = sb2.tile([128, N2], F32, tag="adt")
            nc.sync.dma_start(adt[:], adjm2[128 * t2:128 * (t2 + 1), :])
            e1 = sb2.tile([128, N2], F32, tag="e1b")
            nc.vector.tensor_tensor(e1[:], e3t[:], s2orep_ps[:], ALU.add)
            lr = sb2.tile([128, N2], F32, tag="lrb")
            nc.scalar.activation(lr[:], e1[:], AF.Lrelu, bias=s1c[:], alpha=0.2)
            m1 = sb2.tile([128, N2], F32, tag="m1b")
            nc.vector.scalar_tensor_tensor(m1[:], lr[:], 1e9, adt[:],
                                           op0=ALU.add, op1=ALU.mult)
            nmax = sb2.tile([128, 1], F32, tag="nmaxb")
            nc.vector.tensor_reduce(nmax[:], m1[:], AX.X, ALU.max, negate=True)
            pt = sb2.tile([128, N2], F32, tag="ptb")
            zt = sb2.tile([128, 1], F32, tag="ztb")
            nc.scalar.activation(pt[:], m1[:], AF.Exp, bias=nmax[:], accum_out=zt[:])
            izt = sb2.tile([128, 1], F32, tag="iztb")
            nc.vector.reciprocal(izt[:], zt[:])
            at = sb.tile([128, N2], F32, tag=f"att2_{t2}")
            nc.vector.tensor_scalar(at[:], pt[:], izt[:], None, op0=ALU.mult)
            att2.append(at)

        # att2T + h2T
        attT2 = []
        for lc in range(2):
            big = sb.tile([128, N2], F32, tag=f"attT2_{lc}")
            attT2.append(big)
        for t2 in range(2):
            for lc in range(2):
                tp_ps = psb.tile([128, 128], F32, tag="mmB")
                nc.tensor.transpose(tp_ps[:], att2[t2][:, 128 * lc:128 * (lc + 1)],
                                    ident_s[:])
                nc.vector.tensor_copy(attT2[lc][:, 128 * t2:128 * (t2 + 1)], tp_ps[:])
        h2_ps = psa.tile([OUT, N2], F32, tag="r5")
        for lc in range(2):
            w2n_s = sb2.tile([128, OUT], F32, tag="w2n")
            nc.sync.dma_start(w2n_s[:], Wh2nat[128 * lc:128 * (lc + 1), :])
            nc.tensor.matmul(h2_ps[:], w2n_s[:], attT2[lc][:],
                             start=(lc == 0), stop=(lc == 1))
        h2T = sb.tile([OUT, N2], F32, tag="h2T")
        nc.vector.tensor_copy(h2T[:], h2_ps[:])

        # edge pool 2
        dpa_ps = psa.tile([1, N2], F32, tag="r1")
        nc.tensor.matmul(dpa_ps[:], wp2_s[:, 0:1], h2T[:], start=True, stop=True)
        dpb_ps = psa.tile([1, N2], F32, tag="r2")
        nc.tensor.matmul(dpb_ps[:], wp2_s[:, 1:2], h2T[:], start=True, stop=True)
        dk2 = sb.tile([1, N3], F32, tag="dk2")
        dasb2 = sb.tile([1, N2], F32, tag="dasb2")
        nc.vector.tensor_copy(dasb2[:], dpa_ps[:])
        a_ap = _ap(dasb2[:].tensor, 0, [[N2, 1], [2, N3]])
        b_ap = _ap(dpb_ps[:].tensor, 1, [[N2, 1], [2, N3]])
        nc.vector.tensor_tensor(dk2[:], a_ap, b_ap, ALU.add)
        s2k = sb.tile([1, N3], F32, tag="s2k")
        nc.scalar.activation(s2k[:], dk2[:], AF.Sigmoid, bias=bp2_s[:])
        srep2_ps = psa.tile([128, N3], F32, tag="r3")
        nc.tensor.matmul(srep2_ps[:], ones1[:], s2k[:], start=True, stop=True)
        ev_ap = _ap(h2T[:].tensor, 0, [[N2, OUT], [2, N3]])
        od_ap = _ap(h2T[:].tensor, 1, [[N2, OUT], [2, N3]])
        t12 = sb.tile([OUT, N3], F32, tag="t12")
        nc.vector.tensor_tensor(t12[:], ev_ap, od_ap, ALU.add)
        h3T = sb.tile([OUT, N3], F32, tag="h3T")
        nc.vector.tensor_tensor(h3T[:], t12[:], srep2_ps[:OUT, :], ALU.mult)

        # gpool3 -> hs2 [128, 1]
        g3_ps = psa.tile([1, N3], F32, tag="r1")
        nc.tensor.matmul(g3_ps[:], Wg3_s[:], h3T[:], start=True, stop=True)
        g3s = sb.tile([1, N3], F32, tag="g3s")
        nc.scalar.activation(g3s[:], g3_ps[:], AF.Sigmoid, bias=bg3_s[:])
        nm3 = sb.tile([1, 1], F32, tag="nm3")
        nc.vector.tensor_reduce(nm3[:], g3s[:], AX.X, ALU.max, negate=True)
        w3 = sb.tile([1, N3], F32, tag="w3")
        z3 = sb.tile([1, 1], F32, tag="z3")
        nc.scalar.activation(w3[:], g3s[:], AF.Exp, bias=nm3[:], accum_out=z3[:])
        iz3 = sb.tile([1, 1], F32, tag="iz3")
        nc.vector.reciprocal(iz3[:], z3[:])
        nc.vector.tensor_scalar(w3[:], w3[:], iz3[:], None, op0=ALU.mult)
        w3rep_ps = psa.tile([128, N3], F32, tag="r2")
        nc.tensor.matmul(w3rep_ps[:], ones1[:], w3[:], start=True, stop=True)
        hw3 = sb.tile([OUT, N3], F32, tag="hw3")
        nc.vector.tensor_tensor(hw3[:], h3T[:], w3rep_ps[:OUT, :], ALU.mult)
        hs2 = sb.tile([OUT, 1], F32, tag="hs2")
        nc.vector.tensor_reduce(hs2[:], hw3[:], AX.X, ALU.add)

        # x chunks [128, 18] bf16: cols 0-15 hs1c, col16 [hs0; hs2[0:64]], col17 [hs2[64:]; 1]
        xc = sb.tile([128, 18], F32, tag="xc")
        nc.gpsimd.memset(xc[:], 0.0)
        nc.vector.tensor_copy(xc[:OUT, 0:NH], hs1c[:])
        nc.vector.tensor_copy(xc[:HID, 16:17], hs0_s[:])
        nc.sync.dma_start(xc[HID:128, 16:17], hs2[0:HID, :])
        nc.sync.dma_start(xc[0:HID, 17:18], hs2[HID:OUT, :])
        nc.gpsimd.memset(xc[HID:HID + 1, 17:18], 1.0)
        xcb = sb.tile([128, 18], BF16, tag="xcb")
        nc.vector.tensor_copy(xcb[:], xc[:])

        # LSTM layer 0 (M-orientation, skip f-gate m=1)
        h0 = []
        for d in range(2):
            g_ps = psa.tile([128, 4], F32, tag="gacc")
            for m in (0, 2, 3):
                for k in range(18):
                    rows = 65 if k == 17 else 128
                    w_s = sb2.tile([128, 128], BF16, tag="w0s")
                    nc.sync.dma_start(w_s[:rows, :], W0b[d, k, 0:rows, 128 * m:128 * (m + 1)])
                    nc.tensor.matmul(g_ps[:, m:m + 1], w_s[:rows, :], xcb[:rows, k:k + 1],
                                     start=(k == 0), stop=(k == 17))
            si = sb2.tile([128, 1], F32, tag="si")
            nc.scalar.activation(si[:], g_ps[:, 0:1], AF.Sigmoid)
            tg = sb2.tile([128, 1], F32, tag="tg")
            nc.scalar.activation(tg[:], g_ps[:, 2:3], AF.Tanh)
            so = sb2.tile([128, 1], F32, tag="so")
            nc.scalar.activation(so[:], g_ps[:, 3:4], AF.Sigmoid)
            c = sb2.tile([128, 1], F32, tag="c0")
            nc.vector.tensor_tensor(c[:], si[:], tg[:], ALU.mult)
            tc_ = sb2.tile([128, 1], F32, tag="tc0")
            nc.scalar.activation(tc_[:], c[:], AF.Tanh)
            hd = sb.tile([128, 1], F32, tag=f"h0_{d}")
            nc.vector.tensor_tensor(hd[:], so[:], tc_[:], ALU.mult)
            h0.append(hd)
        h0b_ = []
        for d in range(2):
            hb = sb.tile([128, 1], BF16, tag=f"h0b_{d}")
            nc.vector.tensor_copy(hb[:], h0[d][:])
            h0b_.append(hb)
        onesb = sb.tile([1, 1], BF16, tag="onesb")
        nc.gpsimd.memset(onesb[:], 1.0)

        # LSTM layer 1
        h1o = []
        for d in range(2):
            g_ps = psa.tile([128, 4], F32, tag="gacc")
            for m in (0, 2, 3):
                for k in range(3):
                    rows = 1 if k == 2 else 128
                    w_s = sb2.tile([128, 128], BF16, tag="w1s")
                    nc.sync.dma_start(w_s[:rows, :], W1b[d, k, 0:rows, 128 * m:128 * (m + 1)])
                    rhs = onesb[:] if k == 2 else h0b_[k][:]
                    nc.tensor.matmul(g_ps[:, m:m + 1], w_s[:rows, :], rhs,
                                     start=(k == 0), stop=(k == 2))
            si = sb2.tile([128, 1], F32, tag="si1")
            nc.scalar.activation(si[:], g_ps[:, 0:1], AF.Sigmoid)
            tg = sb2.tile([128, 1], F32, tag="tg1")
            nc.scalar.activation(tg[:], g_ps[:, 2:3], AF.Tanh)
            so = sb2.tile([128, 1], F32, tag="so1")
            nc.scalar.activation(so[:], g_ps[:, 3:4], AF.Sigmoid)
            c = sb2.tile([128, 1], F32, tag="c1")
            nc.vector.tensor_tensor(c[:], si[:], tg[:], ALU.mult)
            tc_ = sb2.tile([128, 1], F32, tag="tc1")
            nc.scalar.activation(tc_[:], c[:], AF.Tanh)
            hd = sb.tile([128, 1], F32, tag=f"h1_{d}")
            nc.vector.tensor_tensor(hd[:], so[:], tc_[:], ALU.mult)
            h1o.append(hd)

        # fc + softmax
        lg_ps = psa.tile([1, NCLS], F32, tag="r1")
        fcw0 = sb.tile([LH, NCLS], F32, tag="fcw0")
        nc.sync.dma_start(fcw0[:], fcWr[0])
        fcw1 = sb.tile([LH, NCLS], F32, tag="fcw1")
        nc.sync.dma_start(fcw1[:], fcWr[1])
        nc.tensor.matmul(lg_ps[:], h1o[0][:], fcw0[:], start=True, stop=False)
        nc.tensor.matmul(lg_ps[:], h1o[1][:], fcw1[:], start=False, stop=True)
        lg = sb.tile([1, NCLS], F32, tag="lg")
        nc.vector.tensor_tensor(lg[:], lg_ps[:], fcb_s[:], ALU.add)
        nmf = sb.tile([1, 1], F32, tag="nmf")
        nc.vector.tensor_reduce(nmf[:], lg[:], AX.X, ALU.max, negate=True)
        pf = sb.tile([1, NCLS], F32, tag="pf")
        zf = sb.tile([1, 1], F32, tag="zf")
        nc.scalar.activation(pf[:], lg[:], AF.Exp, bias=nmf[:], accum_out=zf[:])
        izf = sb.tile([1, 1], F32, tag="izf")
        nc.vector.reciprocal(izf[:], zf[:])
        prob = sb.tile([1, NCLS], F32, tag="prob")
        nc.vector.tensor_scalar(prob[:], pf[:], izf[:], None, op0=ALU.mult)
        nc.sync.dma_start(o_prob[:], prob[:])

    nc.compile()
    return nc


# ---------------------------------------------------------------- host prep
def _prep_A(inputs):
    """Build per-core input maps for dispatch A. Pure layout/indexing."""
    f32 = np.float32
    import ml_dtypes
    bf = ml_dtypes.bfloat16
    ei = np.asarray(inputs["edge_index"])
    feats = np.asarray(inputs["features"], f32)
    n2n = np.asarray(inputs["node2node_features"], f32)
    eattr = np.asarray(inputs["edgesAttr"], f32)
    adjacency = np.asarray(inputs["adjacency"], f32)

    src, dst = np.asarray(ei[0], np.int64), np.asarray(ei[1], np.int64)
    pairs = src * N + dst
    uniq = np.unique(pairs)
    us, ud = uniq // N, uniq % N
    order = np.argsort(us, kind="stable")
    us, ud, uniq = us[order], ud[order], uniq[order]
    counts = np.bincount(us, minlength=N)
    assert counts.max() <= S, f"out-degree {counts.max()} > {S}"
    starts = np.zeros(N + 1, np.int64)
    np.cumsum(counts, out=starts[1:])
    slots = np.arange(len(us)) - starts[us]

    featT = np.ascontiguousarray(feats.T)
    eaT = np.ascontiguousarray(eattr.T)
    W_gat = np.asarray(inputs["W_gat"], f32)

    sh32 = {
        "featT": featT,
        "W_sn": np.asarray(inputs["W_sn"], f32),
        "a_sn": np.asarray(inputs["a_sn"], f32).reshape(HID, 1),
        "Wg1": np.asarray(inputs["Wg1"], f32).reshape(HID, 1),
        "bg1": np.asarray(inputs["bg1"], f32).reshape(1, 1),
        "a12": np.stack([np.asarray(inputs["a1_gat"], f32),
                         np.asarray(inputs["a2_gat"], f32)], -1),
        "a3t128": np.tile(np.asarray(inputs["a3_gat"], f32).T, (1, 8)),
        "a3oT": np.asarray(inputs["a3_o"], f32).reshape(NH, OUT).T,
        "wp1ab": np.stack([
            np.asarray(inputs["Wp1"], f32)[:D1, 0].reshape(NH, OUT),
            np.asarray(inputs["Wp1"], f32)[D1:, 0].reshape(NH, OUT)], -1),
        "Wg2r": np.asarray(inputs["Wg2"], f32).reshape(NH, OUT, 1),
        "bp1": np.asarray(inputs["bp1"], f32).reshape(1, 1),
        "bg2": np.asarray(inputs["bg2"], f32).reshape(1, 1),
        "selh2": np.eye(NH, dtype=f32)[:, np.tile(np.arange(NH), 8)].reshape(NH, 128),
    }
    selrep = np.zeros((NPC, NC * 128), f32)
    for t in range(8):
        for p in range(128):
            selrep[8 * t + p // 16, 128 * t + p] = 1.0
    sh16 = {
        "Wgat": W_gat,
        "Wegat": np.asarray(inputs["We_gat"], f32),
        "Wor": np.asarray(inputs["Wo"], f32).reshape(NH, OUT, OUT),
        "selrep": selrep,
        "ident": np.eye(128, dtype=f32),
    }

    in_maps = []
    for c in range(NC):
        lo = c * NPC
        d32 = dict(sh32)
        d16 = dict(sh16)
        d32["featTm"] = featT[:, lo:lo + NPC]
        mask = (us >= lo) & (us < lo + NPC)
        cs, cd, csl = us[mask] - lo, ud[mask], slots[mask]
        XP = np.zeros((NPC * S, HID), f32)
        XP[cs * S + csl] = n2n[uniq[mask]]
        d16["XP"] = XP.T
        ptr = np.full((NPC, N), NPC * S, np.int64)
        ptr[cs, cd] = cs * S + csl
        g = np.zeros((128, 256), np.int16)
        for t in range(8):
            for gg in range(8):
                row = ptr[8 * t + gg]
                g[16 * gg:16 * gg + 16, 32 * t:32 * t + 32] = \
                    row.reshape(32, 16).T.astype(np.int16)
        d32["gidxbits"] = g.view(f32)
        d16["adjmine"] = adjacency[lo:lo + NPC]
        d16["eaT"] = eaT[:, c * EPC:(c + 1) * EPC]
        blob32 = np.empty(LEN32, f32)
        for name, shape in SPEC32:
            off, _ = OFF32[name]
            blob32[off:off + int(np.prod(shape))] = np.ascontiguousarray(d32[name], f32).reshape(-1)
        blob16 = np.empty(LEN16, bf)
        for name, shape in SPEC16:
            off, _ = OFF16[name]
            blob16[off:off + int(np.prod(shape))] = np.ascontiguousarray(d16[name], f32).reshape(-1).astype(bf)
        in_maps.append({"blob32": blob32, "blob16": blob16})
    return in_maps, (src, dst)


def _prep_B(inputs, resA, ei_sd):
    f32 = np.float32
    src, dst = ei_sd
    unp = []
    for c in range(NC):
        o = resA[c]["o_all"]
        unp.append({"o_P": o[:, 0:16], "o_Wh2T": o[:, 16:48],
                    "o_hs0": o[0:HID, 48:49], "o_Z": o[0:1, 49:50],
                    "o_es": o[:, 50:58].reshape(-1)})
    resA = unp
    es = np.concatenate([resA[c]["o_es"].reshape(-1) for c in range(NC)])
    s2, d2 = src // 2, dst // 2
    adj2 = np.zeros((N2, N2), f32)
    adj2[s2, d2] = 1.0
    e3_2 = np.zeros((N2, N2), f32)
    e3_2[s2, d2] = es  # numpy fancy assignment: last occurrence wins
    Wh2T = np.concatenate([resA[c]["o_Wh2T"] for c in range(NC)], axis=1)
    Pall = np.concatenate([resA[c]["o_P"] for c in range(NC)], axis=1)
    Zall = np.concatenate([resA[c]["o_Z"].reshape(1, 1) for c in range(NC)], axis=1)

    # LSTM weights: my-x order = [hs1(2048), hs0(64), hs2(128), bias(1)]
    perm = np.concatenate([np.arange(64, 2112), np.arange(0, 64), np.arange(2112, 2240)])
    W0 = np.zeros((2, 18, 128, 4 * LH), f32)
    for d in range(2):
        wt = np.asarray(inputs["Wih0"], f32)[d].T[perm]         # [2240, 512]
        wb = np.concatenate([wt, np.asarray(inputs["b0"], f32)[d][None, :]], 0)  # [2241,512]
        for k in range(18):
            rows = wb[128 * k:128 * (k + 1)]
            W0[d, k, :rows.shape[0], :] = rows
    W1 = np.zeros((2, 3, 128, 4 * LH), f32)
    for d in range(2):
        wt = np.asarray(inputs["Wih1"], f32)[d].T               # [256, 512]
        wb = np.concatenate([wt, np.asarray(inputs["b1"], f32)[d][None, :]], 0)
        for k in range(3):
            rows = wb[128 * k:128 * (k + 1)]
            W1[d, k, :rows.shape[0], :] = rows
    import ml_dtypes
    bf = ml_dtypes.bfloat16

    return {
        "adjm2": adj2,
        "e3_2": e3_2,
        "Wh2T": np.ascontiguousarray(Wh2T),
        "Wh2nat": np.ascontiguousarray(Wh2T.T),
        "a12o": np.ascontiguousarray(np.stack(
            [np.asarray(inputs["a1_o"], f32), np.asarray(inputs["a2_o"], f32)], -1)),
        "wp2ab": np.ascontiguousarray(np.stack(
            [np.asarray(inputs["Wp2"], f32)[:OUT, 0], np.asarray(inputs["Wp2"], f32)[OUT:, 0]], -1)),
        "bp2": np.asarray(inputs["bp2"], f32).reshape(1, 1),
        "Wg3": np.asarray(inputs["Wg3"], f32).reshape(OUT, 1),
        "bg3": np.asarray(inputs["bg3"], f32).reshape(1, 1),
        "fcWr": np.asarray(inputs["fc_W"], f32).reshape(2, LH, NCLS, order="C")
                  if False else np.stack([np.asarray(inputs["fc_W"], f32)[:LH],
                                          np.asarray(inputs["fc_W"], f32)[LH:]]),
        "fcb": np.asarray(inputs["fc_b"], f32).reshape(1, NCLS),
        "Pall": np.ascontiguousarray(Pall),
        "Zall": np.ascontiguousarray(Zall),
        "hs0": resA[0]["o_hs0"].reshape(HID, 1),
        "W0b": W0.astype(bf),
        "W1b": W1.astype(bf),
        "ident": np.eye(128, dtype=f32),
    }


# ------------------------------------------------------- cached SPMD runner
class _CachedRunner:
    """Like bass2jax.run_bass_via_pjrt but with the jitted callable built once."""

    def __init__(self, nc, n_cores):
        import jax
        from jax.sharding import Mesh, PartitionSpec
        from jax.experimental.shard_map import shard_map
        from concourse import bass2jax
        bass2jax.install_neuronx_cc_hook()
        self.n_cores = n_cores
        partition_name = nc.partition_id_tensor.name if nc.partition_id_tensor else None
        in_names, out_names, out_avals, zero_outs = [], [], [], []
        for alloc in nc.m.functions[0].allocations:
            if not isinstance(alloc, mybir.MemoryLocationSet):
                continue
            name = alloc.memorylocations[0].name
            if alloc.kind == "ExternalInput":
                if name != partition_name:
                    in_names.append(name)
            elif alloc.kind == "ExternalOutput":
                shape = tuple(alloc.tensor_shape)
                dtype = mybir.dt.np(alloc.dtype)
                out_names.append(name)
                out_avals.append(jax.core.ShapedArray(shape, dtype))
                zero_outs.append(np.zeros(shape, dtype))
        self.in_names, self.out_names = in_names, out_names
        self.out_avals, self.zero_outs = out_avals, zero_outs
        n_params, n_outs = len(in_names), len(out_names)
        all_names = in_names + out_names
        if partition_name is not None:
            all_names = all_names + [partition_name]
        donate = tuple(range(n_params, n_params + n_outs))

        def _body(*args):
            operands = list(args)
            if partition_name is not None:
                operands.append(bass2jax.partition_id_tensor())
            outs = bass2jax._bass_exec_p.bind(
                *operands,
                out_avals=tuple(out_avals),
                in_names=tuple(all_names),
                out_names=tuple(out_names),
                lowering_input_output_aliases=(),
                sim_require_finite=True,
                sim_require_nnan=True,
                nc=nc,
            )
            return tuple(outs)

        self._body = _body
        self._jax = jax
        self._Mesh, self._P, self._shard_map = Mesh, PartitionSpec, shard_map
        self.donate = donate
        self.n_params, self.n_outs = n_params, n_outs
        self.fn = None
        if n_cores == 1:
            self.fn = jax.jit(_body, donate_argnums=donate, keep_unused=True)

    def _build_multi(self, shared_flags):
        jax = self._jax
        devices = jax.devices()[:self.n_cores]
        mesh = self._Mesh(np.asarray(devices), ("core",))
        self.shared_flags = shared_flags
        in_specs = tuple(self._P() if f else self._P("core") for f in shared_flags) \
            + (self._P("core"),) * self.n_outs
        out_specs = (self._P("core"),) * self.n_outs
        self.fn = jax.jit(
            self._shard_map(self._body, mesh=mesh, in_specs=in_specs,
                            out_specs=out_specs, check_rep=False),
            donate_argnums=self.donate, keep_unused=True)

    def __call__(self, in_maps):
        nc_ = self.n_cores
        if nc_ == 1:
            out = self.fn(*[np.asarray(in_maps[0][n]) for n in self.in_names],
                          *self.zero_outs)
            return [{n: np.asarray(out[i]) for i, n in enumerate(self.out_names)}]
        if self.fn is None:
            flags = [all(in_maps[c][n] is in_maps[0][n] for c in range(nc_))
                     for n in self.in_names]
            self._build_multi(flags)
        args = []
        for i, n in enumerate(self.in_names):
            if self.shared_flags[i]:
                args.append(np.asarray(in_maps[0][n]))
            else:
                args.append(np.concatenate(
                    [np.asarray(in_maps[c][n]) for c in range(nc_)], axis=0))
        concat_zeros = [np.zeros((nc_ * z.shape[0], *z.shape[1:]), z.dtype)
                        for z in self.zero_outs]
        out = self.fn(*args, *concat_zeros)
        res = []
        for c in range(nc_):
            res.append({n: np.asarray(out[i]).reshape(nc_, *self.out_avals[i].shape)[c]
                        for i, n in enumerate(self.out_names)})
        return res


# ---------------------------------------------------------------- entrypoint
def kernel(**inputs):
    if "A" not in _cache:
        _cache["A"] = _CachedRunner(build_A(), NC)
    if "B" not in _cache:
        _cache["B"] = _CachedRunner(build_B(), 1)
    in_maps, ei_sd = _prep_A(inputs)
    resA = _cache["A"](in_maps)
    inB = _prep_B(inputs, resA, ei_sd)
    resB = _cache["B"]([inB])
    return resB[0]["o_prob"].reshape(NCLS).astype(np.float32)


# revision 13
# speedup vs baseline: 3.9781x; 1.0175x over previous
"""Trainium2 Bass kernel for nn_DefectDetection (GAT + pooling + LSTM head).

Self-contained: accepts FULL inputs, shards across 8 NeuronCores internally.

Strategy:
  Dispatch A (8 cores, SPMD):
    - replicated small front-end (node-attention layer, gpool1, GAT projections)
    - node-row-sharded dense [N,N] attention maps (64 rows x 16 heads / core),
      with the sparse node2node e3 term built from a host-packed slot grid via
      one matmul + gpsimd ap_gather (no 64MiB dense read)
    - edge-sharded edge-attr score reduction (es)
    - per-core outputs: es slice, gpool2 partials (P,Z), Wh2 rows, hs0
  Host in between: pure data movement (concat / scatter by precomputed indices).
  Dispatch B (1 core): pooled-graph attention (256 nodes), edge pool 2, gpool3,
    2-layer bi-LSTM (T=1) with bf16 weights, fc + softmax -> [2].
"""
import numpy as np
from contextlib import ExitStack

import concourse.bass as bass
import concourse.bacc as bacc
import concourse.tile as tile
import concourse.mybir as mybir
from concourse.bass_utils import run_bass_kernel_spmd

F32 = mybir.dt.float32
BF16 = mybir.dt.bfloat16
I16 = mybir.dt.int16
AF = mybir.ActivationFunctionType
ALU = mybir.AluOpType
AX = mybir.AxisListType

N, E, HID, NH, OUT, NCLS, LH = 512, 8192, 64, 16, 128, 2, 128
NC = 8          # cores
NPC = N // NC   # 64 nodes per core
S = 64          # slot grid per node
EPC = E // NC   # 1024 edges per core (F stage)
D1 = NH * OUT   # 2048
N2 = N // 2     # 256
N3 = N // 4     # 128
JUMP = HID + D1 + OUT  # 2240

_cache = {}

# blob layouts: (name, shape) -> row-major at running offset
SPEC32 = [
    ("featT", (HID, N)), ("featTm", (HID, NPC)), ("W_sn", (HID, HID)),
    ("a_sn", (HID, 1)), ("Wg1", (HID, 1)), ("bg1", (1, 1)),
    ("a12", (NH, OUT, 2)), ("a3t128", (HID, 128)), ("a3oT", (OUT, NH)),
    ("wp1ab", (NH, OUT, 2)), ("Wg2r", (NH, OUT, 1)), ("bp1", (1, 1)),
    ("bg2", (1, 1)), ("selh2", (NH, 128)), ("gidxbits", (128, 128)),
]
SPEC16 = [
    ("Wgat", (NH, HID, OUT)), ("Wegat", (NH, HID, OUT)), ("Wor", (NH, OUT, OUT)),
    ("XP", (HID, NPC * S)), ("eaT", (HID, EPC)), ("adjmine", (NPC, N)),
    ("selrep", (NPC, NC * 128)), ("ident", (128, 128)),
]


def _offsets(spec):
    out, off = {}, 0
    for name, shape in spec:
        n = int(np.prod(shape))
        out[name] = (off, shape)
        off += n
    return out, off

OFF32, LEN32 = _offsets(SPEC32)
OFF16, LEN16 = _offsets(SPEC16)

SPECB32 = [
    ("adjm2", (N2, N2)), ("e3_2", (N2, N2)), ("Wh2T", (OUT, N2)),
    ("Wh2nat", (N2, OUT)), ("a12o", (OUT, 2)), ("wp2ab", (OUT, 2)),
    ("bp2", (1, 1)), ("Wg3", (OUT, 1)), ("bg3", (1, 1)),
    ("fcWr", (2, LH, NCLS)), ("fcb", (1, NCLS)), ("Pall", (OUT, NC * NH)),
    ("Zall", (1, NC)), ("hs0", (HID, 1)), ("identB", (128, 128)),
]
SPECB16 = [("W0b", (2, 18, 128, 4 * LH)), ("W1b", (2, 3, 128, 4 * LH))]
OFFB32, LENB32 = _offsets(SPECB32)
OFFB16, LENB16 = _offsets(SPECB16)



def _ap(t, offset, dims):
    return bass.AP(tensor=t, offset=offset, ap=[list(d) for d in dims])


# ---------------------------------------------------------------- dispatch A
def build_A():
    nc = bacc.Bacc("TRN2", target_bir_lowering=False, debug=False, num_devices=NC)

    def inp(name, shape, dt=F32):
        return nc.dram_tensor(name, shape, dt, kind="ExternalInput").ap()

    def outp(name, shape, dt=F32):
        return nc.dram_tensor(name, shape, dt, kind="ExternalOutput").ap()

    blob32 = inp("blob32", [LEN32])
    blob16 = inp("blob16", [LEN16], BF16)

    def b32(name, head=None):
        off, shape = OFF32[name]
        if head is not None:
            per = int(np.prod(shape[1:]))
            off, shape = off + head * per, shape[1:]
        rows, cols = (shape[0], int(np.prod(shape[1:]))) if len(shape) > 1 else (1, shape[0])
        return _ap(blob32.tensor, off, [[cols, rows], [1, cols]])

    def b16(name, head=None):
        off, shape = OFF16[name]
        if head is not None:
            per = int(np.prod(shape[1:]))
            off, shape = off + head * per, shape[1:]
        rows, cols = (shape[0], int(np.prod(shape[1:]))) if len(shape) > 1 else (1, shape[0])
        return _ap(blob16.tensor, off, [[cols, rows], [1, cols]])

    o_all = outp("o_all", [128, 58])

    with tile.TileContext(nc) as tc, ExitStack() as ctx:
        sb = ctx.enter_context(tc.tile_pool(name="sb", bufs=1))
        sb2 = ctx.enter_context(tc.tile_pool(name="sb2", bufs=2))
        sb3 = ctx.enter_context(tc.tile_pool(name="sb3", bufs=3))
        psa = ctx.enter_context(tc.tile_pool(name="psa", bufs=1, space="PSUM"))
        psb = ctx.enter_context(tc.tile_pool(name="psb", bufs=2, space="PSUM"))
        dram = ctx.enter_context(tc.tile_pool(name="dram", bufs=1, space="DRAM"))

        def load(apx, shape, dt=F32, pool=sb, tag=None):
            t = pool.tile(shape, dt, tag=tag)
            nc.sync.dma_start(t[:], apx)
            return t

        def load16(name, shape, tag):
            t = sb.tile(shape, F32, tag=tag)
            nc.gpsimd.dma_start(t[:], b16(name))
            return t

        featT_s = load(b32("featT"), [HID, N], tag="featT")
        featTm_s = load(b32("featTm"), [HID, NPC], tag="featTm")
        Wsn_s = load(b32("W_sn"), [HID, HID], tag="Wsn")
        asn_s = load(b32("a_sn"), [HID, 1], tag="asn")
        Wg1_s = load(b32("Wg1"), [HID, 1], tag="Wg1")
        bg1_s = load(b32("bg1"), [1, 1], tag="bg1")
        ident_s = load16("ident", [128, 128], tag="ident")
        a3t_s = load(b32("a3t128"), [HID, 128], tag="a3t")
        XP_s = load16("XP", [HID, NPC * S], tag="XP")
        gidxf_s = load(b32("gidxbits"), [128, 128], tag="gidx")
        gidx_s = None
        adjm_s = load16("adjmine", [NPC, N], tag="adjm")
        selh2_s = load(b32("selh2"), [NH, 128], tag="selh2")
        eaT_s = load16("eaT", [HID, EPC], tag="eaT")
        selrep_s = load16("selrep", [NPC, NC * 128], tag="selrep")
        a3oT_s = load(b32("a3oT"), [OUT, NH], tag="a3oT")
        bp1_s = load(b32("bp1"), [1, 1], tag="bp1")
        bg2_s = load(b32("bg2"), [1, 1], tag="bg2")

        ones1_128 = sb.tile([1, 128], F32, tag="ones1")
        nc.gpsimd.memset(ones1_128[:], 1.0)
        ones128 = sb.tile([128, 1], F32, tag="ones128")
        nc.gpsimd.memset(ones128[:], 1.0)

        def elu_inplace(src_ps, dst_sb, shape, pool=sb2, tagp="elu"):
            """dst = elu(src) where src is PSUM [p,f]; dst SBUF."""
            p, f = shape
            ex = pool.tile([p, f], F32, tag=tagp + "_ex")
            nc.scalar.activation(ex[:], src_ps, AF.Exp)
            rl = pool.tile([p, f], F32, tag=tagp + "_rl")
            nc.scalar.activation(rl[:], src_ps, AF.Relu)
            # dst = (min(ex,1) + rl) - 1
            nc.vector.scalar_tensor_tensor(dst_sb, ex[:], 1.0, rl[:],
                                           op0=ALU.min, op1=ALU.add)
            nc.vector.tensor_scalar(dst_sb, dst_sb, 1.0, None, op0=ALU.subtract)

        # ---------------- front: h = elu(sigmoid(lrelu(Wh0@a))*Wh0)
        def front(ft, width, tag):
            wh0_ps = psb.tile([HID, width], F32, tag="mm")
            nc.tensor.matmul(wh0_ps[:], Wsn_s[:], ft, start=True, stop=True)
            wh0 = sb.tile([HID, width], F32, tag="wh0_" + tag)
            nc.scalar.copy(wh0[:], wh0_ps[:])
            ga_ps = psb.tile([1, width], F32, tag="mm")
            nc.tensor.matmul(ga_ps[:], asn_s[:], wh0[:], start=True, stop=True)
            gl = sb.tile([1, width], F32, tag="gl_" + tag)
            nc.scalar.activation(gl[:], ga_ps[:], AF.Lrelu, alpha=0.2)
            gs = sb.tile([1, width], F32, tag="gs_" + tag)
            nc.scalar.activation(gs[:], gl[:], AF.Sigmoid)
            grep_ps = psb.tile([HID, width], F32, tag="mm")
            nc.tensor.matmul(grep_ps[:], ones1_128[:, :HID], gs[:], start=True, stop=True)
            hpre = sb.tile([HID, width], F32, tag="hpre_" + tag)
            nc.vector.tensor_tensor(hpre[:], wh0[:], grep_ps[:], ALU.mult)
            ht = sb.tile([HID, width], F32, tag="ht_" + tag)
            elu_inplace(hpre[:], ht[:], [HID, width], tagp="eluf_" + tag)
            return ht

        hT = front(featT_s[:], N, "full")          # [64, 512]
        hTm = front(featTm_s[:], NPC, "mine")      # [64, 64]

        # ---------------- gpool1 -> hs0
        g1_ps = psb.tile([1, N], F32, tag="mm")
        nc.tensor.matmul(g1_ps[:], Wg1_s[:], hT[:], start=True, stop=True)
        g1s = sb.tile([1, N], F32, tag="g1s")
        nc.scalar.activation(g1s[:], g1_ps[:], AF.Sigmoid, bias=bg1_s[:])
        nmax1 = sb.tile([1, 1], F32, tag="nmax1")
        nc.vector.tensor_reduce(nmax1[:], g1s[:], AX.X, ALU.max, negate=True)
        w1 = sb.tile([1, N], F32, tag="w1")
        z1 = sb.tile([1, 1], F32, tag="z1")
        nc.scalar.activation(w1[:], g1s[:], AF.Exp, bias=nmax1[:], accum_out=z1[:])
        iz1 = sb.tile([1, 1], F32, tag="iz1")
        nc.vector.reciprocal(iz1[:], z1[:])
        nc.vector.tensor_scalar(w1[:], w1[:], iz1[:], None, op0=ALU.mult)
        w1rep_ps = psb.tile([HID, N], F32, tag="mm")
        nc.tensor.matmul(w1rep_ps[:], ones1_128[:, :HID], w1[:], start=True, stop=True)
        hw = sb.tile([HID, N], F32, tag="hw")
        nc.vector.tensor_tensor(hw[:], hT[:], w1rep_ps[:], ALU.mult)
        hs0 = sb.tile([HID, 1], F32, tag="hs0")
        nc.vector.tensor_reduce(hs0[:], hw[:], AX.X, ALU.add)
        nc.sync.dma_start(o_all[0:HID, 48:49], hs0[:])

        # ---------------- v12 = WgatT[h] @ a12[h]  -> vall [64, 32]
        vall = sb.tile([HID, 2 * NH], F32, tag="vall")
        for h in range(NH):
            wg0_s = sb2.tile([HID, OUT], F32, tag="wgT0")
            nc.gpsimd.dma_start(wg0_s[:], b16('Wgat', h))
            wgT_ps = psb.tile([OUT, HID], F32, tag="mm")
            nc.tensor.transpose(wgT_ps[:], wg0_s[:], ident_s[0:HID, 0:HID])
            wgT_s = sb2.tile([OUT, HID], F32, tag="wgT")
            nc.vector.tensor_copy(wgT_s[:], wgT_ps[:])
            a12_s = sb2.tile([OUT, 2], F32, tag="a12s")
            nc.sync.dma_start(a12_s[:], b32('a12', h))
            v_ps = psb.tile([HID, 2], F32, tag="mm")
            nc.tensor.matmul(v_ps[:], wgT_s[:], a12_s[:], start=True, stop=True)
            nc.vector.tensor_copy(vall[:, 2 * h:2 * h + 2], v_ps[:])

        # s1mine [16, 64] / s2all [16, 512]
        v1_ap = _ap(vall[:].tensor, 0, [[2 * NH, HID], [2, NH]])
        v2_ap = _ap(vall[:].tensor, 1, [[2 * NH, HID], [2, NH]])
        s1m_ps = psb.tile([NH, NPC], F32, tag="mm")
        nc.tensor.matmul(s1m_ps[:], v1_ap, hTm[:], start=True, stop=True)
        s1m = sb.tile([NH, NPC], F32, tag="s1m")
        nc.vector.tensor_copy(s1m[:], s1m_ps[:])
        s2a_ps = psb.tile([NH, N], F32, tag="mm")
        nc.tensor.matmul(s2a_ps[:], v2_ap, hT[:], start=True, stop=True)
        s2a = sb.tile([NH, N], F32, tag="s2a")
        nc.vector.tensor_copy(s2a[:], s2a_ps[:])
        # s2rep [128, 512]: row p -> s2a[p%16]
        s2rep_ps = psa.tile([128, N], F32, tag="s2rep")
        nc.tensor.matmul(s2rep_ps[:], selh2_s[:], s2a[:], start=True, stop=True)
        s2rep = sb.tile([128, N], F32, tag="s2repsb")
        nc.vector.tensor_copy(s2rep[:], s2rep_ps[:])

        # s1col [128, 8] via DRAM bounce: scratch [16, 64]
        scr = dram.tile([NH, NPC], F32, tag="scr")
        nc.sync.dma_start(scr[:], s1m[:])
        s1col = sb.tile([128, NC], F32, tag="s1col")
        with nc.allow_non_contiguous_dma(reason="s1col 4B gather"):
            for i in range(8):
                src_ap = _ap(scr[:].tensor, i, [[NPC, NH], [8, 8]])
                nc.sync.dma_start(s1col[16 * i:16 * (i + 1), :], src_ap)

        # ---------------- sc = a3-scores on slot grid, replicated rows
        sc_sb = sb.tile([128, NPC * S + 1], F32, tag="scsb")
        for q in range(8):
            scq_ps = psb.tile([128, 512], F32, tag="mm")
            nc.tensor.matmul(scq_ps[:], a3t_s[:], XP_s[:, 512 * q:512 * (q + 1)],
                             start=True, stop=True)
            nc.vector.tensor_copy(sc_sb[:, 512 * q:512 * (q + 1)], scq_ps[:])
        nc.gpsimd.memset(sc_sb[:, NPC * S:NPC * S + 1], 0.0)

        # ---------------- F stage: es over my 1024 edges
        esA_ps = psa.tile([1, 512], F32, tag="accA")
        esB_ps = psa.tile([1, 512], F32, tag="accB")
        sumo_ps = psa.tile([1, 1], F32, tag="accC")
        es_ps = [esA_ps, esB_ps]
        for h in range(NH):
            weg_s = sb2.tile([HID, OUT], F32, tag="weg")
            nc.gpsimd.dma_start(weg_s[:], b16('Wegat', h))
            st, sp = (h == 0), (h == NH - 1)
            for half in range(2):
                T_ps = psb.tile([128, 512], F32, tag="mm")
                nc.tensor.matmul(T_ps[:], weg_s[:], eaT_s[:, 512 * half:512 * (half + 1)],
                                 start=True, stop=True)
                ex = sb2.tile([128, 512], F32, tag="Fex")
                nc.scalar.activation(ex[:], T_ps[:], AF.Exp)
                rl = sb2.tile([128, 512], F32, tag="Frl")
                nc.scalar.activation(rl[:], T_ps[:], AF.Relu)
                eluP = sb2.tile([128, 512], F32, tag="eluP")
                nc.vector.scalar_tensor_tensor(eluP[:], ex[:], 1.0, rl[:],
                                               op0=ALU.min, op1=ALU.add)
                nc.tensor.matmul(es_ps[half][:], a3oT_s[:, h:h + 1], eluP[:],
                                 start=st, stop=sp)
            nc.tensor.matmul(sumo_ps[:], a3oT_s[:, h:h + 1], ones128[:], start=st, stop=sp)
        sumo = sb.tile([1, 1], F32, tag="sumosb")
        nc.vector.tensor_copy(sumo[:], sumo_ps[:])
        es_sb = sb.tile([1, EPC], F32, tag="essb")
        nc.vector.tensor_scalar(es_sb[:, :512], esA_ps[:], sumo[:], None, op0=ALU.subtract)
        nc.vector.tensor_scalar(es_sb[:, 512:], esB_ps[:], sumo[:], None, op0=ALU.subtract)
        dst_es = _ap(o_all.tensor, 50, [[58, 128], [1, 8]])
        nc.sync.dma_start(dst_es, es_sb[:])

        # ---------------- e-stage: 8 tiles [128 (i*16+h), 512]
        att_tiles = []
        for t in range(8):
            e3g = sb2.tile([128, N], F32, tag="e3g")
            nc.gpsimd.ap_gather(e3g[:], sc_sb[:], gidxf_s[:].bitcast(I16)[:, 32 * t:32 * (t + 1)],
                                channels=128, num_elems=NPC * S + 1, d=1, num_idxs=N)
            e1 = sb2.tile([128, N], F32, tag="e1")
            nc.vector.tensor_tensor(e1[:], e3g[:], s2rep[:], ALU.add)
            lr = sb2.tile([128, N], F32, tag="lr")
            nc.scalar.activation(lr[:], e1[:], AF.Lrelu, bias=s1col[:, t:t + 1], alpha=0.2)
            adjrep_ps = psb.tile([128, N], F32, tag="mm")
            nc.tensor.matmul(adjrep_ps[:], selrep_s[:, 128 * t:128 * (t + 1)], adjm_s[:], start=True, stop=True)
            m1 = sb2.tile([128, N], F32, tag="m1")
            nc.vector.scalar_tensor_tensor(m1[:], lr[:], 1e9, adjrep_ps[:],
                                           op0=ALU.add, op1=ALU.mult)
            nmax = sb2.tile([128, 1], F32, tag="nmax")
            nc.vector.tensor_reduce(nmax[:], m1[:], AX.X, ALU.max, negate=True)
            pt = sb2.tile([128, N], F32, tag="pt")
            zt = sb2.tile([128, 1], F32, tag="zt")
            nc.scalar.activation(pt[:], m1[:], AF.Exp, bias=nmax[:], accum_out=zt[:])
            izt = sb2.tile([128, 1], F32, tag="izt")
            nc.vector.reciprocal(izt[:], zt[:])
            att = sb.tile([128, N], F32, tag=f"att{t}")
            nc.vector.tensor_scalar(att[:], pt[:], izt[:], None, op0=ALU.mult)
            att_tiles.append(att)

        # transposes -> attT[jc] [128, 1024] cols = t*128 + (i*16+h)
        attT = []
        for jc in range(4):
            bigt = sb.tile([128, 1024], F32, tag=f"attT{jc}")
            attT.append(bigt)
        for t in range(8):
            for jc in range(4):
                tp_ps = psb.tile([128, 128], F32, tag="mm")
                nc.tensor.transpose(tp_ps[:], att_tiles[t][:, 128 * jc:128 * (jc + 1)],
                                    ident_s[:])
                nc.vector.tensor_copy(attT[jc][:, 128 * t:128 * (t + 1)], tp_ps[:])

        # AV per head + elu
        hGelu = []
        for h in range(NH):
            wg_s = sb2.tile([HID, OUT], F32, tag="wgnat")
            nc.gpsimd.dma_start(wg_s[:], b16('Wgat', h))
            hg_ps = psa.tile([OUT, NPC], F32, tag="hg")
            for jc in range(4):
                wh_ps = psb.tile([128, OUT], F32, tag="mm")
                nc.tensor.matmul(wh_ps[:], hT[:, 128 * jc:128 * (jc + 1)], wg_s[:],
                                 start=True, stop=True)
                wh_sb = sb2.tile([128, OUT], F32, tag="whsb")
                nc.vector.tensor_copy(wh_sb[:], wh_ps[:])
                rhs = _ap(attT[jc][:].tensor, h, [[1024, 128], [128, 8], [16, 8]])
                nc.tensor.matmul(hg_ps[:], wh_sb[:], rhs, start=(jc == 0), stop=(jc == 3))
            hg = sb.tile([OUT, NPC], F32, tag=f"hgelu{h}")
            elu_inplace(hg_ps[:], hg[:], [OUT, NPC], tagp="elug")
            hGelu.append(hg)

        # pair gates
        dpa_ps = psa.tile([1, NPC], F32, tag="accA")
        dpb_ps = psa.tile([1, NPC], F32, tag="accB")
        for h in range(NH):
            wp_s = sb2.tile([OUT, 2], F32, tag="wps")
            nc.sync.dma_start(wp_s[:], b32('wp1ab', h))
            st, sp = (h == 0), (h == NH - 1)
            nc.tensor.matmul(dpa_ps[:], wp_s[:, 0:1], hGelu[h][:], start=st, stop=sp)
            nc.tensor.matmul(dpb_ps[:], wp_s[:, 1:2], hGelu[h][:], start=st, stop=sp)
        dk = sb.tile([1, NPC // 2], F32, tag="dk")
        dasb = sb.tile([1, NPC], F32, tag="dasb")
        nc.vector.tensor_copy(dasb[:], dpa_ps[:])
        a_ap = _ap(dasb[:].tensor, 0, [[NPC, 1], [2, NPC // 2]])
        b_ap = _ap(dpb_ps[:].tensor, 1, [[NPC, 1], [2, NPC // 2]])
        nc.vector.tensor_tensor(dk[:], a_ap, b_ap, ALU.add)
        sgate = sb.tile([1, NPC // 2], F32, tag="sgate")
        nc.scalar.activation(sgate[:], dk[:], AF.Sigmoid, bias=bp1_s[:])
        srep_ps = psa.tile([128, NPC // 2], F32, tag="accC")
        nc.tensor.matmul(srep_ps[:], ones1_128[:], sgate[:], start=True, stop=True)

        h1T = []
        for h in range(NH):
            ev_ap = _ap(hGelu[h][:].tensor, 0, [[NPC, OUT], [2, NPC // 2]])
            od_ap = _ap(hGelu[h][:].tensor, 1, [[NPC, OUT], [2, NPC // 2]])
            t1 = sb2.tile([OUT, NPC // 2], F32, tag="pairsum")
            nc.vector.tensor_tensor(t1[:], ev_ap, od_ap, ALU.add)
            h1 = sb.tile([OUT, NPC // 2], F32, tag=f"h1T{h}")
            nc.vector.tensor_tensor(h1[:], t1[:], srep_ps[:], ALU.mult)
            h1T.append(h1)

        # g2 / u / Z / P
        g2_ps = psa.tile([1, NPC // 2], F32, tag="accA")
        for h in range(NH):
            wg2_s = sb2.tile([OUT, 1], F32, tag="wg2s")
            nc.sync.dma_start(wg2_s[:], b32('Wg2r', h))
            nc.tensor.matmul(g2_ps[:], wg2_s[:], h1T[h][:],
                             start=(h == 0), stop=(h == NH - 1))
        sg2 = sb.tile([1, NPC // 2], F32, tag="sg2")
        nc.scalar.activation(sg2[:], g2_ps[:], AF.Sigmoid, bias=bg2_s[:])
        u = sb.tile([1, NPC // 2], F32, tag="u")
        nc.scalar.activation(u[:], sg2[:], AF.Exp)
        Zc = sb.tile([1, 1], F32, tag="Zc")
        nc.vector.tensor_reduce(Zc[:], u[:], AX.X, ALU.add)
        nc.sync.dma_start(o_all[0:1, 49:50], Zc[:])
        urep_ps = psa.tile([128, NPC // 2], F32, tag="accB")
        nc.tensor.matmul(urep_ps[:], ones1_128[:], u[:], start=True, stop=True)
        Pout = sb.tile([OUT, NH], F32, tag="Pout")
        for h in range(NH):
            pm = sb2.tile([OUT, NPC // 2], F32, tag="pm")
            nc.vector.tensor_tensor(pm[:], h1T[h][:], urep_ps[:OUT, :], ALU.mult)
            nc.vector.tensor_reduce(Pout[:, h:h + 1], pm[:], AX.X, ALU.add)
        nc.sync.dma_start(o_all[:, 0:16], Pout[:])

        # Wh2T rows
        wh2_ps = psa.tile([OUT, NPC // 2], F32, tag="accC")
        for h in range(NH):
            wo_s = sb2.tile([OUT, OUT], F32, tag="wos")
            nc.gpsimd.dma_start(wo_s[:], b16('Wor', h))
            nc.tensor.matmul(wh2_ps[:], wo_s[:], h1T[h][:],
                             start=(h == 0), stop=(h == NH - 1))
        wh2 = sb.tile([OUT, NPC // 2], F32, tag="wh2sb")
        nc.vector.tensor_copy(wh2[:], wh2_ps[:])
        nc.sync.dma_start(o_all[:, 16:48], wh2[:])

    nc.compile()
    return nc


# ---------------------------------------------------------------- dispatch B
def build_B():
    nc = bacc.Bacc("TRN2", target_bir_lowering=False, debug=False, num_devices=1)

    def inp(name, shape, dt=F32):
        return nc.dram_tensor(name, shape, dt, kind="ExternalInput").ap()

    blob32 = inp("blobB32", [LENB32])
    blob16 = inp("blobB16", [LENB16], BF16)

    def b32(name):
        off, shape = OFFB32[name]
        rows, cols = (shape[0], int(np.prod(shape[1:]))) if len(shape) > 1 else (1, shape[0])
        return _ap(blob32.tensor, off, [[cols, rows], [1, cols]])

    def b16w(d, k, r0, rn, c0, cn, which):
        off, shape = OFFB16[which]
        base = off + ((d * shape[1] + k) * 128) * (4 * LH)
        return _ap(blob16.tensor, base + r0 * 4 * LH + c0, [[4 * LH, rn], [1, cn]])
    o_prob = nc.dram_tensor("o_prob", [1, NCLS], F32, kind="ExternalOutput").ap()

    with tile.TileContext(nc) as tc, ExitStack() as ctx:
        sb = ctx.enter_context(tc.tile_pool(name="sb", bufs=1))
        sb2 = ctx.enter_context(tc.tile_pool(name="sb2", bufs=2))
        psa = ctx.enter_context(tc.tile_pool(name="psa", bufs=1, space="PSUM"))
        psb = ctx.enter_context(tc.tile_pool(name="psb", bufs=2, space="PSUM"))

        def load(apx, shape, dt=F32, pool=sb, tag=None):
            t = pool.tile(shape, dt, tag=tag)
            nc.sync.dma_start(t[:], apx)
            return t

        ident_s = load(b32("identB"), [128, 128], tag="ident")
        ones1 = sb.tile([1, 128], F32, tag="ones1")
        nc.gpsimd.memset(ones1[:], 1.0)
        Pall_s = load(b32("Pall"), [OUT, NC * NH], tag="Pall")
        Zall_s = load(b32("Zall"), [1, NC], tag="Zall")
        hs0_s = load(b32("hs0"), [HID, 1], tag="hs0")
        Wh2T_s = load(b32("Wh2T"), [OUT, N2], tag="Wh2T")
        a12o_s = load(b32("a12o"), [OUT, 2], tag="a12o")
        wp2_s = load(b32("wp2ab"), [OUT, 2], tag="wp2")
        bp2_s = load(b32("bp2"), [1, 1], tag="bp2")
        Wg3_s = load(b32("Wg3"), [OUT, 1], tag="Wg3")
        bg3_s = load(b32("bg3"), [1, 1], tag="bg3")
        fcb_s = load(b32("fcb"), [1, NCLS], tag="fcb")

        # hs1 columns [128, 16] = sum_c Pall[:, c*16+h] / Z
        hs1c = sb.tile([OUT, NH], F32, tag="hs1c")
        src = _ap(Pall_s[:].tensor, 0, [[NC * NH, OUT], [1, NH], [NH, NC]])
        nc.vector.tensor_reduce(hs1c[:], src, AX.X, ALU.add)
        Zt = sb.tile([1, 1], F32, tag="Zt")
        nc.vector.tensor_reduce(Zt[:], Zall_s[:], AX.X, ALU.add)
        iZ = sb.tile([1, 1], F32, tag="iZ")
        nc.vector.reciprocal(iZ[:], Zt[:])
        izrep_ps = psa.tile([128, 1], F32, tag="r1")
        nc.tensor.matmul(izrep_ps[:], ones1[:], iZ[:], start=True, stop=True)
        izcol = sb.tile([128, 1], F32, tag="izcol")
        nc.vector.tensor_copy(izcol[:], izrep_ps[:])
        nc.vector.tensor_scalar(hs1c[:], hs1c[:], izcol[:OUT, :], None, op0=ALU.mult)

        # att2 scores
        s1o_ps = psa.tile([1, N2], F32, tag="r2")
        nc.tensor.matmul(s1o_ps[:], a12o_s[:, 0:1], Wh2T_s[:], start=True, stop=True)
        s2o_ps = psa.tile([1, N2], F32, tag="r3")
        nc.tensor.matmul(s2o_ps[:], a12o_s[:, 1:2], Wh2T_s[:], start=True, stop=True)
        s1o = sb.tile([1, N2], F32, tag="s1osb")
        nc.vector.tensor_copy(s1o[:], s1o_ps[:])
        s2o = sb.tile([1, N2], F32, tag="s2osb")
        nc.vector.tensor_copy(s2o[:], s2o_ps[:])
        s2orep_ps = psa.tile([128, N2], F32, tag="r4")
        nc.tensor.matmul(s2orep_ps[:], ones1[:], s2o[:], start=True, stop=True)

        att2 = []
        for t2 in range(2):
            s1c_ps = psb.tile([128, 1], F32, tag="mmB")
            nc.tensor.transpose(s1c_ps[:], s1o[:, 128 * t2:128 * (t2 + 1)], ident_s[0:1, 0:1])
            s1c = sb2.tile([128, 1], F32, tag="s1c")
            nc.vector.tensor_copy(s1c[:], s1c_ps[:])
            e3t = sb2.tile([128, N2], F32, tag="e3t")
            nc.sync.dma_start(e3t[:], _ap(blob32.tensor, OFFB32['e3_2'][0] + 128 * t2 * N2, [[N2, 128], [1, N2]]))
            adt = sb2.tile([128, N2], F32, tag="adt")
            nc.sync.dma_start(adt[:], _ap(blob32.tensor, OFFB32['adjm2'][0] + 128 * t2 * N2, [[N2, 128], [1, N2]]))
            e1 = sb2.tile([128, N2], F32, tag="e1b")
            nc.vector.tensor_tensor(e1[:], e3t[:], s2orep_ps[:], ALU.add)
            lr = sb2.tile([128, N2], F32, tag="lrb")
            nc.scalar.activation(lr[:], e1[:], AF.Lrelu, bias=s1c[:], alpha=0.2)
            m1 = sb2.tile([128, N2], F32, tag="m1b")
            nc.vector.scalar_tensor_tensor(m1[:], lr[:], 1e9, adt[:],
                                           op0=ALU.add, op1=ALU.mult)
            nmax = sb2.tile([128, 1], F32, tag="nmaxb")
            nc.vector.tensor_reduce(nmax[:], m1[:], AX.X, ALU.max, negate=True)
            pt = sb2.tile([128, N2], F32, tag="ptb")
            zt = sb2.tile([128, 1], F32, tag="ztb")
            nc.scalar.activation(pt[:], m1[:], AF.Exp, bias=nmax[:], accum_out=zt[:])
            izt = sb2.tile([128, 1], F32, tag="iztb")
            nc.vector.reciprocal(izt[:], zt[:])
            at = sb.tile([128, N2], F32, tag=f"att2_{t2}")
            nc.vector.tensor_scalar(at[:], pt[:], izt[:], None, op0=ALU.mult)
            att2.append(at)

        # att2T + h2T
        attT2 = []
        for lc in range(2):
            big = sb.tile([128, N2], F32, tag=f"attT2_{lc}")
            attT2.append(big)
        for t2 in range(2):
            for lc in range(2):
                tp_ps = psb.tile([128, 128], F32, tag="mmB")
                nc.tensor.transpose(tp_ps[:], att2[t2][:, 128 * lc:128 * (lc + 1)],
                                    ident_s[:])
                nc.vector.tensor_copy(attT2[lc][:, 128 * t2:128 * (t2 + 1)], tp_ps[:])
        h2_ps = psa.tile([OUT, N2], F32, tag="r5")
        for lc in range(2):
            w2n_s = sb2.tile([128, OUT], F32, tag="w2n")
            nc.sync.dma_start(w2n_s[:], _ap(blob32.tensor, OFFB32['Wh2nat'][0] + 128 * lc * OUT, [[OUT, 128], [1, OUT]]))
            nc.tensor.matmul(h2_ps[:], w2n_s[:], attT2[lc][:],
                             start=(lc == 0), stop=(lc == 1))
        h2T = sb.tile([OUT, N2], F32, tag="h2T")
        nc.vector.tensor_copy(h2T[:], h2_ps[:])

        # edge pool 2
        dpa_ps = psa.tile([1, N2], F32, tag="r1")
        nc.tensor.matmul(dpa_ps[:], wp2_s[:, 0:1], h2T[:], start=True, stop=True)
        dpb_ps = psa.tile([1, N2], F32, tag="r2")
        nc.tensor.matmul(dpb_ps[:], wp2_s[:, 1:2], h2T[:], start=True, stop=True)
        dk2 = sb.tile([1, N3], F32, tag="dk2")
        dasb2 = sb.tile([1, N2], F32, tag="dasb2")
        nc.vector.tensor_copy(dasb2[:], dpa_ps[:])
        a_ap = _ap(dasb2[:].tensor, 0, [[N2, 1], [2, N3]])
        b_ap = _ap(dpb_ps[:].tensor, 1, [[N2, 1], [2, N3]])
        nc.vector.tensor_tensor(dk2[:], a_ap, b_ap, ALU.add)
        s2k = sb.tile([1, N3], F32, tag="s2k")
        nc.scalar.activation(s2k[:], dk2[:], AF.Sigmoid, bias=bp2_s[:])
        srep2_ps = psa.tile([128, N3], F32, tag="r3")
        nc.tensor.matmul(srep2_ps[:], ones1[:], s2k[:], start=True, stop=True)
        ev_ap = _ap(h2T[:].tensor, 0, [[N2, OUT], [2, N3]])
        od_ap = _ap(h2T[:].tensor, 1, [[N2, OUT], [2, N3]])
        t12 = sb.tile([OUT, N3], F32, tag="t12")
        nc.vector.tensor_tensor(t12[:], ev_ap, od_ap, ALU.add)
        h3T = sb.tile([OUT, N3], F32, tag="h3T")
        nc.vector.tensor_tensor(h3T[:], t12[:], srep2_ps[:OUT, :], ALU.mult)

        # gpool3 -> hs2 [128, 1]
        g3_ps = psa.tile([1, N3], F32, tag="r1")
        nc.tensor.matmul(g3_ps[:], Wg3_s[:], h3T[:], start=True, stop=True)
        g3s = sb.tile([1, N3], F32, tag="g3s")
        nc.scalar.activation(g3s[:], g3_ps[:], AF.Sigmoid, bias=bg3_s[:])
        nm3 = sb.tile([1, 1], F32, tag="nm3")
        nc.vector.tensor_reduce(nm3[:], g3s[:], AX.X, ALU.max, negate=True)
        w3 = sb.tile([1, N3], F32, tag="w3")
        z3 = sb.tile([1, 1], F32, tag="z3")
        nc.scalar.activation(w3[:], g3s[:], AF.Exp, bias=nm3[:], accum_out=z3[:])
        iz3 = sb.tile([1, 1], F32, tag="iz3")
        nc.vector.reciprocal(iz3[:], z3[:])
        nc.vector.tensor_scalar(w3[:], w3[:], iz3[:], None, op0=ALU.mult)
        w3rep_ps = psa.tile([128, N3], F32, tag="r2")
        nc.tensor.matmul(w3rep_ps[:], ones1[:], w3[:], start=True, stop=True)
        hw3 = sb.tile([OUT, N3], F32, tag="hw3")
        nc.vector.tensor_tensor(hw3[:], h3T[:], w3rep_ps[:OUT, :], ALU.mult)
        hs2 = sb.tile([OUT, 1], F32, tag="hs2")
        nc.vector.tensor_reduce(hs2[:], hw3[:], AX.X, ALU.add)

        # x chunks [128, 18] bf16: cols 0-15 hs1c, col16 [hs0; hs2[0:64]], col17 [hs2[64:]; 1]
        xc = sb.tile([128, 18], F32, tag="xc")
        nc.gpsimd.memset(xc[:], 0.0)
        nc.vector.tensor_copy(xc[:OUT, 0:NH], hs1c[:])
        nc.vector.tensor_copy(xc[:HID, 16:17], hs0_s[:])
        nc.sync.dma_start(xc[HID:128, 16:17], hs2[0:HID, :])
        nc.sync.dma_start(xc[0:HID, 17:18], hs2[HID:OUT, :])
        nc.gpsimd.memset(xc[HID:HID + 1, 17:18], 1.0)
        xcb = sb.tile([128, 18], BF16, tag="xcb")
        nc.vector.tensor_copy(xcb[:], xc[:])

        # LSTM layer 0 (M-orientation, skip f-gate m=1)
        h0 = []
        for d in range(2):
            g_ps = psa.tile([128, 4], F32, tag="gacc")
            for m in (0, 2, 3):
                for k in range(18):
                    rows = 65 if k == 17 else 128
                    w_s = sb2.tile([128, 128], BF16, tag="w0s")
                    nc.sync.dma_start(w_s[:rows, :], b16w(d, k, 0, rows, 128 * m, 128, 'W0b'))
                    nc.tensor.matmul(g_ps[:, m:m + 1], w_s[:rows, :], xcb[:rows, k:k + 1],
                                     start=(k == 0), stop=(k == 17))
            si = sb2.tile([128, 1], F32, tag="si")
            nc.scalar.activation(si[:], g_ps[:, 0:1], AF.Sigmoid)
            tg = sb2.tile([128, 1], F32, tag="tg")
            nc.scalar.activation(tg[:], g_ps[:, 2:3], AF.Tanh)
            so = sb2.tile([128, 1], F32, tag="so")
            nc.scalar.activation(so[:], g_ps[:, 3:4], AF.Sigmoid)
            c = sb2.tile([128, 1], F32, tag="c0")
            nc.vector.tensor_tensor(c[:], si[:], tg[:], ALU.mult)
            tc_ = sb2.tile([128, 1], F32, tag="tc0")
            nc.scalar.activation(tc_[:], c[:], AF.Tanh)
            hd = sb.tile([128, 1], F32, tag=f"h0_{d}")
            nc.vector.tensor_tensor(hd[:], so[:], tc_[:], ALU.mult)
            h0.append(hd)
        h0b_ = []
        for d in range(2):
            hb = sb.tile([128, 1], BF16, tag=f"h0b_{d}")
            nc.vector.tensor_copy(hb[:], h0[d][:])
            h0b_.append(hb)
        onesb = sb.tile([1, 1], BF16, tag="onesb")
        nc.gpsimd.memset(onesb[:], 1.0)

        # LSTM layer 1
        h1o = []
        for d in range(2):
            g_ps = psa.tile([128, 4], F32, tag="gacc")
            for m in (0, 2, 3):
                for k in range(3):
                    rows = 1 if k == 2 else 128
                    w_s = sb2.tile([128, 128], BF16, tag="w1s")
                    nc.sync.dma_start(w_s[:rows, :], b16w(d, k, 0, rows, 128 * m, 128, 'W1b'))
                    rhs = onesb[:] if k == 2 else h0b_[k][:]
                    nc.tensor.matmul(g_ps[:, m:m + 1], w_s[:rows, :], rhs,
                                     start=(k == 0), stop=(k == 2))
            si = sb2.tile([128, 1], F32, tag="si1")
            nc.scalar.activation(si[:], g_ps[:, 0:1], AF.Sigmoid)
            tg = sb2.tile([128, 1], F32, tag="tg1")
            nc.scalar.activation(tg[:], g_ps[:, 2:3], AF.Tanh)
            so = sb2.tile([128, 1], F32, tag="so1")
            nc.scalar.activation(so[:], g_ps[:, 3:4], AF.Sigmoid)
            c = sb2.tile([128, 1], F32, tag="c1")
            nc.vector.tensor_tensor(c[:], si[:], tg[:], ALU.mult)
            tc_ = sb2.tile([128, 1], F32, tag="tc1")
            nc.scalar.activation(tc_[:], c[:], AF.Tanh)
            hd = sb.tile([128, 1], F32, tag=f"h1_{d}")
            nc.vector.tensor_tensor(hd[:], so[:], tc_[:], ALU.mult)
            h1o.append(hd)

        # fc + softmax
        lg_ps = psa.tile([1, NCLS], F32, tag="r1")
        fcw0 = sb.tile([LH, NCLS], F32, tag="fcw0")
        nc.sync.dma_start(fcw0[:], _ap(blob32.tensor, OFFB32['fcWr'][0], [[NCLS, LH], [1, NCLS]]))
        fcw1 = sb.tile([LH, NCLS], F32, tag="fcw1")
        nc.sync.dma_start(fcw1[:], _ap(blob32.tensor, OFFB32['fcWr'][0] + LH * NCLS, [[NCLS, LH], [1, NCLS]]))
        nc.tensor.matmul(lg_ps[:], h1o[0][:], fcw0[:], start=True, stop=False)
        nc.tensor.matmul(lg_ps[:], h1o[1][:], fcw1[:], start=False, stop=True)
        lg = sb.tile([1, NCLS], F32, tag="lg")
        nc.vector.tensor_tensor(lg[:], lg_ps[:], fcb_s[:], ALU.add)
        nmf = sb.tile([1, 1], F32, tag="nmf")
        nc.vector.tensor_reduce(nmf[:], lg[:], AX.X, ALU.max, negate=True)
        pf = sb.tile([1, NCLS], F32, tag="pf")
        zf = sb.tile([1, 1], F32, tag="zf")
        nc.scalar.activation(pf[:], lg[:], AF.Exp, bias=nmf[:], accum_out=zf[:])
        izf = sb.tile([1, 1], F32, tag="izf")
        nc.vector.reciprocal(izf[:], zf[:])
        prob = sb.tile([1, NCLS], F32, tag="prob")
        nc.vector.tensor_scalar(prob[:], pf[:], izf[:], None, op0=ALU.mult)
        nc.sync.dma_start(o_prob[:], prob[:])

    nc.compile()
    return nc


# ---------------------------------------------------------------- host prep
def _prep_A(inputs):
    """Build per-core input maps for dispatch A. Pure layout/indexing."""
    f32 = np.float32
    import ml_dtypes
    bf = ml_dtypes.bfloat16
    ei = np.asarray(inputs["edge_index"])
    feats = np.asarray(inputs["features"], f32)
    n2n = np.asarray(inputs["node2node_features"], f32)
    eattr = np.asarray(inputs["edgesAttr"], f32)
    adjacency = np.asarray(inputs["adjacency"], f32)

    src, dst = np.asarray(ei[0], np.int64), np.asarray(ei[1], np.int64)
    pairs = src * N + dst
    uniq = np.unique(pairs)
    us, ud = uniq // N, uniq % N
    order = np.argsort(us, kind="stable")
    us, ud, uniq = us[order], ud[order], uniq[order]
    counts = np.bincount(us, minlength=N)
    assert counts.max() <= S, f"out-degree {counts.max()} > {S}"
    starts = np.zeros(N + 1, np.int64)
    np.cumsum(counts, out=starts[1:])
    slots = np.arange(len(us)) - starts[us]

    featT = np.ascontiguousarray(feats.T)
    eaT = np.ascontiguousarray(eattr.T)
    W_gat = np.asarray(inputs["W_gat"], f32)

    sh32 = {
        "featT": featT,
        "W_sn": np.asarray(inputs["W_sn"], f32),
        "a_sn": np.asarray(inputs["a_sn"], f32).reshape(HID, 1),
        "Wg1": np.asarray(inputs["Wg1"], f32).reshape(HID, 1),
        "bg1": np.asarray(inputs["bg1"], f32).reshape(1, 1),
        "a12": np.stack([np.asarray(inputs["a1_gat"], f32),
                         np.asarray(inputs["a2_gat"], f32)], -1),
        "a3t128": np.tile(np.asarray(inputs["a3_gat"], f32).T, (1, 8)),
        "a3oT": np.asarray(inputs["a3_o"], f32).reshape(NH, OUT).T,
        "wp1ab": np.stack([
            np.asarray(inputs["Wp1"], f32)[:D1, 0].reshape(NH, OUT),
            np.asarray(inputs["Wp1"], f32)[D1:, 0].reshape(NH, OUT)], -1),
        "Wg2r": np.asarray(inputs["Wg2"], f32).reshape(NH, OUT, 1),
        "bp1": np.asarray(inputs["bp1"], f32).reshape(1, 1),
        "bg2": np.asarray(inputs["bg2"], f32).reshape(1, 1),
        "selh2": np.eye(NH, dtype=f32)[:, np.tile(np.arange(NH), 8)].reshape(NH, 128),
    }
    selrep = np.zeros((NPC, NC * 128), f32)
    for t in range(8):
        for p in range(128):
            selrep[8 * t + p // 16, 128 * t + p] = 1.0
    sh16 = {
        "Wgat": W_gat,
        "Wegat": np.asarray(inputs["We_gat"], f32),
        "Wor": np.asarray(inputs["Wo"], f32).reshape(NH, OUT, OUT),
        "selrep": selrep,
        "ident": np.eye(128, dtype=f32),
    }

    in_maps = []
    for c in range(NC):
        lo = c * NPC
        d32 = dict(sh32)
        d16 = dict(sh16)
        d32["featTm"] = featT[:, lo:lo + NPC]
        mask = (us >= lo) & (us < lo + NPC)
        cs, cd, csl = us[mask] - lo, ud[mask], slots[mask]
        XP = np.zeros((NPC * S, HID), f32)
        XP[cs * S + csl] = n2n[uniq[mask]]
        d16["XP"] = XP.T
        ptr = np.full((NPC, N), NPC * S, np.int64)
        ptr[cs, cd] = cs * S + csl
        g = np.zeros((128, 256), np.int16)
        for t in range(8):
            for gg in range(8):
                row = ptr[8 * t + gg]
                g[16 * gg:16 * gg + 16, 32 * t:32 * t + 32] = \
                    row.reshape(32, 16).T.astype(np.int16)
        d32["gidxbits"] = g.view(f32)
        d16["adjmine"] = adjacency[lo:lo + NPC]
        d16["eaT"] = eaT[:, c * EPC:(c + 1) * EPC]
        blob32 = np.empty(LEN32, f32)
        for name, shape in SPEC32:
            off, _ = OFF32[name]
            blob32[off:off + int(np.prod(shape))] = np.ascontiguousarray(d32[name], f32).reshape(-1)
        blob16 = np.empty(LEN16, bf)
        for name, shape in SPEC16:
            off, _ = OFF16[name]
            blob16[off:off + int(np.prod(shape))] = np.ascontiguousarray(d16[name], f32).reshape(-1).astype(bf)
        in_maps.append({"blob32": blob32, "blob16": blob16})
    return in_maps, (src, dst)


def _prep_B(inputs, resA, ei_sd):
    f32 = np.float32
    src, dst = ei_sd
    unp = []
    for c in range(NC):
        o = resA[c]["o_all"]
        unp.append({"o_P": o[:, 0:16], "o_Wh2T": o[:, 16:48],
                    "o_hs0": o[0:HID, 48:49], "o_Z": o[0:1, 49:50],
                    "o_es": o[:, 50:58].reshape(-1)})
    resA = unp
    es = np.concatenate([resA[c]["o_es"].reshape(-1) for c in range(NC)])
    s2, d2 = src // 2, dst // 2
    adj2 = np.zeros((N2, N2), f32)
    adj2[s2, d2] = 1.0
    e3_2 = np.zeros((N2, N2), f32)
    e3_2[s2, d2] = es  # numpy fancy assignment: last occurrence wins
    Wh2T = np.concatenate([resA[c]["o_Wh2T"] for c in range(NC)], axis=1)
    Pall = np.concatenate([resA[c]["o_P"] for c in range(NC)], axis=1)
    Zall = np.concatenate([resA[c]["o_Z"].reshape(1, 1) for c in range(NC)], axis=1)

    # LSTM weights: my-x order = [hs1(2048), hs0(64), hs2(128), bias(1)]
    perm = np.concatenate([np.arange(64, 2112), np.arange(0, 64), np.arange(2112, 2240)])
    W0 = np.zeros((2, 18, 128, 4 * LH), f32)
    for d in range(2):
        wt = np.asarray(inputs["Wih0"], f32)[d].T[perm]         # [2240, 512]
        wb = np.concatenate([wt, np.asarray(inputs["b0"], f32)[d][None, :]], 0)  # [2241,512]
        for k in range(18):
            rows = wb[128 * k:128 * (k + 1)]
            W0[d, k, :rows.shape[0], :] = rows
    W1 = np.zeros((2, 3, 128, 4 * LH), f32)
    for d in range(2):
        wt = np.asarray(inputs["Wih1"], f32)[d].T               # [256, 512]
        wb = np.concatenate([wt, np.asarray(inputs["b1"], f32)[d][None, :]], 0)
        for k in range(3):
            rows = wb[128 * k:128 * (k + 1)]
            W1[d, k, :rows.shape[0], :] = rows
    import ml_dtypes
    bf = ml_dtypes.bfloat16

    d32 = {
        "adjm2": adj2,
        "e3_2": e3_2,
        "Wh2T": Wh2T,
        "Wh2nat": Wh2T.T,
        "a12o": np.stack([np.asarray(inputs["a1_o"], f32), np.asarray(inputs["a2_o"], f32)], -1),
        "wp2ab": np.stack([np.asarray(inputs["Wp2"], f32)[:OUT, 0],
                           np.asarray(inputs["Wp2"], f32)[OUT:, 0]], -1),
        "bp2": np.asarray(inputs["bp2"], f32).reshape(1, 1),
        "Wg3": np.asarray(inputs["Wg3"], f32).reshape(OUT, 1),
        "bg3": np.asarray(inputs["bg3"], f32).reshape(1, 1),
        "fcWr": np.stack([np.asarray(inputs["fc_W"], f32)[:LH],
                          np.asarray(inputs["fc_W"], f32)[LH:]]),
        "fcb": np.asarray(inputs["fc_b"], f32).reshape(1, NCLS),
        "Pall": Pall,
        "Zall": Zall,
        "hs0": resA[0]["o_hs0"].reshape(HID, 1),
        "identB": np.eye(128, dtype=f32),
    }
    blob32 = np.empty(LENB32, f32)
    for name, shape in SPECB32:
        off, _ = OFFB32[name]
        blob32[off:off + int(np.prod(shape))] = np.ascontiguousarray(d32[name], f32).reshape(-1)
    blob16 = np.empty(LENB16, bf)
    o0, _ = OFFB16["W0b"]
    blob16[o0:o0 + W0.size] = W0.reshape(-1).astype(bf)
    o1, _ = OFFB16["W1b"]
    blob16[o1:o1 + W1.size] = W1.reshape(-1).astype(bf)
    return {"blobB32": blob32, "blobB16": blob16}


# ------------------------------------------------------- cached SPMD runner
class _CachedRunner:
    """Like bass2jax.run_bass_via_pjrt but with the jitted callable built once."""

    def __init__(self, nc, n_cores):
        import jax
        from jax.sharding import Mesh, PartitionSpec
        from jax.experimental.shard_map import shard_map
        from concourse import bass2jax
        bass2jax.install_neuronx_cc_hook()
        self.n_cores = n_cores
        partition_name = nc.partition_id_tensor.name if nc.partition_id_tensor else None
        in_names, out_names, out_avals, zero_outs = [], [], [], []
        for alloc in nc.m.functions[0].allocations:
            if not isinstance(alloc, mybir.MemoryLocationSet):
                continue
            name = alloc.memorylocations[0].name
            if alloc.kind == "ExternalInput":
                if name != partition_name:
                    in_names.append(name)
            elif alloc.kind == "ExternalOutput":
                shape = tuple(alloc.tensor_shape)
                dtype = mybir.dt.np(alloc.dtype)
                out_names.append(name)
                out_avals.append(jax.core.ShapedArray(shape, dtype))
                zero_outs.append(np.zeros(shape, dtype))
        self.in_names, self.out_names = in_names, out_names
        self.out_avals, self.zero_outs = out_avals, zero_outs
        n_params, n_outs = len(in_names), len(out_names)
        all_names = in_names + out_names
        if partition_name is not None:
            all_names = all_names + [partition_name]
        donate = tuple(range(n_params, n_params + n_outs))

        def _body(*args):
            operands = list(args)
            if partition_name is not None:
                operands.append(bass2jax.partition_id_tensor())
            outs = bass2jax._bass_exec_p.bind(
                *operands,
                out_avals=tuple(out_avals),
                in_names=tuple(all_names),
                out_names=tuple(out_names),
                lowering_input_output_aliases=(),
                sim_require_finite=True,
                sim_require_nnan=True,
                nc=nc,
            )
            return tuple(outs)

        self._body = _body
        self._jax = jax
        self._Mesh, self._P, self._shard_map = Mesh, PartitionSpec, shard_map
        self.donate = donate
        self.n_params, self.n_outs = n_params, n_outs
        self.fn = None
        if n_cores == 1:
            self.fn = jax.jit(_body, donate_argnums=donate, keep_unused=True)

    def _build_multi(self, shared_flags):
        jax = self._jax
        devices = jax.devices()[:self.n_cores]
        mesh = self._Mesh(np.asarray(devices), ("core",))
        self.shared_flags = shared_flags
        in_specs = tuple(self._P() if f else self._P("core") for f in shared_flags) \
            + (self._P("core"),) * self.n_outs
        out_specs = (self._P("core"),) * self.n_outs
        self.fn = jax.jit(
            self._shard_map(self._body, mesh=mesh, in_specs=in_specs,
                            out_specs=out_specs, check_rep=False),
            donate_argnums=self.donate, keep_unused=True)

    def __call__(self, in_maps):
        nc_ = self.n_cores
        if nc_ == 1:
            out = self.fn(*[np.asarray(in_maps[0][n]) for n in self.in_names],
                          *self.zero_outs)
            return [{n: np.asarray(out[i]) for i, n in enumerate(self.out_names)}]
        if self.fn is None:
            flags = [all(in_maps[c][n] is in_maps[0][n] for c in range(nc_))
                     for n in self.in_names]
            self._build_multi(flags)
        args = []
        for i, n in enumerate(self.in_names):
            if self.shared_flags[i]:
                args.append(np.asarray(in_maps[0][n]))
            else:
                args.append(np.concatenate(
                    [np.asarray(in_maps[c][n]) for c in range(nc_)], axis=0))
        concat_zeros = [np.zeros((nc_ * z.shape[0], *z.shape[1:]), z.dtype)
                        for z in self.zero_outs]
        out = self.fn(*args, *concat_zeros)
        res = []
        for c in range(nc_):
            res.append({n: np.asarray(out[i]).reshape(nc_, *self.out_avals[i].shape)[c]
                        for i, n in enumerate(self.out_names)})
        return res


# ---------------------------------------------------------------- entrypoint
def kernel(**inputs):
    if "A" not in _cache:
        _cache["A"] = _CachedRunner(build_A(), NC)
    if "B" not in _cache:
        _cache["B"] = _CachedRunner(build_B(), 1)
    in_maps, ei_sd = _prep_A(inputs)
    resA = _cache["A"](in_maps)
    inB = _prep_B(inputs, resA, ei_sd)
    resB = _cache["B"]([inB])
    return resB[0]["o_prob"].reshape(NCLS).astype(np.float32)


# revision 15
# speedup vs baseline: 5.5315x; 1.3905x over previous
"""Trainium2 Bass kernel for nn_DefectDetection (GAT + pooling + LSTM head).

Self-contained: accepts FULL inputs, shards across 8 NeuronCores internally.

Strategy:
  Dispatch A (8 cores, SPMD):
    - replicated small front-end (node-attention layer, gpool1, GAT projections)
    - node-row-sharded dense [N,N] attention maps (64 rows x 16 heads / core),
      with the sparse node2node e3 term built from a host-packed slot grid via
      one matmul + gpsimd ap_gather (no 64MiB dense read)
    - edge-sharded edge-attr score reduction (es)
    - per-core outputs: es slice, gpool2 partials (P,Z), Wh2 rows, hs0
  Host in between: pure data movement (concat / scatter by precomputed indices).
  Dispatch B (1 core): pooled-graph attention (256 nodes), edge pool 2, gpool3,
    2-layer bi-LSTM (T=1) with bf16 weights, fc + softmax -> [2].
"""
import numpy as np
from contextlib import ExitStack

import concourse.bass as bass
import concourse.bacc as bacc
import concourse.tile as tile
import concourse.mybir as mybir
from concourse.bass_utils import run_bass_kernel_spmd

F32 = mybir.dt.float32
BF16 = mybir.dt.bfloat16
I16 = mybir.dt.int16
AF = mybir.ActivationFunctionType
ALU = mybir.AluOpType
AX = mybir.AxisListType

N, E, HID, NH, OUT, NCLS, LH = 512, 8192, 64, 16, 128, 2, 128
NC = 8          # cores
NPC = N // NC   # 64 nodes per core
S = 64          # slot grid per node
EPC = E // NC   # 1024 edges per core (F stage)
D1 = NH * OUT   # 2048
N2 = N // 2     # 256
N3 = N // 4     # 128
JUMP = HID + D1 + OUT  # 2240

_cache = {}

# blob layouts: (name, shape) -> row-major at running offset
SPEC32 = [
    ("featT", (HID, N)), ("featTm", (HID, NPC)), ("W_sn", (HID, HID)),
    ("a_sn", (HID, 1)), ("Wg1", (HID, 1)), ("bg1", (1, 1)),
    ("a12", (NH, OUT, 2)), ("a3t128", (HID, 128)), ("a3oT", (OUT, NH)),
    ("wp1ab", (NH, OUT, 2)), ("Wg2r", (NH, OUT, 1)), ("bp1", (1, 1)),
    ("bg2", (1, 1)), ("selh2", (NH, 128)), ("gidxbits", (128, 128)),
]
SPEC16 = [
    ("Wgat", (NH, HID, OUT)), ("Wegat", (NH, HID, OUT)), ("Wor", (NH, OUT, OUT)),
    ("XP", (HID, NPC * S)), ("eaT", (HID, EPC)), ("adjmine", (NPC, N)),
    ("selrep", (NPC, NC * 128)), ("ident", (128, 128)),
]


def _offsets(spec):
    out, off = {}, 0
    for name, shape in spec:
        n = int(np.prod(shape))
        out[name] = (off, shape)
        off += n
    return out, off

OFF32, LEN32 = _offsets(SPEC32)
OFF16, LEN16 = _offsets(SPEC16)

SPECB32 = [
    ("adjm2", (N2, N2)), ("e3_2", (N2, N2)), ("Wh2T", (OUT, N2)),
    ("Wh2nat", (N2, OUT)), ("a12o", (OUT, 2)), ("wp2ab", (OUT, 2)),
    ("bp2", (1, 1)), ("Wg3", (OUT, 1)), ("bg3", (1, 1)),
    ("fcWr", (2, LH, NCLS)), ("fcb", (1, NCLS)), ("Pall", (OUT, NC * NH)),
    ("Zall", (1, NC)), ("hs0", (HID, 1)), ("identB", (128, 128)),
]
SPECB16 = [("W0b", (2, 18, 128, 4 * LH)), ("W1b", (2, 3, 128, 4 * LH))]
OFFB32, LENB32 = _offsets(SPECB32)
OFFB16, LENB16 = _offsets(SPECB16)



def _ap(t, offset, dims):
    return bass.AP(tensor=t, offset=offset, ap=[list(d) for d in dims])


# ---------------------------------------------------------------- dispatch A
def build_A():
    nc = bacc.Bacc("TRN2", target_bir_lowering=False, debug=False, num_devices=NC)

    def inp(name, shape, dt=F32):
        return nc.dram_tensor(name, shape, dt, kind="ExternalInput").ap()

    def outp(name, shape, dt=F32):
        return nc.dram_tensor(name, shape, dt, kind="ExternalOutput").ap()

    blob16 = inp("blobA", [LEN16 + 2 * LEN32], BF16)
    blobf32 = blob16[:].bitcast(F32)

    def b32(name, head=None):
        off, shape = OFF32[name]
        if head is not None:
            per = int(np.prod(shape[1:]))
            off, shape = off + head * per, shape[1:]
        rows, cols = (shape[0], int(np.prod(shape[1:]))) if len(shape) > 1 else (1, shape[0])
        return _ap(blobf32.tensor, LEN16 // 2 + off, [[cols, rows], [1, cols]])

    def b16(name, head=None):
        off, shape = OFF16[name]
        if head is not None:
            per = int(np.prod(shape[1:]))
            off, shape = off + head * per, shape[1:]
        rows, cols = (shape[0], int(np.prod(shape[1:]))) if len(shape) > 1 else (1, shape[0])
        return _ap(blob16.tensor, off, [[cols, rows], [1, cols]])

    o_all = outp("o_all", [128, 58])

    with tile.TileContext(nc) as tc, ExitStack() as ctx:
        sb = ctx.enter_context(tc.tile_pool(name="sb", bufs=1))
        sb2 = ctx.enter_context(tc.tile_pool(name="sb2", bufs=2))
        sb3 = ctx.enter_context(tc.tile_pool(name="sb3", bufs=3))
        psa = ctx.enter_context(tc.tile_pool(name="psa", bufs=1, space="PSUM"))
        psb = ctx.enter_context(tc.tile_pool(name="psb", bufs=2, space="PSUM"))
        dram = ctx.enter_context(tc.tile_pool(name="dram", bufs=1, space="DRAM"))

        def load(apx, shape, dt=F32, pool=sb, tag=None):
            t = pool.tile(shape, dt, tag=tag)
            nc.sync.dma_start(t[:], apx)
            return t

        def load16(name, shape, tag):
            t = sb.tile(shape, F32, tag=tag)
            nc.gpsimd.dma_start(t[:], b16(name))
            return t

        featT_s = load(b32("featT"), [HID, N], tag="featT")
        featTm_s = load(b32("featTm"), [HID, NPC], tag="featTm")
        Wsn_s = load(b32("W_sn"), [HID, HID], tag="Wsn")
        asn_s = load(b32("a_sn"), [HID, 1], tag="asn")
        Wg1_s = load(b32("Wg1"), [HID, 1], tag="Wg1")
        bg1_s = load(b32("bg1"), [1, 1], tag="bg1")
        ident_s = load16("ident", [128, 128], tag="ident")
        a3t_s = load(b32("a3t128"), [HID, 128], tag="a3t")
        XP_s = load16("XP", [HID, NPC * S], tag="XP")
        gidxf_s = load(b32("gidxbits"), [128, 128], tag="gidx")
        gidx_s = None
        adjm_s = load16("adjmine", [NPC, N], tag="adjm")
        selh2_s = load(b32("selh2"), [NH, 128], tag="selh2")
        eaT_s = load16("eaT", [HID, EPC], tag="eaT")
        selrep_s = load16("selrep", [NPC, NC * 128], tag="selrep")
        a3oT_s = load(b32("a3oT"), [OUT, NH], tag="a3oT")
        bp1_s = load(b32("bp1"), [1, 1], tag="bp1")
        bg2_s = load(b32("bg2"), [1, 1], tag="bg2")

        ones1_128 = sb.tile([1, 128], F32, tag="ones1")
        nc.gpsimd.memset(ones1_128[:], 1.0)
        ones128 = sb.tile([128, 1], F32, tag="ones128")
        nc.gpsimd.memset(ones128[:], 1.0)

        def elu_inplace(src_ps, dst_sb, shape, pool=sb2, tagp="elu"):
            """dst = elu(src) where src is PSUM [p,f]; dst SBUF."""
            p, f = shape
            ex = pool.tile([p, f], F32, tag=tagp + "_ex")
            nc.scalar.activation(ex[:], src_ps, AF.Exp)
            rl = pool.tile([p, f], F32, tag=tagp + "_rl")
            nc.scalar.activation(rl[:], src_ps, AF.Relu)
            # dst = (min(ex,1) + rl) - 1
            nc.vector.scalar_tensor_tensor(dst_sb, ex[:], 1.0, rl[:],
                                           op0=ALU.min, op1=ALU.add)
            nc.vector.tensor_scalar(dst_sb, dst_sb, 1.0, None, op0=ALU.subtract)

        # ---------------- front: h = elu(sigmoid(lrelu(Wh0@a))*Wh0)
        def front(ft, width, tag):
            wh0_ps = psb.tile([HID, width], F32, tag="mm")
            nc.tensor.matmul(wh0_ps[:], Wsn_s[:], ft, start=True, stop=True)
            wh0 = sb.tile([HID, width], F32, tag="wh0_" + tag)
            nc.scalar.copy(wh0[:], wh0_ps[:])
            ga_ps = psb.tile([1, width], F32, tag="mm")
            nc.tensor.matmul(ga_ps[:], asn_s[:], wh0[:], start=True, stop=True)
            gl = sb.tile([1, width], F32, tag="gl_" + tag)
            nc.scalar.activation(gl[:], ga_ps[:], AF.Lrelu, alpha=0.2)
            gs = sb.tile([1, width], F32, tag="gs_" + tag)
            nc.scalar.activation(gs[:], gl[:], AF.Sigmoid)
            grep_ps = psb.tile([HID, width], F32, tag="mm")
            nc.tensor.matmul(grep_ps[:], ones1_128[:, :HID], gs[:], start=True, stop=True)
            hpre = sb.tile([HID, width], F32, tag="hpre_" + tag)
            nc.vector.tensor_tensor(hpre[:], wh0[:], grep_ps[:], ALU.mult)
            ht = sb.tile([HID, width], F32, tag="ht_" + tag)
            elu_inplace(hpre[:], ht[:], [HID, width], tagp="eluf_" + tag)
            return ht

        hT = front(featT_s[:], N, "full")          # [64, 512]
        hTm = front(featTm_s[:], NPC, "mine")      # [64, 64]

        # ---------------- gpool1 -> hs0
        g1_ps = psb.tile([1, N], F32, tag="mm")
        nc.tensor.matmul(g1_ps[:], Wg1_s[:], hT[:], start=True, stop=True)
        g1s = sb.tile([1, N], F32, tag="g1s")
        nc.scalar.activation(g1s[:], g1_ps[:], AF.Sigmoid, bias=bg1_s[:])
        nmax1 = sb.tile([1, 1], F32, tag="nmax1")
        nc.vector.tensor_reduce(nmax1[:], g1s[:], AX.X, ALU.max, negate=True)
        w1 = sb.tile([1, N], F32, tag="w1")
        z1 = sb.tile([1, 1], F32, tag="z1")
        nc.scalar.activation(w1[:], g1s[:], AF.Exp, bias=nmax1[:], accum_out=z1[:])
        iz1 = sb.tile([1, 1], F32, tag="iz1")
        nc.vector.reciprocal(iz1[:], z1[:])
        nc.vector.tensor_scalar(w1[:], w1[:], iz1[:], None, op0=ALU.mult)
        w1rep_ps = psb.tile([HID, N], F32, tag="mm")
        nc.tensor.matmul(w1rep_ps[:], ones1_128[:, :HID], w1[:], start=True, stop=True)
        hw = sb.tile([HID, N], F32, tag="hw")
        nc.vector.tensor_tensor(hw[:], hT[:], w1rep_ps[:], ALU.mult)
        hs0 = sb.tile([HID, 1], F32, tag="hs0")
        nc.vector.tensor_reduce(hs0[:], hw[:], AX.X, ALU.add)
        nc.sync.dma_start(o_all[0:HID, 48:49], hs0[:])

        # ---------------- v12 = WgatT[h] @ a12[h]  -> vall [64, 32]
        vall = sb.tile([HID, 2 * NH], F32, tag="vall")
        for h in range(NH):
            wg0_s = sb2.tile([HID, OUT], F32, tag="wgT0")
            nc.gpsimd.dma_start(wg0_s[:], b16('Wgat', h))
            wgT_ps = psb.tile([OUT, HID], F32, tag="mm")
            nc.tensor.transpose(wgT_ps[:], wg0_s[:], ident_s[0:HID, 0:HID])
            wgT_s = sb2.tile([OUT, HID], F32, tag="wgT")
            nc.vector.tensor_copy(wgT_s[:], wgT_ps[:])
            a12_s = sb2.tile([OUT, 2], F32, tag="a12s")
            nc.sync.dma_start(a12_s[:], b32('a12', h))
            v_ps = psb.tile([HID, 2], F32, tag="mm")
            nc.tensor.matmul(v_ps[:], wgT_s[:], a12_s[:], start=True, stop=True)
            nc.vector.tensor_copy(vall[:, 2 * h:2 * h + 2], v_ps[:])

        # s1mine [16, 64] / s2all [16, 512]
        v1_ap = _ap(vall[:].tensor, 0, [[2 * NH, HID], [2, NH]])
        v2_ap = _ap(vall[:].tensor, 1, [[2 * NH, HID], [2, NH]])
        s1m_ps = psb.tile([NH, NPC], F32, tag="mm")
        nc.tensor.matmul(s1m_ps[:], v1_ap, hTm[:], start=True, stop=True)
        s1m = sb.tile([NH, NPC], F32, tag="s1m")
        nc.vector.tensor_copy(s1m[:], s1m_ps[:])
        s2a_ps = psb.tile([NH, N], F32, tag="mm")
        nc.tensor.matmul(s2a_ps[:], v2_ap, hT[:], start=True, stop=True)
        s2a = sb.tile([NH, N], F32, tag="s2a")
        nc.vector.tensor_copy(s2a[:], s2a_ps[:])
        # s2rep [128, 512]: row p -> s2a[p%16]
        s2rep_ps = psa.tile([128, N], F32, tag="s2rep")
        nc.tensor.matmul(s2rep_ps[:], selh2_s[:], s2a[:], start=True, stop=True)
        s2rep = sb.tile([128, N], F32, tag="s2repsb")
        nc.vector.tensor_copy(s2rep[:], s2rep_ps[:])

        # s1col [128, 8] via DRAM bounce: scratch [16, 64]
        scr = dram.tile([NH, NPC], F32, tag="scr")
        nc.sync.dma_start(scr[:], s1m[:])
        s1col = sb.tile([128, NC], F32, tag="s1col")
        with nc.allow_non_contiguous_dma(reason="s1col 4B gather"):
            for i in range(8):
                src_ap = _ap(scr[:].tensor, i, [[NPC, NH], [8, 8]])
                nc.sync.dma_start(s1col[16 * i:16 * (i + 1), :], src_ap)

        # ---------------- sc = a3-scores on slot grid, replicated rows
        sc_sb = sb.tile([128, NPC * S + 1], F32, tag="scsb")
        for q in range(8):
            scq_ps = psb.tile([128, 512], F32, tag="mm")
            nc.tensor.matmul(scq_ps[:], a3t_s[:], XP_s[:, 512 * q:512 * (q + 1)],
                             start=True, stop=True)
            nc.vector.tensor_copy(sc_sb[:, 512 * q:512 * (q + 1)], scq_ps[:])
        nc.gpsimd.memset(sc_sb[:, NPC * S:NPC * S + 1], 0.0)

        # ---------------- F stage: es over my 1024 edges
        esA_ps = psa.tile([1, 512], F32, tag="accA")
        esB_ps = psa.tile([1, 512], F32, tag="accB")
        sumo_ps = psa.tile([1, 1], F32, tag="accC")
        es_ps = [esA_ps, esB_ps]
        for h in range(NH):
            weg_s = sb2.tile([HID, OUT], F32, tag="weg")
            nc.gpsimd.dma_start(weg_s[:], b16('Wegat', h))
            st, sp = (h == 0), (h == NH - 1)
            for half in range(2):
                T_ps = psb.tile([128, 512], F32, tag="mm")
                nc.tensor.matmul(T_ps[:], weg_s[:], eaT_s[:, 512 * half:512 * (half + 1)],
                                 start=True, stop=True)
                ex = sb2.tile([128, 512], F32, tag="Fex")
                nc.scalar.activation(ex[:], T_ps[:], AF.Exp)
                rl = sb2.tile([128, 512], F32, tag="Frl")
                nc.scalar.activation(rl[:], T_ps[:], AF.Relu)
                eluP = sb2.tile([128, 512], F32, tag="eluP")
                nc.vector.scalar_tensor_tensor(eluP[:], ex[:], 1.0, rl[:],
                                               op0=ALU.min, op1=ALU.add)
                nc.tensor.matmul(es_ps[half][:], a3oT_s[:, h:h + 1], eluP[:],
                                 start=st, stop=sp)
            nc.tensor.matmul(sumo_ps[:], a3oT_s[:, h:h + 1], ones128[:], start=st, stop=sp)
        sumo = sb.tile([1, 1], F32, tag="sumosb")
        nc.vector.tensor_copy(sumo[:], sumo_ps[:])
        es_sb = sb.tile([1, EPC], F32, tag="essb")
        nc.vector.tensor_scalar(es_sb[:, :512], esA_ps[:], sumo[:], None, op0=ALU.subtract)
        nc.vector.tensor_scalar(es_sb[:, 512:], esB_ps[:], sumo[:], None, op0=ALU.subtract)
        dst_es = _ap(o_all.tensor, 50, [[58, 128], [1, 8]])
        nc.sync.dma_start(dst_es, es_sb[:])

        # ---------------- e-stage: 8 tiles [128 (i*16+h), 512]
        att_tiles = []
        for t in range(8):
            e3g = sb2.tile([128, N], F32, tag="e3g")
            nc.gpsimd.ap_gather(e3g[:], sc_sb[:], gidxf_s[:].bitcast(I16)[:, 32 * t:32 * (t + 1)],
                                channels=128, num_elems=NPC * S + 1, d=1, num_idxs=N)
            e1 = sb2.tile([128, N], F32, tag="e1")
            nc.vector.tensor_tensor(e1[:], e3g[:], s2rep[:], ALU.add)
            lr = sb2.tile([128, N], F32, tag="lr")
            nc.scalar.activation(lr[:], e1[:], AF.Lrelu, bias=s1col[:, t:t + 1], alpha=0.2)
            adjrep_ps = psb.tile([128, N], F32, tag="mm")
            nc.tensor.matmul(adjrep_ps[:], selrep_s[:, 128 * t:128 * (t + 1)], adjm_s[:], start=True, stop=True)
            m1 = sb2.tile([128, N], F32, tag="m1")
            nc.vector.scalar_tensor_tensor(m1[:], lr[:], 1e9, adjrep_ps[:],
                                           op0=ALU.add, op1=ALU.mult)
            nmax = sb2.tile([128, 1], F32, tag="nmax")
            nc.vector.tensor_reduce(nmax[:], m1[:], AX.X, ALU.max, negate=True)
            pt = sb2.tile([128, N], F32, tag="pt")
            zt = sb2.tile([128, 1], F32, tag="zt")
            nc.scalar.activation(pt[:], m1[:], AF.Exp, bias=nmax[:], accum_out=zt[:])
            izt = sb2.tile([128, 1], F32, tag="izt")
            nc.vector.reciprocal(izt[:], zt[:])
            att = sb.tile([128, N], F32, tag=f"att{t}")
            nc.vector.tensor_scalar(att[:], pt[:], izt[:], None, op0=ALU.mult)
            att_tiles.append(att)

        # transposes -> attT[jc] [128, 1024] cols = t*128 + (i*16+h)
        attT = []
        for jc in range(4):
            bigt = sb.tile([128, 1024], F32, tag=f"attT{jc}")
            attT.append(bigt)
        for t in range(8):
            for jc in range(4):
                tp_ps = psb.tile([128, 128], F32, tag="mm")
                nc.tensor.transpose(tp_ps[:], att_tiles[t][:, 128 * jc:128 * (jc + 1)],
                                    ident_s[:])
                nc.vector.tensor_copy(attT[jc][:, 128 * t:128 * (t + 1)], tp_ps[:])

        # AV per head + elu
        hGelu = []
        for h in range(NH):
            wg_s = sb2.tile([HID, OUT], F32, tag="wgnat")
            nc.gpsimd.dma_start(wg_s[:], b16('Wgat', h))
            hg_ps = psa.tile([OUT, NPC], F32, tag="hg")
            for jc in range(4):
                wh_ps = psb.tile([128, OUT], F32, tag="mm")
                nc.tensor.matmul(wh_ps[:], hT[:, 128 * jc:128 * (jc + 1)], wg_s[:],
                                 start=True, stop=True)
                wh_sb = sb2.tile([128, OUT], F32, tag="whsb")
                nc.vector.tensor_copy(wh_sb[:], wh_ps[:])
                rhs = _ap(attT[jc][:].tensor, h, [[1024, 128], [128, 8], [16, 8]])
                nc.tensor.matmul(hg_ps[:], wh_sb[:], rhs, start=(jc == 0), stop=(jc == 3))
            hg = sb.tile([OUT, NPC], F32, tag=f"hgelu{h}")
            elu_inplace(hg_ps[:], hg[:], [OUT, NPC], tagp="elug")
            hGelu.append(hg)

        # pair gates
        dpa_ps = psa.tile([1, NPC], F32, tag="accA")
        dpb_ps = psa.tile([1, NPC], F32, tag="accB")
        for h in range(NH):
            wp_s = sb2.tile([OUT, 2], F32, tag="wps")
            nc.sync.dma_start(wp_s[:], b32('wp1ab', h))
            st, sp = (h == 0), (h == NH - 1)
            nc.tensor.matmul(dpa_ps[:], wp_s[:, 0:1], hGelu[h][:], start=st, stop=sp)
            nc.tensor.matmul(dpb_ps[:], wp_s[:, 1:2], hGelu[h][:], start=st, stop=sp)
        dk = sb.tile([1, NPC // 2], F32, tag="dk")
        dasb = sb.tile([1, NPC], F32, tag="dasb")
        nc.vector.tensor_copy(dasb[:], dpa_ps[:])
        a_ap = _ap(dasb[:].tensor, 0, [[NPC, 1], [2, NPC // 2]])
        b_ap = _ap(dpb_ps[:].tensor, 1, [[NPC, 1], [2, NPC // 2]])
        nc.vector.tensor_tensor(dk[:], a_ap, b_ap, ALU.add)
        sgate = sb.tile([1, NPC // 2], F32, tag="sgate")
        nc.scalar.activation(sgate[:], dk[:], AF.Sigmoid, bias=bp1_s[:])
        srep_ps = psa.tile([128, NPC // 2], F32, tag="accC")
        nc.tensor.matmul(srep_ps[:], ones1_128[:], sgate[:], start=True, stop=True)

        h1T = []
        for h in range(NH):
            ev_ap = _ap(hGelu[h][:].tensor, 0, [[NPC, OUT], [2, NPC // 2]])
            od_ap = _ap(hGelu[h][:].tensor, 1, [[NPC, OUT], [2, NPC // 2]])
            t1 = sb2.tile([OUT, NPC // 2], F32, tag="pairsum")
            nc.vector.tensor_tensor(t1[:], ev_ap, od_ap, ALU.add)
            h1 = sb.tile([OUT, NPC // 2], F32, tag=f"h1T{h}")
            nc.vector.tensor_tensor(h1[:], t1[:], srep_ps[:], ALU.mult)
            h1T.append(h1)

        # g2 / u / Z / P
        g2_ps = psa.tile([1, NPC // 2], F32, tag="accA")
        for h in range(NH):
            wg2_s = sb2.tile([OUT, 1], F32, tag="wg2s")
            nc.sync.dma_start(wg2_s[:], b32('Wg2r', h))
            nc.tensor.matmul(g2_ps[:], wg2_s[:], h1T[h][:],
                             start=(h == 0), stop=(h == NH - 1))
        sg2 = sb.tile([1, NPC // 2], F32, tag="sg2")
        nc.scalar.activation(sg2[:], g2_ps[:], AF.Sigmoid, bias=bg2_s[:])
        u = sb.tile([1, NPC // 2], F32, tag="u")
        nc.scalar.activation(u[:], sg2[:], AF.Exp)
        Zc = sb.tile([1, 1], F32, tag="Zc")
        nc.vector.tensor_reduce(Zc[:], u[:], AX.X, ALU.add)
        nc.sync.dma_start(o_all[0:1, 49:50], Zc[:])
        urep_ps = psa.tile([128, NPC // 2], F32, tag="accB")
        nc.tensor.matmul(urep_ps[:], ones1_128[:], u[:], start=True, stop=True)
        Pout = sb.tile([OUT, NH], F32, tag="Pout")
        for h in range(NH):
            pm = sb2.tile([OUT, NPC // 2], F32, tag="pm")
            nc.vector.tensor_tensor(pm[:], h1T[h][:], urep_ps[:OUT, :], ALU.mult)
            nc.vector.tensor_reduce(Pout[:, h:h + 1], pm[:], AX.X, ALU.add)
        nc.sync.dma_start(o_all[:, 0:16], Pout[:])

        # Wh2T rows
        wh2_ps = psa.tile([OUT, NPC // 2], F32, tag="accC")
        for h in range(NH):
            wo_s = sb2.tile([OUT, OUT], F32, tag="wos")
            nc.gpsimd.dma_start(wo_s[:], b16('Wor', h))
            nc.tensor.matmul(wh2_ps[:], wo_s[:], h1T[h][:],
                             start=(h == 0), stop=(h == NH - 1))
        wh2 = sb.tile([OUT, NPC // 2], F32, tag="wh2sb")
        nc.vector.tensor_copy(wh2[:], wh2_ps[:])
        nc.sync.dma_start(o_all[:, 16:48], wh2[:])

    nc.compile()
    return nc


# ---------------------------------------------------------------- dispatch B
def build_B():
    nc = bacc.Bacc("TRN2", target_bir_lowering=False, debug=False, num_devices=1)

    def inp(name, shape, dt=F32):
        return nc.dram_tensor(name, shape, dt, kind="ExternalInput").ap()

    blob16 = inp("blobB", [LENB16 + 2 * LENB32], BF16)
    blobf32 = blob16[:].bitcast(F32)
    BOFF = LENB16 // 2

    def b32(name):
        off, shape = OFFB32[name]
        rows, cols = (shape[0], int(np.prod(shape[1:]))) if len(shape) > 1 else (1, shape[0])
        return _ap(blobf32.tensor, BOFF + off, [[cols, rows], [1, cols]])

    def b16w(d, k, r0, rn, c0, cn, which):
        off, shape = OFFB16[which]
        base = off + ((d * shape[1] + k) * 128) * (4 * LH)
        return _ap(blob16.tensor, base + r0 * 4 * LH + c0, [[4 * LH, rn], [1, cn]])
    o_prob = nc.dram_tensor("o_prob", [1, NCLS], F32, kind="ExternalOutput").ap()

    with tile.TileContext(nc) as tc, ExitStack() as ctx:
        sb = ctx.enter_context(tc.tile_pool(name="sb", bufs=1))
        sb2 = ctx.enter_context(tc.tile_pool(name="sb2", bufs=2))
        psa = ctx.enter_context(tc.tile_pool(name="psa", bufs=1, space="PSUM"))
        psb = ctx.enter_context(tc.tile_pool(name="psb", bufs=2, space="PSUM"))

        def load(apx, shape, dt=F32, pool=sb, tag=None):
            t = pool.tile(shape, dt, tag=tag)
            nc.sync.dma_start(t[:], apx)
            return t

        ident_s = load(b32("identB"), [128, 128], tag="ident")
        ones1 = sb.tile([1, 128], F32, tag="ones1")
        nc.gpsimd.memset(ones1[:], 1.0)
        Pall_s = load(b32("Pall"), [OUT, NC * NH], tag="Pall")
        Zall_s = load(b32("Zall"), [1, NC], tag="Zall")
        hs0_s = load(b32("hs0"), [HID, 1], tag="hs0")
        Wh2T_s = load(b32("Wh2T"), [OUT, N2], tag="Wh2T")
        a12o_s = load(b32("a12o"), [OUT, 2], tag="a12o")
        wp2_s = load(b32("wp2ab"), [OUT, 2], tag="wp2")
        bp2_s = load(b32("bp2"), [1, 1], tag="bp2")
        Wg3_s = load(b32("Wg3"), [OUT, 1], tag="Wg3")
        bg3_s = load(b32("bg3"), [1, 1], tag="bg3")
        fcb_s = load(b32("fcb"), [1, NCLS], tag="fcb")

        # hs1 columns [128, 16] = sum_c Pall[:, c*16+h] / Z
        hs1c = sb.tile([OUT, NH], F32, tag="hs1c")
        src = _ap(Pall_s[:].tensor, 0, [[NC * NH, OUT], [1, NH], [NH, NC]])
        nc.vector.tensor_reduce(hs1c[:], src, AX.X, ALU.add)
        Zt = sb.tile([1, 1], F32, tag="Zt")
        nc.vector.tensor_reduce(Zt[:], Zall_s[:], AX.X, ALU.add)
        iZ = sb.tile([1, 1], F32, tag="iZ")
        nc.vector.reciprocal(iZ[:], Zt[:])
        izrep_ps = psa.tile([128, 1], F32, tag="r1")
        nc.tensor.matmul(izrep_ps[:], ones1[:], iZ[:], start=True, stop=True)
        izcol = sb.tile([128, 1], F32, tag="izcol")
        nc.vector.tensor_copy(izcol[:], izrep_ps[:])
        nc.vector.tensor_scalar(hs1c[:], hs1c[:], izcol[:OUT, :], None, op0=ALU.mult)

        # att2 scores
        s1o_ps = psa.tile([1, N2], F32, tag="r2")
        nc.tensor.matmul(s1o_ps[:], a12o_s[:, 0:1], Wh2T_s[:], start=True, stop=True)
        s2o_ps = psa.tile([1, N2], F32, tag="r3")
        nc.tensor.matmul(s2o_ps[:], a12o_s[:, 1:2], Wh2T_s[:], start=True, stop=True)
        s1o = sb.tile([1, N2], F32, tag="s1osb")
        nc.vector.tensor_copy(s1o[:], s1o_ps[:])
        s2o = sb.tile([1, N2], F32, tag="s2osb")
        nc.vector.tensor_copy(s2o[:], s2o_ps[:])
        s2orep_ps = psa.tile([128, N2], F32, tag="r4")
        nc.tensor.matmul(s2orep_ps[:], ones1[:], s2o[:], start=True, stop=True)

        att2 = []
        for t2 in range(2):
            s1c_ps = psb.tile([128, 1], F32, tag="mmB")
            nc.tensor.transpose(s1c_ps[:], s1o[:, 128 * t2:128 * (t2 + 1)], ident_s[0:1, 0:1])
            s1c = sb2.tile([128, 1], F32, tag="s1c")
            nc.vector.tensor_copy(s1c[:], s1c_ps[:])
            e3t = sb2.tile([128, N2], F32, tag="e3t")
            nc.sync.dma_start(e3t[:], _ap(blobf32.tensor, BOFF + OFFB32['e3_2'][0] + 128 * t2 * N2, [[N2, 128], [1, N2]]))
            adt = sb2.tile([128, N2], F32, tag="adt")
            nc.sync.dma_start(adt[:], _ap(blobf32.tensor, BOFF + OFFB32['adjm2'][0] + 128 * t2 * N2, [[N2, 128], [1, N2]]))
            e1 = sb2.tile([128, N2], F32, tag="e1b")
            nc.vector.tensor_tensor(e1[:], e3t[:], s2orep_ps[:], ALU.add)
            lr = sb2.tile([128, N2], F32, tag="lrb")
            nc.scalar.activation(lr[:], e1[:], AF.Lrelu, bias=s1c[:], alpha=0.2)
            m1 = sb2.tile([128, N2], F32, tag="m1b")
            nc.vector.scalar_tensor_tensor(m1[:], lr[:], 1e9, adt[:],
                                           op0=ALU.add, op1=ALU.mult)
            nmax = sb2.tile([128, 1], F32, tag="nmaxb")
            nc.vector.tensor_reduce(nmax[:], m1[:], AX.X, ALU.max, negate=True)
            pt = sb2.tile([128, N2], F32, tag="ptb")
            zt = sb2.tile([128, 1], F32, tag="ztb")
            nc.scalar.activation(pt[:], m1[:], AF.Exp, bias=nmax[:], accum_out=zt[:])
            izt = sb2.tile([128, 1], F32, tag="iztb")
            nc.vector.reciprocal(izt[:], zt[:])
            at = sb.tile([128, N2], F32, tag=f"att2_{t2}")
            nc.vector.tensor_scalar(at[:], pt[:], izt[:], None, op0=ALU.mult)
            att2.append(at)

        # att2T + h2T
        attT2 = []
        for lc in range(2):
            big = sb.tile([128, N2], F32, tag=f"attT2_{lc}")
            attT2.append(big)
        for t2 in range(2):
            for lc in range(2):
                tp_ps = psb.tile([128, 128], F32, tag="mmB")
                nc.tensor.transpose(tp_ps[:], att2[t2][:, 128 * lc:128 * (lc + 1)],
                                    ident_s[:])
                nc.vector.tensor_copy(attT2[lc][:, 128 * t2:128 * (t2 + 1)], tp_ps[:])
        h2_ps = psa.tile([OUT, N2], F32, tag="r5")
        for lc in range(2):
            w2n_s = sb2.tile([128, OUT], F32, tag="w2n")
            nc.sync.dma_start(w2n_s[:], _ap(blobf32.tensor, BOFF + OFFB32['Wh2nat'][0] + 128 * lc * OUT, [[OUT, 128], [1, OUT]]))
            nc.tensor.matmul(h2_ps[:], w2n_s[:], attT2[lc][:],
                             start=(lc == 0), stop=(lc == 1))
        h2T = sb.tile([OUT, N2], F32, tag="h2T")
        nc.vector.tensor_copy(h2T[:], h2_ps[:])

        # edge pool 2
        dpa_ps = psa.tile([1, N2], F32, tag="r1")
        nc.tensor.matmul(dpa_ps[:], wp2_s[:, 0:1], h2T[:], start=True, stop=True)
        dpb_ps = psa.tile([1, N2], F32, tag="r2")
        nc.tensor.matmul(dpb_ps[:], wp2_s[:, 1:2], h2T[:], start=True, stop=True)
        dk2 = sb.tile([1, N3], F32, tag="dk2")
        dasb2 = sb.tile([1, N2], F32, tag="dasb2")
        nc.vector.tensor_copy(dasb2[:], dpa_ps[:])
        a_ap = _ap(dasb2[:].tensor, 0, [[N2, 1], [2, N3]])
        b_ap = _ap(dpb_ps[:].tensor, 1, [[N2, 1], [2, N3]])
        nc.vector.tensor_tensor(dk2[:], a_ap, b_ap, ALU.add)
        s2k = sb.tile([1, N3], F32, tag="s2k")
        nc.scalar.activation(s2k[:], dk2[:], AF.Sigmoid, bias=bp2_s[:])
        srep2_ps = psa.tile([128, N3], F32, tag="r3")
        nc.tensor.matmul(srep2_ps[:], ones1[:], s2k[:], start=True, stop=True)
        ev_ap = _ap(h2T[:].tensor, 0, [[N2, OUT], [2, N3]])
        od_ap = _ap(h2T[:].tensor, 1, [[N2, OUT], [2, N3]])
        t12 = sb.tile([OUT, N3], F32, tag="t12")
        nc.vector.tensor_tensor(t12[:], ev_ap, od_ap, ALU.add)
        h3T = sb.tile([OUT, N3], F32, tag="h3T")
        nc.vector.tensor_tensor(h3T[:], t12[:], srep2_ps[:OUT, :], ALU.mult)

        # gpool3 -> hs2 [128, 1]
        g3_ps = psa.tile([1, N3], F32, tag="r1")
        nc.tensor.matmul(g3_ps[:], Wg3_s[:], h3T[:], start=True, stop=True)
        g3s = sb.tile([1, N3], F32, tag="g3s")
        nc.scalar.activation(g3s[:], g3_ps[:], AF.Sigmoid, bias=bg3_s[:])
        nm3 = sb.tile([1, 1], F32, tag="nm3")
        nc.vector.tensor_reduce(nm3[:], g3s[:], AX.X, ALU.max, negate=True)
        w3 = sb.tile([1, N3], F32, tag="w3")
        z3 = sb.tile([1, 1], F32, tag="z3")
        nc.scalar.activation(w3[:], g3s[:], AF.Exp, bias=nm3[:], accum_out=z3[:])
        iz3 = sb.tile([1, 1], F32, tag="iz3")
        nc.vector.reciprocal(iz3[:], z3[:])
        nc.vector.tensor_scalar(w3[:], w3[:], iz3[:], None, op0=ALU.mult)
        w3rep_ps = psa.tile([128, N3], F32, tag="r2")
        nc.tensor.matmul(w3rep_ps[:], ones1[:], w3[:], start=True, stop=True)
        hw3 = sb.tile([OUT, N3], F32, tag="hw3")
        nc.vector.tensor_tensor(hw3[:], h3T[:], w3rep_ps[:OUT, :], ALU.mult)
        hs2 = sb.tile([OUT, 1], F32, tag="hs2")
        nc.vector.tensor_reduce(hs2[:], hw3[:], AX.X, ALU.add)

        # x chunks [128, 18] bf16: cols 0-15 hs1c, col16 [hs0; hs2[0:64]], col17 [hs2[64:]; 1]
        xc = sb.tile([128, 18], F32, tag="xc")
        nc.gpsimd.memset(xc[:], 0.0)
        nc.vector.tensor_copy(xc[:OUT, 0:NH], hs1c[:])
        nc.vector.tensor_copy(xc[:HID, 16:17], hs0_s[:])
        nc.sync.dma_start(xc[HID:128, 16:17], hs2[0:HID, :])
        nc.sync.dma_start(xc[0:HID, 17:18], hs2[HID:OUT, :])
        nc.gpsimd.memset(xc[HID:HID + 1, 17:18], 1.0)
        xcb = sb.tile([128, 18], BF16, tag="xcb")
        nc.vector.tensor_copy(xcb[:], xc[:])

        # LSTM layer 0 (M-orientation, skip f-gate m=1)
        h0 = []
        for d in range(2):
            g_ps = psa.tile([128, 4], F32, tag="gacc")
            for m in (0, 2, 3):
                for k in range(18):
                    rows = 65 if k == 17 else 128
                    w_s = sb2.tile([128, 128], BF16, tag="w0s")
                    nc.sync.dma_start(w_s[:rows, :], b16w(d, k, 0, rows, 128 * m, 128, 'W0b'))
                    nc.tensor.matmul(g_ps[:, m:m + 1], w_s[:rows, :], xcb[:rows, k:k + 1],
                                     start=(k == 0), stop=(k == 17))
            si = sb2.tile([128, 1], F32, tag="si")
            nc.scalar.activation(si[:], g_ps[:, 0:1], AF.Sigmoid)
            tg = sb2.tile([128, 1], F32, tag="tg")
            nc.scalar.activation(tg[:], g_ps[:, 2:3], AF.Tanh)
            so = sb2.tile([128, 1], F32, tag="so")
            nc.scalar.activation(so[:], g_ps[:, 3:4], AF.Sigmoid)
            c = sb2.tile([128, 1], F32, tag="c0")
            nc.vector.tensor_tensor(c[:], si[:], tg[:], ALU.mult)
            tc_ = sb2.tile([128, 1], F32, tag="tc0")
            nc.scalar.activation(tc_[:], c[:], AF.Tanh)
            hd = sb.tile([128, 1], F32, tag=f"h0_{d}")
            nc.vector.tensor_tensor(hd[:], so[:], tc_[:], ALU.mult)
            h0.append(hd)
        h0b_ = []
        for d in range(2):
            hb = sb.tile([128, 1], BF16, tag=f"h0b_{d}")
            nc.vector.tensor_copy(hb[:], h0[d][:])
            h0b_.append(hb)
        onesb = sb.tile([1, 1], BF16, tag="onesb")
        nc.gpsimd.memset(onesb[:], 1.0)

        # LSTM layer 1
        h1o = []
        for d in range(2):
            g_ps = psa.tile([128, 4], F32, tag="gacc")
            for m in (0, 2, 3):
                for k in range(3):
                    rows = 1 if k == 2 else 128
                    w_s = sb2.tile([128, 128], BF16, tag="w1s")
                    nc.sync.dma_start(w_s[:rows, :], b16w(d, k, 0, rows, 128 * m, 128, 'W1b'))
                    rhs = onesb[:] if k == 2 else h0b_[k][:]
                    nc.tensor.matmul(g_ps[:, m:m + 1], w_s[:rows, :], rhs,
                                     start=(k == 0), stop=(k == 2))
            si = sb2.tile([128, 1], F32, tag="si1")
            nc.scalar.activation(si[:], g_ps[:, 0:1], AF.Sigmoid)
            tg = sb2.tile([128, 1], F32, tag="tg1")
            nc.scalar.activation(tg[:], g_ps[:, 2:3], AF.Tanh)
            so = sb2.tile([128, 1], F32, tag="so1")
            nc.scalar.activation(so[:], g_ps[:, 3:4], AF.Sigmoid)
            c = sb2.tile([128, 1], F32, tag="c1")
            nc.vector.tensor_tensor(c[:], si[:], tg[:], ALU.mult)
            tc_ = sb2.tile([128, 1], F32, tag="tc1")
            nc.scalar.activation(tc_[:], c[:], AF.Tanh)
            hd = sb.tile([128, 1], F32, tag=f"h1_{d}")
            nc.vector.tensor_tensor(hd[:], so[:], tc_[:], ALU.mult)
            h1o.append(hd)

        # fc + softmax
        lg_ps = psa.tile([1, NCLS], F32, tag="r1")
        fcw0 = sb.tile([LH, NCLS], F32, tag="fcw0")
        nc.sync.dma_start(fcw0[:], _ap(blobf32.tensor, BOFF + OFFB32['fcWr'][0], [[NCLS, LH], [1, NCLS]]))
        fcw1 = sb.tile([LH, NCLS], F32, tag="fcw1")
        nc.sync.dma_start(fcw1[:], _ap(blobf32.tensor, BOFF + OFFB32['fcWr'][0] + LH * NCLS, [[NCLS, LH], [1, NCLS]]))
        nc.tensor.matmul(lg_ps[:], h1o[0][:], fcw0[:], start=True, stop=False)
        nc.tensor.matmul(lg_ps[:], h1o[1][:], fcw1[:], start=False, stop=True)
        lg = sb.tile([1, NCLS], F32, tag="lg")
        nc.vector.tensor_tensor(lg[:], lg_ps[:], fcb_s[:], ALU.add)
        nmf = sb.tile([1, 1], F32, tag="nmf")
        nc.vector.tensor_reduce(nmf[:], lg[:], AX.X, ALU.max, negate=True)
        pf = sb.tile([1, NCLS], F32, tag="pf")
        zf = sb.tile([1, 1], F32, tag="zf")
        nc.scalar.activation(pf[:], lg[:], AF.Exp, bias=nmf[:], accum_out=zf[:])
        izf = sb.tile([1, 1], F32, tag="izf")
        nc.vector.reciprocal(izf[:], zf[:])
        prob = sb.tile([1, NCLS], F32, tag="prob")
        nc.vector.tensor_scalar(prob[:], pf[:], izf[:], None, op0=ALU.mult)
        nc.sync.dma_start(o_prob[:], prob[:])

    nc.compile()
    return nc


# ---------------------------------------------------------------- host prep
def _prep_A(inputs):
    """Build per-core input maps for dispatch A. Pure layout/indexing."""
    f32 = np.float32
    import ml_dtypes
    bf = ml_dtypes.bfloat16
    ei = np.asarray(inputs["edge_index"])
    feats = np.asarray(inputs["features"], f32)
    n2n = np.asarray(inputs["node2node_features"], f32)
    eattr = np.asarray(inputs["edgesAttr"], f32)
    adjacency = np.asarray(inputs["adjacency"], f32)

    src, dst = np.asarray(ei[0], np.int64), np.asarray(ei[1], np.int64)
    pairs = src * N + dst
    uniq = np.unique(pairs)
    us, ud = uniq // N, uniq % N
    order = np.argsort(us, kind="stable")
    us, ud, uniq = us[order], ud[order], uniq[order]
    counts = np.bincount(us, minlength=N)
    assert counts.max() <= S, f"out-degree {counts.max()} > {S}"
    starts = np.zeros(N + 1, np.int64)
    np.cumsum(counts, out=starts[1:])
    slots = np.arange(len(us)) - starts[us]

    featT = np.ascontiguousarray(feats.T)
    eaT = np.ascontiguousarray(eattr.T)
    W_gat = np.asarray(inputs["W_gat"], f32)

    sh32 = {
        "featT": featT,
        "W_sn": np.asarray(inputs["W_sn"], f32),
        "a_sn": np.asarray(inputs["a_sn"], f32).reshape(HID, 1),
        "Wg1": np.asarray(inputs["Wg1"], f32).reshape(HID, 1),
        "bg1": np.asarray(inputs["bg1"], f32).reshape(1, 1),
        "a12": np.stack([np.asarray(inputs["a1_gat"], f32),
                         np.asarray(inputs["a2_gat"], f32)], -1),
        "a3t128": np.tile(np.asarray(inputs["a3_gat"], f32).T, (1, 8)),
        "a3oT": np.asarray(inputs["a3_o"], f32).reshape(NH, OUT).T,
        "wp1ab": np.stack([
            np.asarray(inputs["Wp1"], f32)[:D1, 0].reshape(NH, OUT),
            np.asarray(inputs["Wp1"], f32)[D1:, 0].reshape(NH, OUT)], -1),
        "Wg2r": np.asarray(inputs["Wg2"], f32).reshape(NH, OUT, 1),
        "bp1": np.asarray(inputs["bp1"], f32).reshape(1, 1),
        "bg2": np.asarray(inputs["bg2"], f32).reshape(1, 1),
        "selh2": np.eye(NH, dtype=f32)[:, np.tile(np.arange(NH), 8)].reshape(NH, 128),
    }
    selrep = np.zeros((NPC, NC * 128), f32)
    for t in range(8):
        for p in range(128):
            selrep[8 * t + p // 16, 128 * t + p] = 1.0
    sh16 = {
        "Wgat": W_gat,
        "Wegat": np.asarray(inputs["We_gat"], f32),
        "Wor": np.asarray(inputs["Wo"], f32).reshape(NH, OUT, OUT),
        "selrep": selrep,
        "ident": np.eye(128, dtype=f32),
    }

    in_maps = []
    for c in range(NC):
        lo = c * NPC
        d32 = dict(sh32)
        d16 = dict(sh16)
        d32["featTm"] = featT[:, lo:lo + NPC]
        mask = (us >= lo) & (us < lo + NPC)
        cs, cd, csl = us[mask] - lo, ud[mask], slots[mask]
        XP = np.zeros((NPC * S, HID), f32)
        XP[cs * S + csl] = n2n[uniq[mask]]
        d16["XP"] = XP.T
        ptr = np.full((NPC, N), NPC * S, np.int64)
        ptr[cs, cd] = cs * S + csl
        g = np.zeros((128, 256), np.int16)
        for t in range(8):
            for gg in range(8):
                row = ptr[8 * t + gg]
                g[16 * gg:16 * gg + 16, 32 * t:32 * t + 32] = \
                    row.reshape(32, 16).T.astype(np.int16)
        d32["gidxbits"] = g.view(f32)
        d16["adjmine"] = adjacency[lo:lo + NPC]
        d16["eaT"] = eaT[:, c * EPC:(c + 1) * EPC]
        blob = np.empty(LEN16 + 2 * LEN32, bf)
        for name, shape in SPEC16:
            off, _ = OFF16[name]
            blob[off:off + int(np.prod(shape))] = np.ascontiguousarray(d16[name], f32).reshape(-1).astype(bf)
        f32v = blob[LEN16:].view(f32)
        for name, shape in SPEC32:
            off, _ = OFF32[name]
            f32v[off:off + int(np.prod(shape))] = np.ascontiguousarray(d32[name], f32).reshape(-1)
        in_maps.append({"blobA": blob})
    return in_maps, (src, dst)


def _prep_B(inputs, resA, ei_sd):
    f32 = np.float32
    src, dst = ei_sd
    unp = []
    for c in range(NC):
        o = resA[c]["o_all"]
        unp.append({"o_P": o[:, 0:16], "o_Wh2T": o[:, 16:48],
                    "o_hs0": o[0:HID, 48:49], "o_Z": o[0:1, 49:50],
                    "o_es": o[:, 50:58].reshape(-1)})
    resA = unp
    es = np.concatenate([resA[c]["o_es"].reshape(-1) for c in range(NC)])
    s2, d2 = src // 2, dst // 2
    adj2 = np.zeros((N2, N2), f32)
    adj2[s2, d2] = 1.0
    e3_2 = np.zeros((N2, N2), f32)
    e3_2[s2, d2] = es  # numpy fancy assignment: last occurrence wins
    Wh2T = np.concatenate([resA[c]["o_Wh2T"] for c in range(NC)], axis=1)
    Pall = np.concatenate([resA[c]["o_P"] for c in range(NC)], axis=1)
    Zall = np.concatenate([resA[c]["o_Z"].reshape(1, 1) for c in range(NC)], axis=1)

    # LSTM weights: my-x order = [hs1(2048), hs0(64), hs2(128), bias(1)]
    perm = np.concatenate([np.arange(64, 2112), np.arange(0, 64), np.arange(2112, 2240)])
    W0 = np.zeros((2, 18, 128, 4 * LH), f32)
    for d in range(2):
        wt = np.asarray(inputs["Wih0"], f32)[d].T[perm]         # [2240, 512]
        wb = np.concatenate([wt, np.asarray(inputs["b0"], f32)[d][None, :]], 0)  # [2241,512]
        for k in range(18):
            rows = wb[128 * k:128 * (k + 1)]
            W0[d, k, :rows.shape[0], :] = rows
    W1 = np.zeros((2, 3, 128, 4 * LH), f32)
    for d in range(2):
        wt = np.asarray(inputs["Wih1"], f32)[d].T               # [256, 512]
        wb = np.concatenate([wt, np.asarray(inputs["b1"], f32)[d][None, :]], 0)
        for k in range(3):
            rows = wb[128 * k:128 * (k + 1)]
            W1[d, k, :rows.shape[0], :] = rows
    import ml_dtypes
    bf = ml_dtypes.bfloat16

    d32 = {
        "adjm2": adj2,
        "e3_2": e3_2,
        "Wh2T": Wh2T,
        "Wh2nat": Wh2T.T,
        "a12o": np.stack([np.asarray(inputs["a1_o"], f32), np.asarray(inputs["a2_o"], f32)], -1),
        "wp2ab": np.stack([np.asarray(inputs["Wp2"], f32)[:OUT, 0],
                           np.asarray(inputs["Wp2"], f32)[OUT:, 0]], -1),
        "bp2": np.asarray(inputs["bp2"], f32).reshape(1, 1),
        "Wg3": np.asarray(inputs["Wg3"], f32).reshape(OUT, 1),
        "bg3": np.asarray(inputs["bg3"], f32).reshape(1, 1),
        "fcWr": np.stack([np.asarray(inputs["fc_W"], f32)[:LH],
                          np.asarray(inputs["fc_W"], f32)[LH:]]),
        "fcb": np.asarray(inputs["fc_b"], f32).reshape(1, NCLS),
        "Pall": Pall,
        "Zall": Zall,
        "hs0": resA[0]["o_hs0"].reshape(HID, 1),
        "identB": np.eye(128, dtype=f32),
    }
    blob = np.empty(LENB16 + 2 * LENB32, bf)
    o0, _ = OFFB16["W0b"]
    blob[o0:o0 + W0.size] = W0.reshape(-1).astype(bf)
    o1, _ = OFFB16["W1b"]
    blob[o1:o1 + W1.size] = W1.reshape(-1).astype(bf)
    f32v = blob[LENB16:].view(f32)
    for name, shape in SPECB32:
        off, _ = OFFB32[name]
        f32v[off:off + int(np.prod(shape))] = np.ascontiguousarray(d32[name], f32).reshape(-1)
    return {"blobB": blob}


# ------------------------------------------------------- cached SPMD runner
class _CachedRunner:
    """Like bass2jax.run_bass_via_pjrt but with the jitted callable built once."""

    def __init__(self, nc, n_cores):
        import jax
        from jax.sharding import Mesh, PartitionSpec
        from jax.experimental.shard_map import shard_map
        from concourse import bass2jax
        bass2jax.install_neuronx_cc_hook()
        self.n_cores = n_cores
        partition_name = nc.partition_id_tensor.name if nc.partition_id_tensor else None
        in_names, out_names, out_avals, zero_outs = [], [], [], []
        for alloc in nc.m.functions[0].allocations:
            if not isinstance(alloc, mybir.MemoryLocationSet):
                continue
            name = alloc.memorylocations[0].name
            if alloc.kind == "ExternalInput":
                if name != partition_name:
                    in_names.append(name)
            elif alloc.kind == "ExternalOutput":
                shape = tuple(alloc.tensor_shape)
                dtype = mybir.dt.np(alloc.dtype)
                out_names.append(name)
                out_avals.append(jax.core.ShapedArray(shape, dtype))
                zero_outs.append(np.zeros(shape, dtype))
        self.in_names, self.out_names = in_names, out_names
        self.out_avals, self.zero_outs = out_avals, zero_outs
        n_params, n_outs = len(in_names), len(out_names)
        all_names = in_names + out_names
        if partition_name is not None:
            all_names = all_names + [partition_name]
        donate = tuple(range(n_params, n_params + n_outs))

        def _body(*args):
            operands = list(args)
            if partition_name is not None:
                operands.append(bass2jax.partition_id_tensor())
            outs = bass2jax._bass_exec_p.bind(
                *operands,
                out_avals=tuple(out_avals),
                in_names=tuple(all_names),
                out_names=tuple(out_names),
                lowering_input_output_aliases=(),
                sim_require_finite=True,
                sim_require_nnan=True,
                nc=nc,
            )
            return tuple(outs)

        self._body = _body
        self._jax = jax
        self._Mesh, self._P, self._shard_map = Mesh, PartitionSpec, shard_map
        self.donate = donate
        self.n_params, self.n_outs = n_params, n_outs
        self.fn = None
        if n_cores == 1:
            self.fn = jax.jit(_body, donate_argnums=donate, keep_unused=True)

    def _build_multi(self, shared_flags):
        jax = self._jax
        devices = jax.devices()[:self.n_cores]
        mesh = self._Mesh(np.asarray(devices), ("core",))
        self.shared_flags = shared_flags
        in_specs = tuple(self._P() if f else self._P("core") for f in shared_flags) \
            + (self._P("core"),) * self.n_outs
        out_specs = (self._P("core"),) * self.n_outs
        self.fn = jax.jit(
            self._shard_map(self._body, mesh=mesh, in_specs=in_specs,
                            out_specs=out_specs, check_rep=False),
            donate_argnums=self.donate, keep_unused=True)

    def _dev(self, arr, sharded):
        """device_put with content-hash caching (skips re-upload on repeat calls)."""
        import hashlib, jax
        from jax.sharding import Mesh, PartitionSpec, NamedSharding
        h = hashlib.md5(arr.tobytes()).hexdigest()
        cached = self._devcache.get((arr.shape, h))
        if cached is not None:
            return cached
        if sharded:
            mesh = Mesh(np.asarray(jax.devices()[:self.n_cores]), ("core",))
            sh = NamedSharding(mesh, PartitionSpec("core"))
            d = jax.device_put(arr, sh)
        else:
            d = jax.device_put(arr, jax.devices()[0])
        self._devcache[(arr.shape, h)] = d
        return d

    def __call__(self, in_maps):
        if not hasattr(self, "_devcache"):
            self._devcache = {}
        nc_ = self.n_cores
        if nc_ == 1:
            args = [self._dev(np.asarray(in_maps[0][n]), False) for n in self.in_names]
            out = self.fn(*args, *self.zero_outs)
            return [{n: np.asarray(out[i]) for i, n in enumerate(self.out_names)}]
        if self.fn is None:
            flags = [all(in_maps[c][n] is in_maps[0][n] for c in range(nc_))
                     for n in self.in_names]
            self._build_multi(flags)
        args = []
        for i, n in enumerate(self.in_names):
            if self.shared_flags[i]:
                args.append(np.asarray(in_maps[0][n]))
            else:
                cat = np.concatenate([np.asarray(in_maps[c][n]) for c in range(nc_)], axis=0)
                args.append(self._dev(cat, True))
        concat_zeros = [np.zeros((nc_ * z.shape[0], *z.shape[1:]), z.dtype)
                        for z in self.zero_outs]
        out = self.fn(*args, *concat_zeros)
        res = []
        for c in range(nc_):
            res.append({n: np.asarray(out[i]).reshape(nc_, *self.out_avals[i].shape)[c]
                        for i, n in enumerate(self.out_names)})
        return res


# ---------------------------------------------------------------- entrypoint
def kernel(**inputs):
    if "A" not in _cache:
        _cache["A"] = _CachedRunner(build_A(), NC)
    if "B" not in _cache:
        _cache["B"] = _CachedRunner(build_B(), 1)
    in_maps, ei_sd = _prep_A(inputs)
    resA = _cache["A"](in_maps)
    inB = _prep_B(inputs, resA, ei_sd)
    resB = _cache["B"]([inB])
    return resB[0]["o_prob"].reshape(NCLS).astype(np.float32)
